# revision 1
# baseline (speedup 1.0000x reference)
"""Trainium2 Bass kernel for nn_CapsuleNeuralNetworkV2 (8 cores, data-parallel).

Math (per sample, 8 capsule iterations then decoder):
  v = h.reshape(4, 196)
  q = v @ W1.T + b1 ; k = v @ W2.T + b2 ; u = v @ W3.T + b3
  scores[t,s] = q_t . k_s  ->  softmax over s -> h'_t = sum_s P[t,s] u_s
  dec = relu(h Wd1.T + bd1) Wd2.T + bd2 ; out = softmax(dec Wo.T + bo)

Key restructuring (host-side algebra):
  scores[t,s] = v_t . z_s + r_s  where  z_s = G v_s + c, r_s = a.v_s + d,
  G = W1.T W2, a = W2.T b1, c = W1.T b2, d = b1.b2.
  Since softmax rows sum to 1, u's bias b3 passes through the combine
  unchanged, so u = W3 v + b3 is computed with the bias fused in the matmul.

On-chip layout: batch-major h tile [128, 4, 197] (slot-stride 197, col 196
of each slot is a constant 1.0 used as the matmul bias row after the PE
transpose). All matmuls in float32r (full-rate fp32 on the PE for N>=256).
"""

import numpy as np

import concourse.bass as bass
import concourse.tile as tile
from concourse import bacc, mybir
from concourse.bass import ds
from concourse.bass_utils import run_bass_kernel_spmd
from concourse.masks import make_identity

FR = mybir.dt.float32r
BF = mybir.dt.bfloat16
F32 = mybir.dt.float32
AF = mybir.ActivationFunctionType
ALU = mybir.AluOpType

B = 32768
NCORES = 8
P = 128
T = 4
FV = 196
FEAT = 784
SLOT = FV + 2  # 198: slot data + ones column + zero-pad (keeps STT FD even-ish and fuses r)


def _ap(t, dims, offset_elems=0):
    """Hand-built AP over a tile's tensor: dims = [[step, count], ...] in elements."""
    a = t[:] if hasattr(t, "tile") or not isinstance(t, bass.AP) else t
    return bass.AP(tensor=a.tensor, offset=a.offset + offset_elems, ap=dims)


def build(nsub=4, ngroups=8):
    """One NeuronCore program processing nsub*ngroups*128 samples."""
    bpc = nsub * ngroups * P
    nc = bacc.Bacc("TRN2", target_bir_lowering=False, debug=False)

    x_d = nc.dram_tensor("x", [bpc, FEAT], FR, kind="ExternalInput")
    zu_d = nc.dram_tensor("zu_w", [P, 2, 394], FR, kind="ExternalInput")
    d1_d = nc.dram_tensor("dec1_w", [P, 8, FEAT], FR, kind="ExternalInput")
    d2_d = nc.dram_tensor("dec2_w", [P, 7, FEAT], FR, kind="ExternalInput")
    ow_d = nc.dram_tensor("out_w", [P, 7, 10], FR, kind="ExternalInput")
    out_d = nc.dram_tensor("out", [bpc, 10], F32, kind="ExternalOutput")

    with tile.TileContext(nc) as tc:
        consts = tc.alloc_tile_pool(name="consts", bufs=1)
        hp = tc.alloc_tile_pool(name="h", bufs=3)
        wk = tc.alloc_tile_pool(name="wk", bufs=3)
        wkd = tc.alloc_tile_pool(name="wkd", bufs=1)
        sm = tc.alloc_tile_pool(name="small", bufs=6)
        pp = tc.alloc_tile_pool(name="ps", bufs=2, space="PSUM")
        zup = pp
        dp = pp

        ident_f = consts.tile([P, P], F32)
        make_identity(nc, ident_f)
        ident = consts.tile([P, P], FR)
        nc.vector.tensor_copy(ident, ident_f)
        ones_c = consts.tile([P, 512], F32)
        nc.vector.memset(ones_c, 1.0)
        zu_w = consts.tile([P, 2, 394], FR)
        nc.sync.dma_start(out=zu_w, in_=zu_d[:, :, :])
        d1_w = consts.tile([P, 8, FEAT], FR)
        nc.sync.dma_start(out=d1_w, in_=d1_d[:, :, :])
        d2_w = consts.tile([P, 7, FEAT], FR)
        nc.sync.dma_start(out=d2_w, in_=d2_d[:, :, :])
        ow_w = consts.tile([P, 7, 10], FR)
        nc.sync.dma_start(out=ow_w, in_=ow_d[:, :, :])

        def capsule_iter(h_cur, h_nxt, j):
            """One capsule-attention iteration: h_nxt <- attn(h_cur)."""
            hb = wk.tile([P, T, SLOT], BF, tag="hb")
            nc.gpsimd.tensor_copy(hb, h_cur)
            # --- PE transposes: batch-major h -> feature-major V.T chunks ---
            vt1_ps = pp.tile([P, T, P], FR, tag="vt1ps")
            vt2_ps = pp.tile([69, T, P], FR, tag="vt2ps")
            for t in range(T):
                nc.tensor.transpose(vt1_ps[:, t, :], h_cur[:, t, 0:P], ident)
                # includes the ones column -> row 68 of the chunk is 1.0
                nc.tensor.transpose(vt2_ps[:, t, :], h_cur[:, t, P : P + 69], ident)
            vt1 = wk.tile([P, T, P], FR, tag="vt1")
            vt2 = wk.tile([69, T, P], FR, tag="vt2")
            nc.scalar.copy(vt1, vt1_ps)
            nc.scalar.copy(vt2, vt2_ps)

            u_sb = wk.tile([P, T, FV], FR, tag="usb")
            zb = wk.tile([P, T, SLOT], BF, tag="zb")
            dots = sm.tile([P, T, T], F32, tag="dots")
            scratch = sm.tile([P, SLOT], BF, tag="scr")

            for s in range(T):
                # z|r|u fused matmul for slot s: [128, 393] PSUM
                zu_ps = zup.tile([P, 394], F32, tag="zups")
                nc.tensor.matmul(zu_ps, vt1[:, s, :], zu_w[:, 0, :],
                                 start=True, stop=False)
                nc.tensor.matmul(zu_ps, vt2[:, s, :], zu_w[0:69, 1, :],
                                 start=False, stop=True)
                # evacuate u (fp32) and z|r|pad (bf16 for the dots)
                nc.scalar.copy(u_sb[:, s, :], zu_ps[:, 198:394])
                nc.scalar.copy(zb[:, s, :], zu_ps[:, 0:198])
                for t in range(T):
                    nc.vector.scalar_tensor_tensor(
                        out=scratch,
                        in0=hb[:, t, :],
                        scalar=1.0,
                        in1=zb[:, s, :],
                        op0=ALU.mult,
                        op1=ALU.mult,
                        accum_out=dots[:, t, s : s + 1],
                    )

            # softmax over s (no max subtraction; |scores| stays < 30)
            e = sm.tile([P, T, T], F32, tag="e")
            nc.scalar.activation(e, dots, AF.Exp)
            sums = sm.tile([P, T], F32, tag="sums")
            nc.vector.reduce_sum(sums, e, axis=mybir.AxisListType.X)
            rec = sm.tile([P, T], F32, tag="rec")
            nc.vector.reciprocal(rec, sums)
            probs = sm.tile([P, T, T], F32, tag="probs")
            nc.vector.scalar_tensor_tensor(
                out=probs, in0=e, scalar=1.0,
                in1=_ap(rec, [rec[:].ap[0], [1, T], [0, T]]),
                op0=ALU.mult, op1=ALU.mult,
            )

            # ones column for the next h
            nc.gpsimd.tensor_copy(h_nxt[:, :, 196:198], ones_c[:, 0 : 2 * T])
            # combine: h'_t = sum_s P[t,s] * u_s
            # chains t=0..2 on DVE (Pool seeds s=0); chain t=3 fully on Pool
            for t in range(3):
                nc.gpsimd.tensor_scalar_mul(
                    h_nxt[:, t, 0:FV], u_sb[:, 0, :], probs[:, t, 0:1]
                )
                for s in range(1, T):
                    nc.vector.scalar_tensor_tensor(
                        out=h_nxt[:, t, 0:FV],
                        in0=u_sb[:, s, :],
                        scalar=probs[:, t, s : s + 1],
                        in1=h_nxt[:, t, 0:FV],
                        op0=ALU.mult,
                        op1=ALU.add,
                    )
            pc_t = wk.tile([P, FV], F32, tag="pct")
            nc.gpsimd.tensor_scalar_mul(
                h_nxt[:, 3, 0:FV], u_sb[:, 0, :], probs[:, 3, 0:1]
            )
            for s in range(1, T):
                nc.gpsimd.tensor_scalar_mul(
                    pc_t, u_sb[:, s, :], probs[:, 3, s : s + 1]
                )
                nc.gpsimd.tensor_add(
                    h_nxt[:, 3, 0:FV], h_nxt[:, 3, 0:FV], pc_t
                )

        def decoder(hs, g):
            """Decoder over nsub tiles (N = nsub*128 wide matmuls)."""
            W = nsub * P
            # h.T chunks, slot-major: [128] x4 and [69] x4 (with ones row)
            ht1 = wkd.tile([P, T, W], FR, tag="ht1")
            ht2 = wkd.tile([69, T, W], FR, tag="ht2")
            for t in range(T):
                t1_ps = dp.tile([P, W], FR, tag="vt1ps")
                t2_ps = dp.tile([69, W], FR, tag="vt2ps")
                for j in range(nsub):
                    nc.tensor.transpose(
                        t1_ps[:, j * P : (j + 1) * P], hs[j][:, t, 0:P], ident
                    )
                    nc.tensor.transpose(
                        t2_ps[:, j * P : (j + 1) * P], hs[j][:, t, P : P + 69], ident
                    )
                nc.scalar.copy(ht1[:, t, :], t1_ps)
                nc.vector.tensor_copy(ht2[:, t, :], t2_ps)

            # dec1 = relu(Wd1 @ h.T + bd1), feature-major, 7 M-chunks
            d1a = wkd.tile([P, 6, W], FR, tag="d1a")
            d1b = wkd.tile([17, W], FR, tag="d1b")
            nc.vector.tensor_copy(d1b, ones_c[0:17, 0:W])
            for m in range(7):
                mw = min(P, FEAT - m * P)
                mp = dp.tile([P, W], F32, tag="zups")
                msl = slice(m * P, m * P + mw)
                for t in range(T):
                    nc.tensor.matmul(mp[0:mw, :], d1_w[:, t, msl], ht1[:, t, :],
                                     start=(t == 0), stop=False)
                for t in range(T):
                    nc.tensor.matmul(mp[0:mw, :], d1_w[0:69, 4 + t, msl],
                                     ht2[:, t, :], start=False, stop=(t == 3))
                if m < 6:
                    nc.scalar.activation(d1a[:, m, :], mp, AF.Relu)
                else:
                    nc.scalar.activation(d1b[0:16, :], mp[0:16, :], AF.Relu)

            # dec2 = Wd2 @ relu1 + bd2, feature-major
            d2a = wkd.tile([P, 6, W], FR, tag="d2a")
            d2b = wkd.tile([17, W], FR, tag="d2b")
            nc.vector.tensor_copy(d2b, ones_c[0:17, 0:W])
            for m in range(7):
                mw = min(P, FEAT - m * P)
                mp = dp.tile([P, W], F32, tag="zups")
                msl = slice(m * P, m * P + mw)
                for c in range(6):
                    nc.tensor.matmul(mp[0:mw, :], d2_w[:, c, msl], d1a[:, c, :],
                                     start=(c == 0), stop=False)
                nc.tensor.matmul(mp[0:mw, :], d2_w[0:17, 6, msl], d1b,
                                 start=False, stop=True)
                if m < 6:
                    nc.scalar.copy(d2a[:, m, :], mp)
                else:
                    nc.scalar.copy(d2b[0:16, :], mp[0:16, :])

            # logits + softmax per subtile
            for j in range(nsub):
                jsl = slice(j * P, (j + 1) * P)
                lg = dp.tile([P, 10], F32, tag="zups")
                for c in range(6):
                    nc.tensor.matmul(lg, d2a[:, c, jsl], ow_w[:, c, :],
                                     start=(c == 0), stop=False)
                nc.tensor.matmul(lg, d2b[:, jsl], ow_w[0:17, 6, :],
                                 start=False, stop=True)
                mx = sm.tile([P, 1], F32, tag="mx")
                nc.vector.reduce_max(mx, lg, axis=mybir.AxisListType.X)
                nmx = sm.tile([P, 1], F32, tag="nmx")
                nc.vector.tensor_scalar_mul(nmx, mx, -1.0)
                e10 = sm.tile([P, 10], F32, tag="e10")
                s10 = sm.tile([P, 1], F32, tag="s10")
                nc.scalar.activation(e10, lg, AF.Exp, bias=nmx, accum_out=s10)
                r10 = sm.tile([P, 1], F32, tag="r10")
                nc.vector.reciprocal(r10, s10)
                o10 = sm.tile([P, 10], F32, tag="o10")
                nc.vector.tensor_scalar_mul(o10, e10, r10)
                nc.sync.dma_start(
                    out=out_d[ds(g * (nsub * P) + j * P, P), :], in_=o10
                )

        def body(g):
            hs = []
            for j in range(nsub):
                h0 = hp.tile([P, T, SLOT], FR, tag=f"h{j}")
                nc.sync.dma_start(
                    out=h0[:, :, 0:FV],
                    in_=x_d[ds(g * (nsub * P) + j * P, P), :].rearrange(
                        "p (t f) -> p t f", t=T
                    ),
                )
                nc.gpsimd.tensor_copy(h0[:, :, 196:198], ones_c[:, 0 : 2 * T])
                hs.append(h0)
            for it in range(8):
                for j in range(nsub):
                    h_nxt = hp.tile([P, T, SLOT], FR, tag=f"h{j}")
                    capsule_iter(hs[j], h_nxt, j)
                    hs[j] = h_nxt
            decoder(hs, g)

        if ngroups == 1:
            body(0)
        else:
            with tc.For_i(0, ngroups, 1) as g:
                body(g)
        for _pool in (pp, sm, wkd, wk, hp, consts):
            _pool.release()

    nc.compile()
    return nc


def pack_weights(W1, b1, W2, b2, W3, b3, Wd1, bd1, Wd2, bd2, Wo, bo):
    f64 = np.float64
    W1, b1, W2, b2, W3, b3 = (np.asarray(t, f64) for t in (W1, b1, W2, b2, W3, b3))
    G = W1.T @ W2
    a = W2.T @ b1
    c = W1.T @ b2
    d = float(b1 @ b2)

    zu = np.zeros((P, 2, 394), np.float32)
    full = np.zeros((197, 394), f64)
    full[:196, :196] = G.T
    full[:196, 196] = a
    full[:196, 198:] = W3.T
    full[196, :196] = c
    full[196, 196] = d
    full[196, 198:] = b3
    zu[:, 0, :] = full[0:128]
    zu[0:69, 1, :] = full[128:197]

    d1 = np.zeros((P, 8, FEAT), np.float32)
    W1T = np.asarray(Wd1, f64).T  # [784 f_in, 784 j]
    for t in range(T):
        d1[:, t, :] = W1T[t * FV : t * FV + P, :]
        d1[0:68, 4 + t, :] = W1T[t * FV + P : (t + 1) * FV, :]
    d1[68, 4, :] = np.asarray(bd1, f64)

    d2 = np.zeros((P, 7, FEAT), np.float32)
    W2T = np.asarray(Wd2, f64).T
    for cidx in range(6):
        d2[:, cidx, :] = W2T[cidx * P : (cidx + 1) * P, :]
    d2[0:16, 6, :] = W2T[768:784, :]
    d2[16, 6, :] = np.asarray(bd2, f64)

    ow = np.zeros((P, 7, 10), np.float32)
    WoT = np.asarray(Wo, f64).T
    for cidx in range(6):
        ow[:, cidx, :] = WoT[cidx * P : (cidx + 1) * P, :]
    ow[0:16, 6, :] = WoT[768:784, :]
    ow[16, 6, :] = np.asarray(bo, f64)
    return zu, d1, d2, ow


_NC_CACHE = {}


def kernel(**inputs):
    x = np.ascontiguousarray(np.asarray(inputs["x"], np.float32))
    zu, d1, d2, ow = pack_weights(
        inputs["W1"], inputs["b1"], inputs["W2"], inputs["b2"], inputs["W3"],
        inputs["b3"], inputs["Wd1"], inputs["bd1"], inputs["Wd2"],
        inputs["bd2"], inputs["Wo"], inputs["bo"],
    )
    if "nc" not in _NC_CACHE:
        _NC_CACHE["nc"] = build(4, 8)
    nc = _NC_CACHE["nc"]
    bpc = B // NCORES
    in_maps = [
        {
            "x": x[c * bpc : (c + 1) * bpc],
            "zu_w": zu,
            "dec1_w": d1,
            "dec2_w": d2,
            "out_w": ow,
        }
        for c in range(NCORES)
    ]
    res = run_bass_kernel_spmd(nc, in_maps, core_ids=list(range(NCORES)))
    return np.concatenate([res.results[c]["out"] for c in range(NCORES)], axis=0)



# revision 11
# speedup vs baseline: 1.7526x; 1.7526x over previous
"""Trainium2 Bass kernel for nn_CapsuleNeuralNetworkV2 (8 cores, data-parallel).

Reference math (per sample, 8 capsule iterations then decoder):
  v = h.reshape(4, 196); q,k,u = affine(v); scores = q k^T;
  P = softmax(scores); h' = P u;  dec = relu(h Wd1^T+bd1) Wd2^T+bd2;
  out = softmax(dec Wo^T + bo).

Restructuring (host-side algebra):
  Since each P has rows summing to 1, the state stays in the span of the 4
  initial slots: v^(k) = W3^k w^(k) + m_k with w^(k) = C^(k) V (C is a
  per-sample 4x4 convex-coefficient matrix, V the initial slots).
  scores^(k)[t,s] = C[t] M_k C[s]^T (mod per-t constants that cancel in
  softmax), where M_k[i,j] = v_i.(G_k v_j) + a_k.v_j depends only on the
  INITIAL slots: G_k = (W3^k)^T G W3^k, G = W1^T W2,
  a_k = (W3^k)^T (G^T m_k + W2^T b1).  G_k is numerically low-rank for k>=1
  (powers of a random matrix), so M_k is computed from rank-r_k SVD
  projections p_i = U_r^T v_i, q_j = (S V_r^T) v_j: M[i,j] ~ p_i.q_j + r_j.
  Per iteration only the tiny 4x4 chain is sequential:
  scores = C M C^T -> softmax -> C' = P C.  All projections/M_k are
  C-independent and pipeline on PE/Act/DVE ahead of the chain.
  Final w^(8) = C^(8) V; W3^8/m_8 are folded into Wd1/bd1 on the host.

Engines: PE transposes V once per tile + small bf16 projection matmuls +
decoder; DVE/Pool share the per-sample dot products and the 4x4 chain; Act
does PSUM evacuation, exp, and decoder activations.
"""

import numpy as np
import ml_dtypes

import concourse.bass as bass
import concourse.tile as tile
from concourse import bacc, mybir
from concourse.bass import ds
from concourse.bass_utils import run_bass_kernel_spmd
from concourse.masks import make_identity

FR = mybir.dt.float32r
BF = mybir.dt.bfloat16
F32 = mybir.dt.float32
AF = mybir.ActivationFunctionType
ALU = mybir.AluOpType

B = 32768
NCORES = 8
P = 128
T = 4
FV = 196
FEAT = 784
SLOT = 198  # h slot: 196 data + ones col (196) + spare (197)

RANKS = [196, 64, 48, 32, 24, 16, 12, 8]
NCOLS = [197] + [2 * (r + 1) for r in RANKS[1:]]  # proj cols per slot per k
POFF = [0]
for _n in NCOLS:
    POFF.append(POFF[-1] + _n)
PTOT = POFF[-1]


def _ap(t, dims, offset_elems=0):
    """Hand-built AP over a tile's tensor: dims = [[step, count], ...]."""
    a = t[:] if hasattr(t, "tile") or not isinstance(t, bass.AP) else t
    return bass.AP(tensor=a.tensor, offset=a.offset + offset_elems, ap=dims)


def build(nsub=4, ngroups=8):
    """One NeuronCore program processing nsub*ngroups*128 samples."""
    bpc = nsub * ngroups * P
    nc = bacc.Bacc("TRN2", target_bir_lowering=False, debug=False)

    x_d = nc.dram_tensor("x", [bpc, FEAT], FR, kind="ExternalInput")
    pw_d = nc.dram_tensor("zu_w", [P, 2, PTOT], BF, kind="ExternalInput")
    d1_d = nc.dram_tensor("dec1_w", [P, 8, FEAT], FR, kind="ExternalInput")
    d2_d = nc.dram_tensor("dec2_w", [P, 7, FEAT], FR, kind="ExternalInput")
    ow_d = nc.dram_tensor("out_w", [P, 7, 10], FR, kind="ExternalInput")
    out_d = nc.dram_tensor("out", [bpc, 10], F32, kind="ExternalOutput")

    with tile.TileContext(nc) as tc:
        consts = tc.alloc_tile_pool(name="consts", bufs=1)
        hp = tc.alloc_tile_pool(name="h", bufs=2)
        vp = tc.alloc_tile_pool(name="vt", bufs=2)
        pkp = tc.alloc_tile_pool(name="pk", bufs=3)
        scp = tc.alloc_tile_pool(name="scr", bufs=4)
        mtp = tc.alloc_tile_pool(name="mt", bufs=8)
        sm = tc.alloc_tile_pool(name="small", bufs=3)
        wp = tc.alloc_tile_pool(name="w", bufs=2)
        wkd = tc.alloc_tile_pool(name="wkd", bufs=1)
        pp = tc.alloc_tile_pool(name="ps", bufs=2, space="PSUM")

        ident_f = consts.tile([P, P], F32)
        make_identity(nc, ident_f)
        ident_b = consts.tile([P, P], BF)
        nc.vector.tensor_copy(ident_b, ident_f)
        ident_r = consts.tile([P, P], FR)
        nc.vector.tensor_copy(ident_r, ident_f)
        ones_c = consts.tile([P, 512], F32)
        nc.vector.memset(ones_c, 1.0)
        pw = consts.tile([P, 2, PTOT], BF)
        nc.sync.dma_start(out=pw, in_=pw_d[:, :, :])
        d1_w = consts.tile([P, 8, FEAT], FR)
        nc.sync.dma_start(out=d1_w, in_=d1_d[:, :, :])
        d2_w = consts.tile([P, 7, FEAT], FR)
        nc.sync.dma_start(out=d2_w, in_=d2_d[:, :, :])
        ow_w = consts.tile([P, 7, 10], FR)
        nc.sync.dma_start(out=ow_w, in_=ow_d[:, :, :])

        def load_tile(g, j):
            h0 = hp.tile([P, T, SLOT], FR, tag=f"h{j}")
            nc.sync.dma_start(
                out=h0[:, :, 0:FV],
                in_=x_d[ds(g * (nsub * P) + j * P, P), :].rearrange(
                    "p (t f) -> p t f", t=T
                ),
            )
            nc.gpsimd.tensor_copy(h0[:, :, 196:198], ones_c[:, 0 : 2 * T])
            hb = hp.tile([P, T, SLOT], BF, tag=f"hb{j}", bufs=1)
            nc.gpsimd.tensor_copy(hb, h0)
            vt1 = vp.tile([P, T, P], BF, tag=f"vt1{j}")
            vt2 = vp.tile([69, T, P], BF, tag=f"vt2{j}")
            t1_ps = pp.tile([P, T, P], BF, tag="t1ps", bufs=1)
            t2_ps = pp.tile([69, T, P], BF, tag="t2ps", bufs=1)
            for t in range(T):
                nc.tensor.transpose(t1_ps[:, t, :], hb[:, t, 0:P], ident_b)
                nc.tensor.transpose(t2_ps[:, t, :], hb[:, t, P : P + 69], ident_b)
            nc.scalar.copy(vt1, t1_ps)
            nc.scalar.copy(vt2, t2_ps)
            return h0, hb, vt1, vt2

        def proj(j, k, vt1, vt2):
            """PE projections for iteration k -> pk [128, 4, nc] bf16."""
            nco = NCOLS[k]
            off = POFF[k]
            pk = pkp.tile([P, T, 197], BF, tag=f"pk{j}")
            for half in range(2):
                ps = pp.tile([P, 2, 197], F32, tag="pkps")
                for sl in range(2):
                    s = half * 2 + sl
                    nc.tensor.matmul(
                        ps[:, sl, 0:nco], vt1[:, s, :],
                        pw[:, 0, off : off + nco], start=True, stop=False)
                    nc.tensor.matmul(
                        ps[:, sl, 0:nco], vt2[0:69, s, :],
                        pw[0:69, 1, off : off + nco], start=False, stop=True)
                nc.scalar.copy(
                    pk[:, 2 * half : 2 * half + 2, 0:nco], ps[:, :, 0:nco])
            return pk

        def dots(j, k, hb, pk):
            """M_k[i,j] for all 16 slot pairs -> Mt [128, 4, 4] f32."""
            mt = mtp.tile([P, T, T], F32, tag=f"mt{j}")
            if k <= 1:
                # stt with accumulate; split across DVE and Pool
                if k == 0:
                    ncols, i_off = 197, None  # in0 = hb slots (v | 1)
                else:
                    r1 = RANKS[k] + 1
                    ncols, i_off = r1, r1
                scr = scp.tile([P, 256], BF, tag="scr197", bufs=6)
                for idx in range(16):
                    i, jj = idx // 4, idx % 4
                    if k == 0:
                        in0 = hb[:, i, 0:197]
                    else:
                        in0 = _ap(pk, [pk[:].ap[0], [1, ncols]],
                                  offset_elems=i * 197 + i_off)
                    in1 = _ap(pk, [pk[:].ap[0], [1, ncols]],
                              offset_elems=jj * 197)
                    eng = nc.vector
                    eng.scalar_tensor_tensor(
                        out=scr[:, 0:ncols], in0=in0, scalar=1.0, in1=in1,
                        op0=ALU.mult, op1=ALU.mult,
                        accum_out=mt[:, i, jj : jj + 1])
            else:
                r1 = RANKS[k] + 1
                scr = scp.tile([P, T, T, 65], BF, tag="scr")
                pap = pk[:].ap[0]
                in0 = _ap(pk, [pap, [197, 4], [0, 4], [1, r1]],
                          offset_elems=r1)
                in1 = _ap(pk, [pap, [0, 4], [197, 4], [1, r1]])
                nc.vector.tensor_tensor(
                    out=scr[:, :, :, 0:r1], in0=in0, in1=in1, op=ALU.mult)
                nc.vector.tensor_reduce(
                    out=mt, in_=scr[:, :, :, 0:r1], axis=mybir.AxisListType.X,
                    op=ALU.add)
            return mt

        def serial_step(j, k, mt, c_prev):
            """scores = C mt C^T -> softmax -> C' = P C. Returns C'."""
            if k == 0:
                s_t = mt
            else:
                cap = c_prev[:].ap[0]
                map_ = mt[:].ap[0]
                scrd = scp.tile([P, T, T, T], F32, tag="scrd")
                nc.gpsimd.tensor_tensor(  # D[i,s] = sum_j mt[i,j] C[s,j]
                    out=scrd,
                    in0=_ap(mt, [map_, [4, 4], [0, 4], [1, 4]]),
                    in1=_ap(c_prev, [cap, [0, 4], [4, 4], [1, 4]]),
                    op=ALU.mult)
                dm = sm.tile([P, T, T], F32, tag=f"d{j}")
                nc.vector.tensor_reduce(
                    out=dm, in_=scrd, axis=mybir.AxisListType.X, op=ALU.add)
                scrd2 = scp.tile([P, T, T, T], F32, tag="scrd")
                nc.gpsimd.tensor_tensor(  # S[t,s] = sum_i C[t,i] D[i,s]
                    out=scrd2,
                    in0=_ap(c_prev, [cap, [4, 4], [0, 4], [1, 4]]),
                    in1=_ap(dm, [dm[:].ap[0], [0, 4], [1, 4], [4, 4]]),
                    op=ALU.mult)
                s_t = sm.tile([P, T, T], F32, tag=f"s{j}")
                nc.vector.tensor_reduce(
                    out=s_t, in_=scrd2, axis=mybir.AxisListType.X, op=ALU.add)
            e = sm.tile([P, T, T], F32, tag=f"e{j}")
            nc.scalar.activation(e, s_t, AF.Exp)
            sums = sm.tile([P, T], F32, tag=f"su{j}")
            nc.vector.reduce_sum(sums, e, axis=mybir.AxisListType.X)
            rec = sm.tile([P, T], F32, tag=f"re{j}")
            nc.vector.reciprocal(rec, sums)
            pr = sm.tile([P, T, T], F32, tag=f"pr{j}")
            nc.vector.scalar_tensor_tensor(
                out=pr, in0=e, scalar=1.0,
                in1=_ap(rec, [rec[:].ap[0], [1, T], [0, T]]),
                op0=ALU.mult, op1=ALU.mult)
            if k == 0:
                return pr
            c_new = sm.tile([P, T, T], F32, tag=f"c{j}", bufs=2)
            scrd3 = scp.tile([P, T, T, T], F32, tag="scrd")
            nc.gpsimd.tensor_tensor(  # C'[t,jj] = sum_s P[t,s] C[s,jj]
                out=scrd3,
                in0=_ap(pr, [pr[:].ap[0], [4, 4], [0, 4], [1, 4]]),
                in1=_ap(c_prev, [c_prev[:].ap[0], [0, 4], [1, 4], [4, 4]]),
                op=ALU.mult)
            nc.vector.tensor_reduce(
                out=c_new, in_=scrd3, axis=mybir.AxisListType.X, op=ALU.add)
            return c_new

        def recon(j, h0, c8):
            """w[:, t, :] = sum_s C8[t,s] * h0[:, s, :] (ones col rides along)."""
            w = wp.tile([P, T, SLOT], FR, tag=f"w{j}", bufs=1)
            for t in range(T):
                nc.scalar.activation(
                    w[:, t, :], h0[:, 0, :], AF.Copy,
                    scale=c8[:, t, 0:1])
            for t in range(3):
                for s in range(1, T):
                    nc.vector.scalar_tensor_tensor(
                        out=w[:, t, :], in0=h0[:, s, :],
                        scalar=c8[:, t, s : s + 1], in1=w[:, t, :],
                        op0=ALU.mult, op1=ALU.add)
            pct = wp.tile([P, SLOT], F32, tag="pct", bufs=2)
            for s in range(1, T):
                nc.gpsimd.tensor_scalar_mul(
                    pct, h0[:, s, :], c8[:, 3, s : s + 1])
                nc.gpsimd.tensor_add(w[:, 3, :], w[:, 3, :], pct)
            return w

        def decoder(ws, g):
            """Decoder over nsub tiles (N = nsub*128 wide matmuls)."""
            W = nsub * P
            ht1 = wkd.tile([P, T, W], FR, tag="ht1")
            ht2 = wkd.tile([69, T, W], FR, tag="ht2")
            for t in range(T):
                t1_ps = pp.tile([P, W], FR, tag="dt1ps", bufs=1)
                t2_ps = pp.tile([69, W], FR, tag="dt2ps", bufs=1)
                for j in range(nsub):
                    nc.tensor.transpose(
                        t1_ps[:, j * P : (j + 1) * P], ws[j][:, t, 0:P], ident_r
                    )
                    nc.tensor.transpose(
                        t2_ps[:, j * P : (j + 1) * P], ws[j][:, t, P : P + 69],
                        ident_r
                    )
                nc.scalar.copy(ht1[:, t, :], t1_ps)
                nc.vector.tensor_copy(ht2[:, t, :], t2_ps)

            # dec1 = relu(Wd1~ @ w.T + bd1~), feature-major, 7 M-chunks
            d1a = wkd.tile([P, 6, W], FR, tag="d1a")
            d1b = wkd.tile([17, W], FR, tag="d1b")
            nc.vector.tensor_copy(d1b, ones_c[0:17, 0:W])
            for m in range(7):
                mw = min(P, FEAT - m * P)
                mp = pp.tile([P, W], F32, tag="mp")
                msl = slice(m * P, m * P + mw)
                for t in range(T):
                    nc.tensor.matmul(mp[0:mw, :], d1_w[:, t, msl], ht1[:, t, :],
                                     start=(t == 0), stop=False)
                for t in range(T):
                    nc.tensor.matmul(mp[0:mw, :], d1_w[0:69, 4 + t, msl],
                                     ht2[:, t, :], start=False, stop=(t == 3))
                if m < 6:
                    nc.scalar.activation(d1a[:, m, :], mp, AF.Relu)
                else:
                    nc.scalar.activation(d1b[0:16, :], mp[0:16, :], AF.Relu)

            # dec2 = Wd2 @ relu1 + bd2, feature-major
            d2a = wkd.tile([P, 6, W], FR, tag="d2a")
            d2b = wkd.tile([17, W], FR, tag="d2b")
            nc.vector.tensor_copy(d2b, ones_c[0:17, 0:W])
            for m in range(7):
                mw = min(P, FEAT - m * P)
                mp = pp.tile([P, W], F32, tag="mp")
                msl = slice(m * P, m * P + mw)
                for c in range(6):
                    nc.tensor.matmul(mp[0:mw, :], d2_w[:, c, msl], d1a[:, c, :],
                                     start=(c == 0), stop=False)
                nc.tensor.matmul(mp[0:mw, :], d2_w[0:17, 6, msl], d1b,
                                 start=False, stop=True)
                if m < 6:
                    nc.scalar.copy(d2a[:, m, :], mp)
                else:
                    nc.scalar.copy(d2b[0:16, :], mp[0:16, :])

            # logits + softmax per subtile
            for j in range(nsub):
                jsl = slice(j * P, (j + 1) * P)
                lgt = pp.tile([P, W], F32, tag="mp")
                lg = lgt[:, 0:10]
                for c in range(6):
                    nc.tensor.matmul(lg, d2a[:, c, jsl], ow_w[:, c, :],
                                     start=(c == 0), stop=False)
                nc.tensor.matmul(lg, d2b[:, jsl], ow_w[0:17, 6, :],
                                 start=False, stop=True)
                mx = sm.tile([P, 1], F32, tag="mx")
                nc.vector.reduce_max(mx, lg, axis=mybir.AxisListType.X)
                nmx = sm.tile([P, 1], F32, tag="nmx")
                nc.vector.tensor_scalar_mul(nmx, mx, -1.0)
                e10 = sm.tile([P, 10], F32, tag="e10")
                s10 = sm.tile([P, 1], F32, tag="s10")
                nc.scalar.activation(e10, lg, AF.Exp, bias=nmx, accum_out=s10)
                r10 = sm.tile([P, 1], F32, tag="r10")
                nc.vector.reciprocal(r10, s10)
                o10 = sm.tile([P, 10], F32, tag="o10")
                nc.vector.tensor_scalar_mul(o10, e10, r10)
                nc.sync.dma_start(
                    out=out_d[ds(g * (nsub * P) + j * P, P), :], in_=o10
                )

        def body(g):
            h0s, hbs, cs, ws = [], [], [None] * nsub, []
            vts = []
            for j in range(nsub):
                h0, hb, vt1, vt2 = load_tile(g, j)
                h0s.append(h0)
                hbs.append(hb)
                vts.append((vt1, vt2))
            for k in range(8):
                mts = []
                for j in range(nsub):
                    pk = proj(j, k, *vts[j])
                    mts.append(dots(j, k, hbs[j], pk))
                for j in range(nsub):
                    cs[j] = serial_step(j, k, mts[j], cs[j])
            for j in range(nsub):
                ws.append(recon(j, h0s[j], cs[j]))
            decoder(ws, g)

        if ngroups == 1:
            body(0)
        else:
            with tc.For_i(0, ngroups, 1) as g:
                body(g)
        for _pool in (pp, wkd, wp, sm, mtp, scp, pkp, vp, hp, consts):
            _pool.release()

    nc.compile()
    return nc


def pack_weights(W1, b1, W2, b2, W3, b3, Wd1, bd1, Wd2, bd2, Wo, bo):
    f64 = np.float64
    W1, b1, W2, b2, W3, b3 = (np.asarray(t, f64) for t in (W1, b1, W2, b2, W3, b3))
    G = W1.T @ W2
    a = W2.T @ b1

    A = np.eye(FV)
    m = np.zeros(FV)
    pw = np.zeros((P, 2, PTOT), np.float32)
    for k in range(8):
        Gk = A.T @ G @ A
        ak = A.T @ (G.T @ m + a)
        nco = NCOLS[k]
        Wk = np.zeros((197, nco), f64)
        if k == 0:
            Wk[:FV, :FV] = Gk.T
            Wk[:FV, FV] = ak
        else:
            r = RANKS[k]
            r1 = r + 1
            U, S, Vh = np.linalg.svd(Gk)
            Wk[:FV, :r] = (np.diag(S[:r]) @ Vh[:r]).T
            Wk[:FV, r] = ak
            Wk[:FV, r1 : r1 + r] = U[:, :r]
            Wk[FV, r1 + r] = 1.0
        off = POFF[k]
        pw[:, 0, off : off + nco] = Wk[0:128]
        pw[0:69, 1, off : off + nco] = Wk[128:197]
        A = W3 @ A
        m = W3 @ m + b3
    A8, m8 = A, m

    # fold W3^8 / m8 into the first decoder layer
    BD = np.zeros((FEAT, FEAT), f64)
    mm = np.zeros(FEAT, f64)
    for t in range(T):
        BD[t * FV : (t + 1) * FV, t * FV : (t + 1) * FV] = A8
        mm[t * FV : (t + 1) * FV] = m8
    Wd1f = np.asarray(Wd1, f64) @ BD
    bd1f = np.asarray(bd1, f64) + np.asarray(Wd1, f64) @ mm

    d1 = np.zeros((P, 8, FEAT), np.float32)
    W1T = Wd1f.T  # [784 f_in, 784 j]
    for t in range(T):
        d1[:, t, :] = W1T[t * FV : t * FV + P, :]
        d1[0:68, 4 + t, :] = W1T[t * FV + P : (t + 1) * FV, :]
    d1[68, 4, :] = bd1f

    d2 = np.zeros((P, 7, FEAT), np.float32)
    W2T = np.asarray(Wd2, f64).T
    for cidx in range(6):
        d2[:, cidx, :] = W2T[cidx * P : (cidx + 1) * P, :]
    d2[0:16, 6, :] = W2T[768:784, :]
    d2[16, 6, :] = np.asarray(bd2, f64)

    ow = np.zeros((P, 7, 10), np.float32)
    WoT = np.asarray(Wo, f64).T
    for cidx in range(6):
        ow[:, cidx, :] = WoT[cidx * P : (cidx + 1) * P, :]
    ow[0:16, 6, :] = WoT[768:784, :]
    ow[16, 6, :] = np.asarray(bo, f64)
    return pw.astype(ml_dtypes.bfloat16), d1, d2, ow


_NC_CACHE = {}


def kernel(**inputs):
    x = np.ascontiguousarray(np.asarray(inputs["x"], np.float32))
    zu, d1, d2, ow = pack_weights(
        inputs["W1"], inputs["b1"], inputs["W2"], inputs["b2"], inputs["W3"],
        inputs["b3"], inputs["Wd1"], inputs["bd1"], inputs["Wd2"],
        inputs["bd2"], inputs["Wo"], inputs["bo"],
    )
    if "nc" not in _NC_CACHE:
        _NC_CACHE["nc"] = build(4, 8)
    nc = _NC_CACHE["nc"]
    bpc = B // NCORES
    in_maps = [
        {
            "x": x[c * bpc : (c + 1) * bpc],
            "zu_w": zu,
            "dec1_w": d1,
            "dec2_w": d2,
            "out_w": ow,
        }
        for c in range(NCORES)
    ]
    res = run_bass_kernel_spmd(nc, in_maps, core_ids=list(range(NCORES)))
    return np.concatenate([res.results[c]["out"] for c in range(NCORES)], axis=0)


# revision 38
# speedup vs baseline: 2.2673x; 1.2937x over previous
"""Trainium2 Bass kernel for nn_CapsuleNeuralNetworkV2 (8 cores, data-parallel).

Reference math (per sample, 8 capsule iterations then decoder):
  v = h.reshape(4, 196); q,k,u = affine(v); scores = q k^T;
  P = softmax(scores); h' = P u;  dec = relu(h Wd1^T+bd1) Wd2^T+bd2;
  out = softmax(dec Wo^T + bo).

Restructuring (host-side algebra):
  Since each P has rows summing to 1, the state stays in the span of the 4
  initial slots: v^(k) = W3^k w^(k) + m_k with w^(k) = C^(k) V (C is a
  per-sample 4x4 convex-coefficient matrix, V the initial slots).
  scores^(k)[t,s] = C[t] M_k C[s]^T (mod per-t constants that cancel in
  softmax), where M_k[i,j] = v_i.(G_k v_j) + a_k.v_j depends only on the
  INITIAL slots: G_k = (W3^k)^T G W3^k, G = W1^T W2,
  a_k = (W3^k)^T (G^T m_k + W2^T b1).  G_k is numerically low-rank for k>=1
  (powers of a random matrix), so M_k is computed from rank-r_k SVD
  projections p_i = U_r^T v_i, q_j = (S V_r^T) v_j: M[i,j] ~ p_i.q_j + r_j.
  Per iteration only the tiny 4x4 chain is sequential:
  scores = C M C^T -> softmax -> C' = P C.  All projections/M_k are
  C-independent and pipeline on PE/Act/DVE ahead of the chain.
  Final w^(8) = C^(8) V; W3^8/m_8 are folded into Wd1/bd1 on the host.

Engines: PE transposes V once per tile + small bf16 projection matmuls +
decoder; DVE/Pool share the per-sample dot products and the 4x4 chain; Act
does PSUM evacuation, exp, and decoder activations.
"""

import numpy as np
import ml_dtypes

import concourse.bass as bass
import concourse.tile as tile
from concourse import bacc, mybir
from concourse.bass import ds
from concourse.bass_utils import run_bass_kernel_spmd
from concourse.masks import make_identity

FR = mybir.dt.float32r
BF = mybir.dt.bfloat16
F32 = mybir.dt.float32
AF = mybir.ActivationFunctionType
ALU = mybir.AluOpType

B = 32768
NCORES = 8
P = 128
T = 4
FV = 196
FEAT = 784
SLOT = 198  # h slot: 196 data + ones col (196) + spare (197)

RANKS = [126, 64, 48, 32, 24, 16, 12, 8]
NCOLS = [2 * (r + 1) for r in RANKS]  # proj cols per slot per k
POFF = [0]
for _n in NCOLS:
    POFF.append(POFF[-1] + _n)
PTOT = POFF[-1]
NCMAX = max(NCOLS)


def _ap(t, dims, offset_elems=0):
    """Hand-built AP over a tile's tensor: dims = [[step, count], ...]."""
    a = t[:] if hasattr(t, "tile") or not isinstance(t, bass.AP) else t
    return bass.AP(tensor=a.tensor, offset=a.offset + offset_elems, ap=dims)


def build(nsub=4, ngroups=8):
    """One NeuronCore program processing nsub*ngroups*128 samples."""
    bpc = nsub * ngroups * P
    nc = bacc.Bacc("TRN2", target_bir_lowering=False, debug=False)

    x_d = nc.dram_tensor("x", [bpc, FEAT], FR, kind="ExternalInput")
    pw_d = nc.dram_tensor("zu_w", [P, 2, PTOT], BF, kind="ExternalInput")
    d1_d = nc.dram_tensor("dec1_w", [P, 8, FEAT], FR, kind="ExternalInput")
    d2_d = nc.dram_tensor("dec2_w", [P, 7, FEAT], FR, kind="ExternalInput")
    ow_d = nc.dram_tensor("out_w", [P, 7, 10], FR, kind="ExternalInput")
    out_d = nc.dram_tensor("out", [bpc, 10], F32, kind="ExternalOutput")

    with tile.TileContext(nc) as tc:
        consts = tc.alloc_tile_pool(name="consts", bufs=1)
        hp = tc.alloc_tile_pool(name="h", bufs=2)
        vp = tc.alloc_tile_pool(name="vt", bufs=2)
        pkp = tc.alloc_tile_pool(name="pk", bufs=3)
        scp = tc.alloc_tile_pool(name="scr", bufs=4)
        mtp = tc.alloc_tile_pool(name="mt", bufs=8)
        sm = tc.alloc_tile_pool(name="small", bufs=3)
        wp = tc.alloc_tile_pool(name="w", bufs=2)
        wkd = tc.alloc_tile_pool(name="wkd", bufs=1)
        pp = tc.alloc_tile_pool(name="ps", bufs=2, space="PSUM")

        ident_f = consts.tile([P, P], F32)
        make_identity(nc, ident_f)
        ident_b = consts.tile([P, P], BF)
        nc.vector.tensor_copy(ident_b, ident_f)
        ident_r = consts.tile([P, P], FR)
        nc.vector.tensor_copy(ident_r, ident_f)
        ones_c = consts.tile([P, 512], F32)
        nc.vector.memset(ones_c, 1.0)
        pw = consts.tile([P, 2, PTOT], BF)
        nc.sync.dma_start(out=pw, in_=pw_d[:, :, :])
        # decoder weights DMA'd after group 0's x tiles (emitted in build
        # below) so the first group's compute isn't starved behind 6MB
        d1_w = consts.tile([P, 8, FEAT], FR)
        d2_w = consts.tile([P, 7, FEAT], FR)
        ow_w = consts.tile([P, 7, 10], FR)

        def load_tile(g, j):
            h0 = hp.tile([P, T, SLOT], FR, tag=f"h{j}")
            nc.sync.dma_start(
                out=h0[:, :, 0:FV],
                in_=x_d[ds(g * (nsub * P) + j * P, P), :].rearrange(
                    "p (t f) -> p t f", t=T
                ),
            )
            nc.gpsimd.tensor_copy(h0[:, :, 196:198], ones_c[:, 0 : 2 * T])
            hb = hp.tile([P, T, SLOT], BF, tag=f"hb{j}", bufs=1)
            nc.gpsimd.tensor_copy(hb, h0)
            vt1 = vp.tile([P, T, P], BF, tag=f"vt1{j}")
            vt2 = vp.tile([69, T, P], BF, tag=f"vt2{j}")
            t1_ps = pp.tile([P, T, P], BF, tag="t1ps", bufs=1)
            t2_ps = pp.tile([69, T, P], BF, tag="t2ps", bufs=1)
            for t in range(T):
                nc.tensor.transpose(t1_ps[:, t, :], hb[:, t, 0:P], ident_b)
                nc.tensor.transpose(t2_ps[:, t, :], hb[:, t, P : P + 69], ident_b)
            nc.vector.tensor_copy(vt1, t1_ps)
            nc.scalar.copy(vt2, t2_ps)
            return h0, hb, vt1, vt2

        def proj(j, k, vt1, vt2):
            """PE projections for iteration k -> pk [128, 4, nc] bf16."""
            nco = NCOLS[k]
            off = POFF[k]
            pk = pkp.tile([P, T, NCMAX], BF, tag=f"pk{j}")
            for half in range(2):
                ps = pp.tile([P, 2, NCMAX], F32, tag="pkps", bufs=2)
                for sl in range(2):
                    s = half * 2 + sl
                    nc.tensor.matmul(
                        ps[:, sl, 0:nco], vt1[:, s, :],
                        pw[:, 0, off : off + nco], start=True, stop=False)
                    nc.tensor.matmul(
                        ps[:, sl, 0:nco], vt2[0:69, s, :],
                        pw[0:69, 1, off : off + nco], start=False, stop=True)
                nc.scalar.copy(
                    pk[:, 2 * half : 2 * half + 2, 0:nco], ps[:, :, 0:nco])
            return pk

        def dots(j, k, mtc, pk):
            """M_k[i,j] for all 16 slot pairs -> mtc[:, 4j:4j+4, :] f32."""
            r1 = RANKS[k] + 1
            pap = pk[:].ap[0]
            if k == 0:
                # big r: fused stt (mult + f32 accumulate in one 1x pass)
                scr = scp.tile([P, 256], BF, tag="scr197", bufs=6)
                for idx in range(16):
                    i, jj = idx // 4, idx % 4
                    in0 = _ap(pk, [pap, [1, r1]],
                              offset_elems=i * NCMAX + r1)
                    in1 = _ap(pk, [pap, [1, r1]], offset_elems=jj * NCMAX)
                    nc.vector.scalar_tensor_tensor(
                        out=scr[:, 0:r1], in0=in0, scalar=1.0, in1=in1,
                        op0=ALU.mult, op1=ALU.mult,
                        accum_out=mtc[:, 4 * j + i, jj : jj + 1])
            else:
                # small r: one bf16 2x tensor_tensor + one inner-axis reduce
                scr = scp.tile([P, T, T, 65], BF, tag="scr")
                in0 = _ap(pk, [pap, [NCMAX, 4], [0, 4], [1, r1]],
                          offset_elems=r1)
                in1 = _ap(pk, [pap, [0, 4], [NCMAX, 4], [1, r1]])
                nc.vector.tensor_tensor(
                    out=scr[:, :, :, 0:r1], in0=in0, in1=in1, op=ALU.mult)
                nc.vector.tensor_reduce(
                    out=mtc[:, 4 * j : 4 * j + 4, :], in_=scr[:, :, :, 0:r1],
                    axis=mybir.AxisListType.X, op=ALU.add)

        def serial_phase(k, mtc, c_prev):
            """Per-k 4x4 chain for ALL tiles in single wide DVE ops over the
            combined [128, (j,t), s] layout: scores = C mt C^T -> e = exp ->
            C'u = e C -> C' = C'u / rowsum. Returns new combined C tile."""
            if k == 0:
                s_t = mtc
            else:
                cap = c_prev[:].ap[0]
                # replicate C 4x -> crep[j, rep, s, jj] so every TT operand
                # stays within the ISA's 3-free-dim AP limit
                crep = sm.tile([P, 256], F32, tag="crep", bufs=2)
                nc.vector.tensor_copy(
                    _ap(crep, [crep[:].ap[0], [64, 4], [16, 4], [1, 16]]),
                    _ap(c_prev, [cap, [16, 4], [0, 4], [1, 16]]))
                scrd = scp.tile([P, 16, T, T], F32, tag="scrd", bufs=4)
                nc.gpsimd.tensor_tensor(  # D[j,i,s] = sum_jj mt[j,i,jj] C[j,s,jj]
                    out=scrd,
                    in0=_ap(mtc, [mtc[:].ap[0], [4, 16], [0, 4], [1, 4]]),
                    in1=crep[:],
                    op=ALU.mult)
                dm = sm.tile([P, 16, T], F32, tag="dm")
                nc.vector.tensor_reduce(
                    out=dm, in_=scrd, axis=mybir.AxisListType.X, op=ALU.add)
                drep = sm.tile([P, 256], F32, tag="drep", bufs=2)
                nc.vector.tensor_copy(
                    _ap(drep, [drep[:].ap[0], [64, 4], [16, 4], [1, 16]]),
                    _ap(dm, [dm[:].ap[0], [16, 4], [0, 4], [1, 16]]))
                scrd2 = scp.tile([P, 16, T, T], F32, tag="scrd", bufs=4)
                nc.gpsimd.tensor_tensor(  # S[j,t,s] = sum_i C[j,t,i] D[j,i,s]
                    out=scrd2,
                    in0=_ap(c_prev, [cap, [4, 16], [0, 4], [1, 4]]),
                    in1=_ap(drep, [drep[:].ap[0], [16, 16], [1, 4], [4, 4]]),
                    op=ALU.mult)
                s_t = sm.tile([P, 16, T], F32, tag="st")
                nc.vector.tensor_reduce(
                    out=s_t, in_=scrd2, axis=mybir.AxisListType.X, op=ALU.add)
            e = sm.tile([P, 16, T], F32, tag="e")
            nc.scalar.activation(e, s_t, AF.Exp)
            sums = sm.tile([P, 16], F32, tag="su")
            nc.vector.reduce_sum(sums, e, axis=mybir.AxisListType.X)
            rec = sm.tile([P, 16], F32, tag="re")
            nc.vector.reciprocal(rec, sums)
            if k == 0:
                cnum = e
            else:
                scrd3 = scp.tile([P, 16, T, T], F32, tag="scrd", bufs=4)
                nc.gpsimd.tensor_tensor(  # C'u[j,t,jj] = sum_s e[j,t,s] C[j,s,jj]
                    out=scrd3,
                    in0=_ap(e, [e[:].ap[0], [4, 16], [0, 4], [1, 4]]),
                    in1=_ap(crep, [crep[:].ap[0], [16, 16], [1, 4], [4, 4]]),
                    op=ALU.mult)
                cnum = sm.tile([P, 16, T], F32, tag="cu")
                nc.vector.tensor_reduce(
                    out=cnum, in_=scrd3, axis=mybir.AxisListType.X, op=ALU.add)
            c_new = sm.tile([P, 16, T], F32, tag="call", bufs=3)
            nc.vector.scalar_tensor_tensor(
                out=c_new, in0=cnum, scalar=1.0,
                in1=_ap(rec, [rec[:].ap[0], [1, 16], [0, T]]),
                op0=ALU.mult, op1=ALU.mult)
            return c_new

        def recon(j, h0, c8):
            """w[:, t, :] = sum_s C8[t,s] * h0[:, s, :] (ones col rides along)."""
            w = wp.tile([P, T, SLOT], FR, tag=f"w{j}", bufs=1)
            for t in range(T):
                nc.gpsimd.tensor_scalar_mul(
                    w[:, t, :], h0[:, 0, :], c8[:, 4 * j + t, 0:1])
            for t in range(2):
                for s in range(1, T):
                    nc.vector.scalar_tensor_tensor(
                        out=w[:, t, :], in0=h0[:, s, :],
                        scalar=c8[:, 4 * j + t, s : s + 1], in1=w[:, t, :],
                        op0=ALU.mult, op1=ALU.add)
            for t in range(2, T):
                pct = wp.tile([P, SLOT], F32, tag="pct", bufs=2)
                for s in range(1, T):
                    nc.gpsimd.tensor_scalar_mul(
                        pct, h0[:, s, :], c8[:, 4 * j + t, s : s + 1])
                    nc.gpsimd.tensor_add(w[:, t, :], w[:, t, :], pct)
            return w

        def decoder(ws, g):
            """Decoder over nsub tiles (N = nsub*128 wide matmuls)."""
            W = nsub * P
            ht1 = wkd.tile([P, T, W], FR, tag="ht1")
            ht2 = wkd.tile([69, T, W], FR, tag="ht2")
            for t in range(T):
                t1_ps = pp.tile([P, W], FR, tag="dt1ps", bufs=1)
                t2_ps = pp.tile([69, W], FR, tag="dt2ps", bufs=1)
                for j in range(nsub):
                    nc.tensor.transpose(
                        t1_ps[:, j * P : (j + 1) * P], ws[j][:, t, 0:P], ident_r
                    )
                    nc.tensor.transpose(
                        t2_ps[:, j * P : (j + 1) * P], ws[j][:, t, P : P + 69],
                        ident_r
                    )
                nc.scalar.copy(ht1[:, t, :], t1_ps)
                nc.vector.tensor_copy(ht2[:, t, :], t2_ps)

            # dec1 = relu(Wd1~ @ w.T + bd1~), feature-major, 7 M-chunks
            d1a = wkd.tile([P, 6, W], FR, tag="d1a")
            d1b = wkd.tile([17, W], FR, tag="d1b")
            nc.vector.tensor_copy(d1b, ones_c[0:17, 0:W])
            for m in range(7):
                mw = min(P, FEAT - m * P)
                mp = pp.tile([P, W], F32, tag="mp")
                msl = slice(m * P, m * P + mw)
                for t in range(T):
                    nc.tensor.matmul(mp[0:mw, :], d1_w[:, t, msl], ht1[:, t, :],
                                     start=(t == 0), stop=False)
                for t in range(T):
                    nc.tensor.matmul(mp[0:mw, :], d1_w[0:69, 4 + t, msl],
                                     ht2[:, t, :], start=False, stop=(t == 3))
                if m < 6:
                    nc.scalar.activation(d1a[:, m, :], mp, AF.Relu)
                else:
                    nc.scalar.activation(d1b[0:16, :], mp[0:16, :], AF.Relu)

            # dec2 = Wd2 @ relu1 + bd2, feature-major
            d2a = wkd.tile([P, 6, W], FR, tag="d2a")
            d2b = wkd.tile([17, W], FR, tag="d2b")
            nc.vector.tensor_copy(d2b, ones_c[0:17, 0:W])
            for m in range(7):
                mw = min(P, FEAT - m * P)
                mp = pp.tile([P, W], F32, tag="mp")
                msl = slice(m * P, m * P + mw)
                for c in range(6):
                    nc.tensor.matmul(mp[0:mw, :], d2_w[:, c, msl], d1a[:, c, :],
                                     start=(c == 0), stop=False)
                nc.tensor.matmul(mp[0:mw, :], d2_w[0:17, 6, msl], d1b,
                                 start=False, stop=True)
                if m < 6:
                    nc.scalar.copy(d2a[:, m, :], mp)
                else:
                    nc.scalar.copy(d2b[0:16, :], mp[0:16, :])

            # logits + softmax per subtile
            for j in range(nsub):
                jsl = slice(j * P, (j + 1) * P)
                lgt = pp.tile([P, W], F32, tag="mp")
                lg = lgt[:, 0:10]
                for c in range(6):
                    nc.tensor.matmul(lg, d2a[:, c, jsl], ow_w[:, c, :],
                                     start=(c == 0), stop=False)
                nc.tensor.matmul(lg, d2b[:, jsl], ow_w[0:17, 6, :],
                                 start=False, stop=True)
                mx = sm.tile([P, 1], F32, tag="mx")
                nc.vector.reduce_max(mx, lg, axis=mybir.AxisListType.X)
                nmx = sm.tile([P, 1], F32, tag="nmx")
                nc.vector.tensor_scalar_mul(nmx, mx, -1.0)
                e10 = sm.tile([P, 10], F32, tag="e10")
                s10 = sm.tile([P, 1], F32, tag="s10")
                nc.scalar.activation(e10, lg, AF.Exp, bias=nmx, accum_out=s10)
                r10 = sm.tile([P, 1], F32, tag="r10")
                nc.vector.reciprocal(r10, s10)
                o10 = sm.tile([P, 10], F32, tag="o10")
                nc.vector.tensor_scalar_mul(o10, e10, r10)
                nc.sync.dma_start(
                    out=out_d[ds(g * (nsub * P) + j * P, P), :], in_=o10
                )

        def body(g, preloaded=None):
            h0s, hbs, cs, ws = [], [], None, []
            vts = []
            if preloaded is None:
                preloaded = [load_tile(g, j) for j in range(nsub)]
            for h0, hb, vt1, vt2 in preloaded:
                h0s.append(h0)
                hbs.append(hb)
                vts.append((vt1, vt2))
            mtk = []  # combined Mt tile per k
            for k in range(8):
                mtc = mtp.tile([P, 16, T], F32, tag="mtk", bufs=3)
                for j in range(nsub):
                    pk = proj(j, k, *vts[j])
                    dots(j, k, mtc, pk)
                mtk.append(mtc)
                # serial chain runs one k behind so the independent dots of
                # the next k fill engine gaps while the chain ping-pongs
                if k >= 1:
                    cs = serial_phase(k - 1, mtk[k - 1], cs)
            cs = serial_phase(7, mtk[7], cs)
            for j in range(nsub):
                ws.append(recon(j, h0s[j], cs))
            decoder(ws, g)

        # group 0 loads first so its x DMAs precede the 6MB of decoder
        # weights on the sync queue; weights stream in during attention
        pre0 = [load_tile(0, j) for j in range(nsub)]
        nc.sync.dma_start(out=d1_w, in_=d1_d[:, :, :])
        nc.sync.dma_start(out=d2_w, in_=d2_d[:, :, :])
        nc.sync.dma_start(out=ow_w, in_=ow_d[:, :, :])
        body(0, preloaded=pre0)
        if ngroups > 1:
            with tc.For_i(1, ngroups, 1) as g:
                body(g)
        for _pool in (pp, wkd, wp, sm, mtp, scp, pkp, vp, hp, consts):
            _pool.release()

    nc.compile()
    return nc


def pack_weights(W1, b1, W2, b2, W3, b3, Wd1, bd1, Wd2, bd2, Wo, bo):
    f64 = np.float64
    W1, b1, W2, b2, W3, b3 = (np.asarray(t, f64) for t in (W1, b1, W2, b2, W3, b3))
    G = W1.T @ W2
    a = W2.T @ b1

    A = np.eye(FV)
    m = np.zeros(FV)
    pw = np.zeros((P, 2, PTOT), np.float32)
    for k in range(8):
        Gk = A.T @ G @ A
        ak = A.T @ (G.T @ m + a)
        nco = NCOLS[k]
        Wk = np.zeros((197, nco), f64)
        r = RANKS[k]
        r1 = r + 1
        U, S, Vh = np.linalg.svd(Gk)
        Wk[:FV, :r] = (np.diag(S[:r]) @ Vh[:r]).T
        Wk[:FV, r] = ak
        Wk[:FV, r1 : r1 + r] = U[:, :r]
        Wk[FV, r1 + r] = 1.0
        off = POFF[k]
        pw[:, 0, off : off + nco] = Wk[0:128]
        pw[0:69, 1, off : off + nco] = Wk[128:197]
        A = W3 @ A
        m = W3 @ m + b3
    A8, m8 = A, m

    # fold W3^8 / m8 into the first decoder layer
    BD = np.zeros((FEAT, FEAT), f64)
    mm = np.zeros(FEAT, f64)
    for t in range(T):
        BD[t * FV : (t + 1) * FV, t * FV : (t + 1) * FV] = A8
        mm[t * FV : (t + 1) * FV] = m8
    Wd1f = np.asarray(Wd1, f64) @ BD
    bd1f = np.asarray(bd1, f64) + np.asarray(Wd1, f64) @ mm

    d1 = np.zeros((P, 8, FEAT), np.float32)
    W1T = Wd1f.T  # [784 f_in, 784 j]
    for t in range(T):
        d1[:, t, :] = W1T[t * FV : t * FV + P, :]
        d1[0:68, 4 + t, :] = W1T[t * FV + P : (t + 1) * FV, :]
    d1[68, 4, :] = bd1f

    d2 = np.zeros((P, 7, FEAT), np.float32)
    W2T = np.asarray(Wd2, f64).T
    for cidx in range(6):
        d2[:, cidx, :] = W2T[cidx * P : (cidx + 1) * P, :]
    d2[0:16, 6, :] = W2T[768:784, :]
    d2[16, 6, :] = np.asarray(bd2, f64)

    ow = np.zeros((P, 7, 10), np.float32)
    WoT = np.asarray(Wo, f64).T
    for cidx in range(6):
        ow[:, cidx, :] = WoT[cidx * P : (cidx + 1) * P, :]
    ow[0:16, 6, :] = WoT[768:784, :]
    ow[16, 6, :] = np.asarray(bo, f64)
    return pw.astype(ml_dtypes.bfloat16), d1, d2, ow


_NC_CACHE = {}


def kernel(**inputs):
    x = np.ascontiguousarray(np.asarray(inputs["x"], np.float32))
    zu, d1, d2, ow = pack_weights(
        inputs["W1"], inputs["b1"], inputs["W2"], inputs["b2"], inputs["W3"],
        inputs["b3"], inputs["Wd1"], inputs["bd1"], inputs["Wd2"],
        inputs["bd2"], inputs["Wo"], inputs["bo"],
    )
    if "nc" not in _NC_CACHE:
        _NC_CACHE["nc"] = build(4, 8)
    nc = _NC_CACHE["nc"]
    bpc = B // NCORES
    in_maps = [
        {
            "x": x[c * bpc : (c + 1) * bpc],
            "zu_w": zu,
            "dec1_w": d1,
            "dec2_w": d2,
            "out_w": ow,
        }
        for c in range(NCORES)
    ]
    res = run_bass_kernel_spmd(nc, in_maps, core_ids=list(range(NCORES)))
    return np.concatenate([res.results[c]["out"] for c in range(NCORES)], axis=0)


# revision 44
# speedup vs baseline: 2.4866x; 1.0967x over previous
"""Trainium2 Bass kernel for nn_CapsuleNeuralNetworkV2 (8 cores, data-parallel).

Reference math (per sample, 8 capsule iterations then decoder):
  v = h.reshape(4, 196); q,k,u = affine(v); scores = q k^T;
  P = softmax(scores); h' = P u;  dec = relu(h Wd1^T+bd1) Wd2^T+bd2;
  out = softmax(dec Wo^T + bo).

Restructuring (host-side algebra):
  Since each P has rows summing to 1, the state stays in the span of the 4
  initial slots: v^(k) = W3^k w^(k) + m_k with w^(k) = C^(k) V (C is a
  per-sample 4x4 convex-coefficient matrix, V the initial slots).
  scores^(k)[t,s] = C[t] M_k C[s]^T (mod per-t constants that cancel in
  softmax), where M_k[i,j] = v_i.(G_k v_j) + a_k.v_j depends only on the
  INITIAL slots: G_k = (W3^k)^T G W3^k, G = W1^T W2,
  a_k = (W3^k)^T (G^T m_k + W2^T b1).  G_k is numerically low-rank for k>=1
  (powers of a random matrix), so M_k is computed from rank-r_k SVD
  projections p_i = U_r^T v_i, q_j = (S V_r^T) v_j: M[i,j] ~ p_i.q_j + r_j.
  Per iteration only the tiny 4x4 chain is sequential:
  scores = C M C^T -> softmax -> C' = P C.  All projections/M_k are
  C-independent and pipeline on PE/Act/DVE ahead of the chain.
  Final w^(8) = C^(8) V; W3^8/m_8 are folded into Wd1/bd1 on the host.

Engines: PE transposes V once per tile + small bf16 projection matmuls +
decoder; DVE/Pool share the per-sample dot products and the 4x4 chain; Act
does PSUM evacuation, exp, and decoder activations.
"""

import numpy as np
import ml_dtypes

import concourse.bass as bass
import concourse.tile as tile
from concourse import bacc, mybir
from concourse.bass import ds
from concourse.bass_utils import run_bass_kernel_spmd
from concourse.masks import make_identity

FR = mybir.dt.float32r
BF = mybir.dt.bfloat16
F32 = mybir.dt.float32
AF = mybir.ActivationFunctionType
ALU = mybir.AluOpType

B = 32768
NCORES = 8
P = 128
T = 4
FV = 196
FEAT = 784
SLOT = 198  # h slot: 196 data + ones col (196) + spare (197)

RANKS = [126, 64, 48, 32, 24, 16, 12, 8]
NCOLS = [2 * (r + 1) for r in RANKS]  # proj cols per slot per k
POFF = [0]
for _n in NCOLS:
    POFF.append(POFF[-1] + _n)
PTOT = POFF[-1]
NCMAX = max(NCOLS)


def _ap(t, dims, offset_elems=0):
    """Hand-built AP over a tile's tensor: dims = [[step, count], ...]."""
    a = t[:] if hasattr(t, "tile") or not isinstance(t, bass.AP) else t
    return bass.AP(tensor=a.tensor, offset=a.offset + offset_elems, ap=dims)


def build(nsub=4, ngroups=8):
    """One NeuronCore program processing nsub*ngroups*128 samples."""
    bpc = nsub * ngroups * P
    nc = bacc.Bacc("TRN2", target_bir_lowering=False, debug=False)

    x_d = nc.dram_tensor("x", [bpc, FEAT], FR, kind="ExternalInput")
    pw_d = nc.dram_tensor("zu_w", [P, 2, PTOT], BF, kind="ExternalInput")
    d1_d = nc.dram_tensor("dec1_w", [P, 8, FEAT], FR, kind="ExternalInput")
    d2_d = nc.dram_tensor("dec2_w", [P, 7, FEAT], FR, kind="ExternalInput")
    ow_d = nc.dram_tensor("out_w", [P, 7, 10], FR, kind="ExternalInput")
    out_d = nc.dram_tensor("out", [bpc, 10], F32, kind="ExternalOutput")

    with tile.TileContext(nc) as tc:
        consts = tc.alloc_tile_pool(name="consts", bufs=1)
        hp = tc.alloc_tile_pool(name="h", bufs=2)
        vp = tc.alloc_tile_pool(name="vt", bufs=2)
        pkp = tc.alloc_tile_pool(name="pk", bufs=3)
        scp = tc.alloc_tile_pool(name="scr", bufs=4)
        mtp = tc.alloc_tile_pool(name="mt", bufs=8)
        sm = tc.alloc_tile_pool(name="small", bufs=3)
        wp = tc.alloc_tile_pool(name="w", bufs=2)
        wkd = tc.alloc_tile_pool(name="wkd", bufs=1)
        pp = tc.alloc_tile_pool(name="ps", bufs=2, space="PSUM")

        ident_f = consts.tile([P, P], F32)
        make_identity(nc, ident_f)
        ident_b = consts.tile([P, P], BF)
        nc.vector.tensor_copy(ident_b, ident_f)
        ident_r = consts.tile([P, P], FR)
        nc.vector.tensor_copy(ident_r, ident_f)
        ones_c = consts.tile([P, 512], F32)
        nc.vector.memset(ones_c, 1.0)
        pw = consts.tile([P, 2, PTOT], BF)
        nc.sync.dma_start(out=pw, in_=pw_d[:, :, :])
        # decoder weights DMA'd after group 0's x tiles (emitted in build
        # below) so the first group's compute isn't starved behind 6MB
        d1_w = consts.tile([P, 8, FEAT], FR)
        d2_w = consts.tile([P, 7, FEAT], FR)
        ow_w = consts.tile([P, 7, 10], FR)

        def load_tile(g, j):
            h0 = hp.tile([P, T, SLOT], FR, tag=f"h{j}")
            nc.sync.dma_start(
                out=h0[:, :, 0:FV],
                in_=x_d[ds(g * (nsub * P) + j * P, P), :].rearrange(
                    "p (t f) -> p t f", t=T
                ),
            )
            nc.gpsimd.tensor_copy(h0[:, :, 196:198], ones_c[:, 0 : 2 * T])
            hb = hp.tile([P, T, SLOT], BF, tag=f"hb{j}", bufs=1)
            nc.gpsimd.tensor_copy(hb, h0)
            vt1 = vp.tile([P, T, P], BF, tag=f"vt1{j}")
            vt2 = vp.tile([69, T, P], BF, tag=f"vt2{j}")
            t1_ps = pp.tile([P, T, P], BF, tag="t1ps", bufs=1)
            t2_ps = pp.tile([69, T, P], BF, tag="t2ps", bufs=1)
            for t in range(T):
                nc.tensor.transpose(t1_ps[:, t, :], hb[:, t, 0:P], ident_b)
                nc.tensor.transpose(t2_ps[:, t, :], hb[:, t, P : P + 69], ident_b)
            nc.scalar.copy(vt1, t1_ps)
            nc.scalar.copy(vt2, t2_ps)
            return h0, hb, vt1, vt2

        def proj(j, k, vt1, vt2):
            """PE projections for iteration k -> pk [128, 4, nc] bf16."""
            nco = NCOLS[k]
            off = POFF[k]
            pk = pkp.tile([P, T, NCMAX], BF, tag=f"pk{j}")
            for half in range(2):
                ps = pp.tile([P, 2, NCMAX], F32, tag="pkps", bufs=2)
                for sl in range(2):
                    s = half * 2 + sl
                    nc.tensor.matmul(
                        ps[:, sl, 0:nco], vt1[:, s, :],
                        pw[:, 0, off : off + nco], start=True, stop=False)
                    nc.tensor.matmul(
                        ps[:, sl, 0:nco], vt2[0:69, s, :],
                        pw[0:69, 1, off : off + nco], start=False, stop=True)
                nc.scalar.copy(
                    pk[:, 2 * half : 2 * half + 2, 0:nco], ps[:, :, 0:nco])
            return pk

        def dots(j, k, mtc, pk):
            """M_k[i,j] for all 16 slot pairs -> mtc[:, 4j:4j+4, :] f32."""
            r1 = RANKS[k] + 1
            pap = pk[:].ap[0]
            if k == 0:
                # big r: fused stt (mult + f32 accumulate in one 1x pass)
                scr = scp.tile([P, 256], BF, tag="scr197", bufs=6)
                for idx in range(16):
                    i, jj = idx // 4, idx % 4
                    in0 = _ap(pk, [pap, [1, r1]],
                              offset_elems=i * NCMAX + r1)
                    in1 = _ap(pk, [pap, [1, r1]], offset_elems=jj * NCMAX)
                    nc.vector.scalar_tensor_tensor(
                        out=scr[:, 0:r1], in0=in0, scalar=1.0, in1=in1,
                        op0=ALU.mult, op1=ALU.mult,
                        accum_out=mtc[:, 4 * j + i, jj : jj + 1])
            else:
                # small r: one bf16 2x tensor_tensor + one inner-axis reduce
                scr = scp.tile([P, T, T, 65], BF, tag="scr")
                in0 = _ap(pk, [pap, [NCMAX, 4], [0, 4], [1, r1]],
                          offset_elems=r1)
                in1 = _ap(pk, [pap, [0, 4], [NCMAX, 4], [1, r1]])
                nc.vector.tensor_tensor(
                    out=scr[:, :, :, 0:r1], in0=in0, in1=in1, op=ALU.mult)
                nc.vector.tensor_reduce(
                    out=mtc[:, 4 * j : 4 * j + 4, :], in_=scr[:, :, :, 0:r1],
                    axis=mybir.AxisListType.X, op=ALU.add)

        def serial_phase(k, mtc, c_prev):
            """Per-k 4x4 chain for ALL tiles in single wide DVE ops over the
            combined [128, (j,t), s] layout: scores = C mt C^T -> e = exp ->
            C'u = e C -> C' = C'u / rowsum. Returns new combined C tile."""
            if k == 0:
                s_t = mtc
            else:
                cap = c_prev[:].ap[0]
                # replicate C 4x -> crep[j, rep, s, jj] so every TT operand
                # stays within the ISA's 3-free-dim AP limit
                crep = sm.tile([P, 256], F32, tag="crep", bufs=2)
                nc.vector.tensor_copy(
                    _ap(crep, [crep[:].ap[0], [64, 4], [16, 4], [1, 16]]),
                    _ap(c_prev, [cap, [16, 4], [0, 4], [1, 16]]))
                scrd = scp.tile([P, 16, T, T], F32, tag="scrd", bufs=4)
                nc.gpsimd.tensor_tensor(  # D[j,i,s] = sum_jj mt[j,i,jj] C[j,s,jj]
                    out=scrd,
                    in0=_ap(mtc, [mtc[:].ap[0], [4, 16], [0, 4], [1, 4]]),
                    in1=crep[:],
                    op=ALU.mult)
                dm = sm.tile([P, 16, T], F32, tag="dm")
                nc.vector.tensor_reduce(
                    out=dm, in_=scrd, axis=mybir.AxisListType.X, op=ALU.add)
                drep = sm.tile([P, 256], F32, tag="drep", bufs=2)
                nc.vector.tensor_copy(
                    _ap(drep, [drep[:].ap[0], [64, 4], [16, 4], [1, 16]]),
                    _ap(dm, [dm[:].ap[0], [16, 4], [0, 4], [1, 16]]))
                scrd2 = scp.tile([P, 16, T, T], F32, tag="scrd", bufs=4)
                nc.gpsimd.tensor_tensor(  # S[j,t,s] = sum_i C[j,t,i] D[j,i,s]
                    out=scrd2,
                    in0=_ap(c_prev, [cap, [4, 16], [0, 4], [1, 4]]),
                    in1=_ap(drep, [drep[:].ap[0], [16, 16], [1, 4], [4, 4]]),
                    op=ALU.mult)
                s_t = sm.tile([P, 16, T], F32, tag="st")
                nc.vector.tensor_reduce(
                    out=s_t, in_=scrd2, axis=mybir.AxisListType.X, op=ALU.add)
            e = sm.tile([P, 16, T], F32, tag="e")
            nc.scalar.activation(e, s_t, AF.Exp)
            sums = sm.tile([P, 16], F32, tag="su")
            nc.vector.reduce_sum(sums, e, axis=mybir.AxisListType.X)
            rec = sm.tile([P, 16], F32, tag="re")
            nc.vector.reciprocal(rec, sums)
            if k == 0:
                cnum = e
            else:
                scrd3 = scp.tile([P, 16, T, T], F32, tag="scrd", bufs=4)
                nc.gpsimd.tensor_tensor(  # C'u[j,t,jj] = sum_s e[j,t,s] C[j,s,jj]
                    out=scrd3,
                    in0=_ap(e, [e[:].ap[0], [4, 16], [0, 4], [1, 4]]),
                    in1=_ap(crep, [crep[:].ap[0], [16, 16], [1, 4], [4, 4]]),
                    op=ALU.mult)
                cnum = sm.tile([P, 16, T], F32, tag="cu")
                nc.vector.tensor_reduce(
                    out=cnum, in_=scrd3, axis=mybir.AxisListType.X, op=ALU.add)
            c_new = sm.tile([P, 16, T], F32, tag="call", bufs=3)
            nc.vector.scalar_tensor_tensor(
                out=c_new, in0=cnum, scalar=1.0,
                in1=_ap(rec, [rec[:].ap[0], [1, 16], [0, T]]),
                op0=ALU.mult, op1=ALU.mult)
            return c_new

        def recon(j, h0, c8):
            """w[:, t, :] = sum_s C8[t,s] * h0[:, s, :] (ones col rides along)."""
            w = wp.tile([P, T, SLOT], FR, tag=f"w{j}", bufs=1)
            for t in range(T):
                nc.scalar.activation(
                    w[:, t, :], h0[:, 0, :], AF.Copy,
                    scale=c8[:, 4 * j + t, 0:1])
            for t in range(3):
                for s in range(1, T):
                    nc.vector.scalar_tensor_tensor(
                        out=w[:, t, :], in0=h0[:, s, :],
                        scalar=c8[:, 4 * j + t, s : s + 1], in1=w[:, t, :],
                        op0=ALU.mult, op1=ALU.add)
            for t in range(3, T):
                pct = wp.tile([P, SLOT], F32, tag="pct", bufs=2)
                for s in range(1, T):
                    nc.gpsimd.tensor_scalar_mul(
                        pct, h0[:, s, :], c8[:, 4 * j + t, s : s + 1])
                    nc.gpsimd.tensor_add(w[:, t, :], w[:, t, :], pct)
            return w

        def decoder(ws, g):
            """Decoder over nsub tiles (N = nsub*128 wide matmuls)."""
            W = nsub * P
            ht1 = wkd.tile([P, T, W], FR, tag="ht1")
            ht2 = wkd.tile([69, T, W], FR, tag="ht2")
            for t in range(T):
                t1_ps = pp.tile([P, W], FR, tag="dt1ps", bufs=1)
                t2_ps = pp.tile([69, W], FR, tag="dt2ps", bufs=1)
                for j in range(nsub):
                    nc.tensor.transpose(
                        t1_ps[:, j * P : (j + 1) * P], ws[j][:, t, 0:P], ident_r
                    )
                    nc.tensor.transpose(
                        t2_ps[:, j * P : (j + 1) * P], ws[j][:, t, P : P + 69],
                        ident_r
                    )
                nc.scalar.copy(ht1[:, t, :], t1_ps)
                nc.vector.tensor_copy(ht2[:, t, :], t2_ps)

            # dec1 = relu(Wd1~ @ w.T + bd1~), feature-major, 7 M-chunks
            d1a = wkd.tile([P, 6, W], FR, tag="d1a")
            d1b = wkd.tile([17, W], FR, tag="d1b")
            nc.vector.tensor_copy(d1b, ones_c[0:17, 0:W])
            for m in range(7):
                mw = min(P, FEAT - m * P)
                mp = pp.tile([P, W], F32, tag="mp")
                msl = slice(m * P, m * P + mw)
                for t in range(T):
                    nc.tensor.matmul(mp[0:mw, :], d1_w[:, t, msl], ht1[:, t, :],
                                     start=(t == 0), stop=False)
                for t in range(T):
                    nc.tensor.matmul(mp[0:mw, :], d1_w[0:69, 4 + t, msl],
                                     ht2[:, t, :], start=False, stop=(t == 3))
                if m < 6:
                    nc.scalar.activation(d1a[:, m, :], mp, AF.Relu)
                else:
                    nc.scalar.activation(d1b[0:16, :], mp[0:16, :], AF.Relu)

            # dec2 = Wd2 @ relu1 + bd2, feature-major
            d2a = wkd.tile([P, 6, W], FR, tag="d2a")
            d2b = wkd.tile([17, W], FR, tag="d2b")
            nc.vector.tensor_copy(d2b, ones_c[0:17, 0:W])
            for m in range(7):
                mw = min(P, FEAT - m * P)
                mp = pp.tile([P, W], F32, tag="mp")
                msl = slice(m * P, m * P + mw)
                for c in range(6):
                    nc.tensor.matmul(mp[0:mw, :], d2_w[:, c, msl], d1a[:, c, :],
                                     start=(c == 0), stop=False)
                nc.tensor.matmul(mp[0:mw, :], d2_w[0:17, 6, msl], d1b,
                                 start=False, stop=True)
                if m < 6:
                    nc.scalar.copy(d2a[:, m, :], mp)
                else:
                    nc.scalar.copy(d2b[0:16, :], mp[0:16, :])

            # logits + softmax per subtile
            for j in range(nsub):
                jsl = slice(j * P, (j + 1) * P)
                lgt = pp.tile([P, W], F32, tag="mp")
                lg = lgt[:, 0:10]
                for c in range(6):
                    nc.tensor.matmul(lg, d2a[:, c, jsl], ow_w[:, c, :],
                                     start=(c == 0), stop=False)
                nc.tensor.matmul(lg, d2b[:, jsl], ow_w[0:17, 6, :],
                                 start=False, stop=True)
                mx = sm.tile([P, 1], F32, tag="mx")
                nc.vector.reduce_max(mx, lg, axis=mybir.AxisListType.X)
                nmx = sm.tile([P, 1], F32, tag="nmx")
                nc.vector.tensor_scalar_mul(nmx, mx, -1.0)
                e10 = sm.tile([P, 10], F32, tag="e10")
                s10 = sm.tile([P, 1], F32, tag="s10")
                nc.scalar.activation(e10, lg, AF.Exp, bias=nmx, accum_out=s10)
                r10 = sm.tile([P, 1], F32, tag="r10")
                nc.vector.reciprocal(r10, s10)
                o10 = sm.tile([P, 10], F32, tag="o10")
                nc.vector.tensor_scalar_mul(o10, e10, r10)
                nc.sync.dma_start(
                    out=out_d[ds(g * (nsub * P) + j * P, P), :], in_=o10
                )

        def body(g, preloaded=None):
            h0s, hbs, cs, ws = [], [], None, []
            vts = []
            # k=0 proj+dots interleaved per tile so the first tile's chain
            # races ahead of later tiles' loads
            mtc0 = mtp.tile([P, 16, T], F32, tag="mtk", bufs=3)
            for j in range(nsub):
                h0, hb, vt1, vt2 = (
                    load_tile(g, j) if preloaded is None else preloaded[j]
                )
                h0s.append(h0)
                hbs.append(hb)
                vts.append((vt1, vt2))
                pk = proj(j, 0, vt1, vt2)
                dots(j, 0, mtc0, pk)
            mtk = [mtc0]  # combined Mt tile per k
            for k in range(1, 8):
                mtc = mtp.tile([P, 16, T], F32, tag="mtk", bufs=3)
                for j in range(nsub):
                    pk = proj(j, k, *vts[j])
                    dots(j, k, mtc, pk)
                mtk.append(mtc)
                # serial chain runs one k behind so the independent dots of
                # the next k fill engine gaps while the chain ping-pongs
                cs = serial_phase(k - 1, mtk[k - 1], cs)
            cs = serial_phase(7, mtk[7], cs)
            for j in range(nsub):
                ws.append(recon(j, h0s[j], cs))
            decoder(ws, g)

        # group 0 loads first so its x DMAs precede the 6MB of decoder
        # weights; weight DMAs ride the Act queue so their triggers land
        # after the x transfers are in flight
        pre0 = [load_tile(0, j) for j in range(nsub)]
        nc.sync.dma_start(out=d1_w, in_=d1_d[:, :, :])
        nc.sync.dma_start(out=d2_w, in_=d2_d[:, :, :])
        nc.sync.dma_start(out=ow_w, in_=ow_d[:, :, :])
        body(0, preloaded=pre0)
        if ngroups > 1:
            with tc.For_i(1, ngroups, 1) as g:
                body(g)
        for _pool in (pp, wkd, wp, sm, mtp, scp, pkp, vp, hp, consts):
            _pool.release()

    nc.compile()
    return nc


def pack_weights(W1, b1, W2, b2, W3, b3, Wd1, bd1, Wd2, bd2, Wo, bo):
    f64 = np.float64
    W1, b1, W2, b2, W3, b3 = (np.asarray(t, f64) for t in (W1, b1, W2, b2, W3, b3))
    G = W1.T @ W2
    a = W2.T @ b1

    A = np.eye(FV)
    m = np.zeros(FV)
    pw = np.zeros((P, 2, PTOT), np.float32)
    for k in range(8):
        Gk = A.T @ G @ A
        ak = A.T @ (G.T @ m + a)
        nco = NCOLS[k]
        Wk = np.zeros((197, nco), f64)
        r = RANKS[k]
        r1 = r + 1
        U, S, Vh = np.linalg.svd(Gk)
        Wk[:FV, :r] = (np.diag(S[:r]) @ Vh[:r]).T
        Wk[:FV, r] = ak
        Wk[:FV, r1 : r1 + r] = U[:, :r]
        Wk[FV, r1 + r] = 1.0
        off = POFF[k]
        pw[:, 0, off : off + nco] = Wk[0:128]
        pw[0:69, 1, off : off + nco] = Wk[128:197]
        A = W3 @ A
        m = W3 @ m + b3
    A8, m8 = A, m

    # fold W3^8 / m8 into the first decoder layer
    BD = np.zeros((FEAT, FEAT), f64)
    mm = np.zeros(FEAT, f64)
    for t in range(T):
        BD[t * FV : (t + 1) * FV, t * FV : (t + 1) * FV] = A8
        mm[t * FV : (t + 1) * FV] = m8
    Wd1f = np.asarray(Wd1, f64) @ BD
    bd1f = np.asarray(bd1, f64) + np.asarray(Wd1, f64) @ mm

    d1 = np.zeros((P, 8, FEAT), np.float32)
    W1T = Wd1f.T  # [784 f_in, 784 j]
    for t in range(T):
        d1[:, t, :] = W1T[t * FV : t * FV + P, :]
        d1[0:68, 4 + t, :] = W1T[t * FV + P : (t + 1) * FV, :]
    d1[68, 4, :] = bd1f

    d2 = np.zeros((P, 7, FEAT), np.float32)
    W2T = np.asarray(Wd2, f64).T
    for cidx in range(6):
        d2[:, cidx, :] = W2T[cidx * P : (cidx + 1) * P, :]
    d2[0:16, 6, :] = W2T[768:784, :]
    d2[16, 6, :] = np.asarray(bd2, f64)

    ow = np.zeros((P, 7, 10), np.float32)
    WoT = np.asarray(Wo, f64).T
    for cidx in range(6):
        ow[:, cidx, :] = WoT[cidx * P : (cidx + 1) * P, :]
    ow[0:16, 6, :] = WoT[768:784, :]
    ow[16, 6, :] = np.asarray(bo, f64)
    return pw.astype(ml_dtypes.bfloat16), d1, d2, ow


_NC_CACHE = {}


def kernel(**inputs):
    x = np.ascontiguousarray(np.asarray(inputs["x"], np.float32))
    zu, d1, d2, ow = pack_weights(
        inputs["W1"], inputs["b1"], inputs["W2"], inputs["b2"], inputs["W3"],
        inputs["b3"], inputs["Wd1"], inputs["bd1"], inputs["Wd2"],
        inputs["bd2"], inputs["Wo"], inputs["bo"],
    )
    if "nc" not in _NC_CACHE:
        _NC_CACHE["nc"] = build(4, 8)
    nc = _NC_CACHE["nc"]
    bpc = B // NCORES
    in_maps = [
        {
            "x": x[c * bpc : (c + 1) * bpc],
            "zu_w": zu,
            "dec1_w": d1,
            "dec2_w": d2,
            "out_w": ow,
        }
        for c in range(NCORES)
    ]
    res = run_bass_kernel_spmd(nc, in_maps, core_ids=list(range(NCORES)))
    return np.concatenate([res.results[c]["out"] for c in range(NCORES)], axis=0)


# revision 59
# speedup vs baseline: 2.6976x; 1.0849x over previous
"""Trainium2 Bass kernel for nn_CapsuleNeuralNetworkV2 (8 cores, data-parallel).

Reference math (per sample, 8 capsule iterations then decoder):
  v = h.reshape(4, 196); q,k,u = affine(v); scores = q k^T;
  P = softmax(scores); h' = P u;  dec = relu(h Wd1^T+bd1) Wd2^T+bd2;
  out = softmax(dec Wo^T + bo).

Restructuring (host-side algebra):
  Since each P has rows summing to 1, the state stays in the span of the 4
  initial slots: v^(k) = W3^k w^(k) + m_k with w^(k) = C^(k) V (C is a
  per-sample 4x4 convex-coefficient matrix, V the initial slots).
  scores^(k)[t,s] = C[t] M_k C[s]^T (mod per-t constants that cancel in
  softmax), where M_k[i,j] = v_i.(G_k v_j) + a_k.v_j depends only on the
  INITIAL slots: G_k = (W3^k)^T G W3^k, G = W1^T W2,
  a_k = (W3^k)^T (G^T m_k + W2^T b1).  G_k is numerically low-rank for k>=1
  (powers of a random matrix), so M_k is computed from rank-r_k SVD
  projections p_i = U_r^T v_i, q_j = (S V_r^T) v_j: M[i,j] ~ p_i.q_j + r_j.
  Per iteration only the tiny 4x4 chain is sequential:
  scores = C M C^T -> softmax -> C' = P C.  All projections/M_k are
  C-independent and pipeline on PE/Act/DVE ahead of the chain.
  Final w^(8) = C^(8) V; W3^8/m_8 are folded into Wd1/bd1 on the host.

Engines: PE transposes V once per tile + small bf16 projection matmuls +
decoder; DVE/Pool share the per-sample dot products and the 4x4 chain; Act
does PSUM evacuation, exp, and decoder activations.
"""

import numpy as np
import ml_dtypes

import concourse.bass as bass
import concourse.tile as tile
from concourse import bacc, mybir
from concourse.bass import ds
from concourse.bass_utils import run_bass_kernel_spmd
from concourse.masks import make_identity

FR = mybir.dt.float32r
BF = mybir.dt.bfloat16
F32 = mybir.dt.float32
AF = mybir.ActivationFunctionType
ALU = mybir.AluOpType

B = 32768
NCORES = 8
P = 128
T = 4
FV = 196
FEAT = 784
SLOT = 198  # h slot: 196 data + ones col (196) + spare (197)

RANKS = [96, 48, 32, 24, 16, 12, 8, 8]
NCOLS = [2 * (r + 1) for r in RANKS]  # proj cols per slot per k
POFF = [0]
for _n in NCOLS:
    POFF.append(POFF[-1] + _n)
PTOT = POFF[-1]
NCMAX = max(NCOLS)


def _ap(t, dims, offset_elems=0):
    """Hand-built AP over a tile's tensor: dims = [[step, count], ...]."""
    a = t[:] if hasattr(t, "tile") or not isinstance(t, bass.AP) else t
    return bass.AP(tensor=a.tensor, offset=a.offset + offset_elems, ap=dims)


def build(nsub=4, ngroups=8):
    """One NeuronCore program processing nsub*ngroups*128 samples."""
    bpc = nsub * ngroups * P
    nc = bacc.Bacc("TRN2", target_bir_lowering=False, debug=False)

    x_d = nc.dram_tensor("x", [bpc, FEAT], FR, kind="ExternalInput")
    pw_d = nc.dram_tensor("zu_w", [P, 2, PTOT], BF, kind="ExternalInput")
    d1_d = nc.dram_tensor("dec1_w", [P, 8, FEAT], FR, kind="ExternalInput")
    d2_d = nc.dram_tensor("dec2_w", [P, 7, FEAT], FR, kind="ExternalInput")
    ow_d = nc.dram_tensor("out_w", [P, 7, 10], FR, kind="ExternalInput")
    out_d = nc.dram_tensor("out", [bpc, 10], F32, kind="ExternalOutput")

    with tile.TileContext(nc) as tc:
        consts = tc.alloc_tile_pool(name="consts", bufs=1)
        hp = tc.alloc_tile_pool(name="h", bufs=2)
        vp = tc.alloc_tile_pool(name="vt", bufs=2)
        pkp = tc.alloc_tile_pool(name="pk", bufs=3)
        scp = tc.alloc_tile_pool(name="scr", bufs=4)
        mtp = tc.alloc_tile_pool(name="mt", bufs=8)
        sm = tc.alloc_tile_pool(name="small", bufs=3)
        wp = tc.alloc_tile_pool(name="w", bufs=2)
        wkd = tc.alloc_tile_pool(name="wkd", bufs=1)
        pp = tc.alloc_tile_pool(name="ps", bufs=2, space="PSUM")

        ident_f = consts.tile([P, P], F32)
        make_identity(nc, ident_f)
        ident_b = consts.tile([P, P], BF)
        nc.vector.tensor_copy(ident_b, ident_f)
        ident_r = consts.tile([P, P], FR)
        nc.vector.tensor_copy(ident_r, ident_f)
        ones_c = consts.tile([P, 512], F32)
        nc.vector.memset(ones_c, 1.0)
        pw = consts.tile([P, 2, PTOT], BF)
        nc.sync.dma_start(out=pw, in_=pw_d[:, :, :])
        # decoder weights DMA'd after group 0's x tiles (emitted in build
        # below) so the first group's compute isn't starved behind 6MB
        d1_w = consts.tile([P, 8, FEAT], FR)
        d2_w = consts.tile([P, 7, FEAT], FR)
        ow_w = consts.tile([P, 7, 10], FR)

        def load_tile(g, j):
            h0 = hp.tile([P, T, SLOT], FR, tag=f"h{j}")
            nc.sync.dma_start(
                out=h0[:, :, 0:FV],
                in_=x_d[ds(g * (nsub * P) + j * P, P), :].rearrange(
                    "p (t f) -> p t f", t=T
                ),
            )
            nc.gpsimd.tensor_copy(h0[:, :, 196:198], ones_c[:, 0 : 2 * T])
            hb = hp.tile([P, T, SLOT], BF, tag=f"hb{j}", bufs=1)
            nc.gpsimd.tensor_copy(hb, h0)
            vt1 = vp.tile([P, T, P], BF, tag=f"vt1{j}")
            vt2 = vp.tile([69, T, P], BF, tag=f"vt2{j}")
            t1_ps = pp.tile([P, T, P], BF, tag="t1ps", bufs=1)
            t2_ps = pp.tile([69, T, P], BF, tag="t2ps", bufs=1)
            for t in range(T):
                nc.tensor.transpose(t1_ps[:, t, :], hb[:, t, 0:P], ident_b)
                nc.tensor.transpose(t2_ps[:, t, :], hb[:, t, P : P + 69], ident_b)
            nc.scalar.copy(vt1, t1_ps)
            nc.scalar.copy(vt2, t2_ps)
            return h0, hb, vt1, vt2

        def proj(j, k, vt1, vt2):
            """PE projections for iteration k -> pk [128, 4, nc] bf16."""
            nco = NCOLS[k]
            off = POFF[k]
            pk = pkp.tile([P, T, NCMAX], BF, tag=f"pk{j}")
            for half in range(2):
                ps = pp.tile([P, 2, NCMAX], F32, tag="pkps", bufs=2)
                for sl in range(2):
                    s = half * 2 + sl
                    nc.tensor.matmul(
                        ps[:, sl, 0:nco], vt1[:, s, :],
                        pw[:, 0, off : off + nco], start=True, stop=False)
                    nc.tensor.matmul(
                        ps[:, sl, 0:nco], vt2[0:69, s, :],
                        pw[0:69, 1, off : off + nco], start=False, stop=True)
                nc.scalar.copy(
                    pk[:, 2 * half : 2 * half + 2, 0:nco], ps[:, :, 0:nco])
            return pk

        def dots(j, k, mtc, pk):
            """M_k[i,j] for all 16 slot pairs -> wave-mtc rows 4(j%2)+i."""
            r1 = RANKS[k] + 1
            pap = pk[:].ap[0]
            jw = j % 2
            if k == 0:
                # big r: fused stt (mult + f32 accumulate in one 1x pass)
                scr = scp.tile([P, 256], BF, tag="scr197", bufs=6)
                for idx in range(16):
                    i, jj = idx // 4, idx % 4
                    in0 = _ap(pk, [pap, [1, r1]],
                              offset_elems=i * NCMAX + r1)
                    in1 = _ap(pk, [pap, [1, r1]], offset_elems=jj * NCMAX)
                    nc.vector.scalar_tensor_tensor(
                        out=scr[:, 0:r1], in0=in0, scalar=1.0, in1=in1,
                        op0=ALU.mult, op1=ALU.mult,
                        accum_out=mtc[:, 4 * jw + i, jj : jj + 1])
            else:
                # small r: one bf16 2x tensor_tensor + one inner-axis reduce
                scr = scp.tile([P, T, T, 65], BF, tag="scr")
                in0 = _ap(pk, [pap, [NCMAX, 4], [0, 4], [1, r1]],
                          offset_elems=r1)
                in1 = _ap(pk, [pap, [0, 4], [NCMAX, 4], [1, r1]])
                nc.vector.tensor_tensor(
                    out=scr[:, :, :, 0:r1], in0=in0, in1=in1, op=ALU.mult)
                nc.vector.tensor_reduce(
                    out=mtc[:, 4 * jw : 4 * jw + 4, :], in_=scr[:, :, :, 0:r1],
                    axis=mybir.AxisListType.X, op=ALU.add)

        def serial_phase(k, w, mtc, c_prev):
            """Per-k 4x4 chain for one WAVE (2 tiles) in wide DVE ops over a
            [128, (j,t), s] layout (j in the wave): scores = C mt C^T ->
            e = exp -> C'u = e C -> C' = C'u / rowsum. Returns new C tile."""
            JT = 8   # (2 tiles) x (4 slots)
            JR = 32  # replicated size per tile pair
            if k == 0:
                s_t = mtc
            else:
                cap = c_prev[:].ap[0]
                # replicate C 4x -> crep[j, rep, s, jj] so every TT operand
                # stays within the ISA's 3-free-dim AP limit
                crep = sm.tile([P, 4 * JR], F32, tag=f"crep{w}", bufs=2)
                nc.vector.tensor_copy(
                    _ap(crep, [crep[:].ap[0], [64, 2], [16, 4], [1, 16]]),
                    _ap(c_prev, [cap, [16, 2], [0, 4], [1, 16]]))
                tt_eng = nc.vector if k >= 4 else nc.gpsimd
                scrd = scp.tile([P, JT, T, T], F32, tag="scrd", bufs=6)
                tt_eng.tensor_tensor(  # D[j,i,s] = sum_jj mt[j,i,jj] C[j,s,jj]
                    out=scrd,
                    in0=_ap(mtc, [mtc[:].ap[0], [4, JT], [0, 4], [1, 4]]),
                    in1=crep[:],
                    op=ALU.mult)
                dm = sm.tile([P, JT, T], F32, tag=f"dm{w}")
                nc.vector.tensor_reduce(
                    out=dm, in_=scrd, axis=mybir.AxisListType.X, op=ALU.add)
                drep = sm.tile([P, 4 * JR], F32, tag=f"drep{w}", bufs=2)
                nc.vector.tensor_copy(
                    _ap(drep, [drep[:].ap[0], [64, 2], [16, 4], [1, 16]]),
                    _ap(dm, [dm[:].ap[0], [16, 2], [0, 4], [1, 16]]))
                scrd2 = scp.tile([P, JT, T, T], F32, tag="scrd", bufs=6)
                tt_eng.tensor_tensor(  # S[j,t,s] = sum_i C[j,t,i] D[j,i,s]
                    out=scrd2,
                    in0=_ap(c_prev, [cap, [4, JT], [0, 4], [1, 4]]),
                    in1=_ap(drep, [drep[:].ap[0], [16, JT], [1, 4], [4, 4]]),
                    op=ALU.mult)
                s_t = sm.tile([P, JT, T], F32, tag=f"st{w}")
                nc.vector.tensor_reduce(
                    out=s_t, in_=scrd2, axis=mybir.AxisListType.X, op=ALU.add)
            e = sm.tile([P, JT, T], F32, tag=f"e{w}")
            nc.scalar.activation(e, s_t, AF.Exp)
            sums = sm.tile([P, JT], F32, tag=f"su{w}")
            nc.vector.reduce_sum(sums, e, axis=mybir.AxisListType.X)
            rec = sm.tile([P, JT], F32, tag=f"re{w}")
            nc.vector.reciprocal(rec, sums)
            if k == 0:
                cnum = e
            else:
                scrd3 = scp.tile([P, JT, T, T], F32, tag="scrd", bufs=6)
                tt_eng.tensor_tensor(  # C'u[j,t,jj] = sum_s e[j,t,s] C[j,s,jj]
                    out=scrd3,
                    in0=_ap(e, [e[:].ap[0], [4, JT], [0, 4], [1, 4]]),
                    in1=_ap(crep, [crep[:].ap[0], [16, JT], [1, 4], [4, 4]]),
                    op=ALU.mult)
                cnum = sm.tile([P, JT, T], F32, tag=f"cu{w}")
                nc.vector.tensor_reduce(
                    out=cnum, in_=scrd3, axis=mybir.AxisListType.X, op=ALU.add)
            c_new = sm.tile([P, JT, T], F32, tag=f"call{w}", bufs=3)
            nc.vector.scalar_tensor_tensor(
                out=c_new, in0=cnum, scalar=1.0,
                in1=_ap(rec, [rec[:].ap[0], [1, JT], [0, T]]),
                op0=ALU.mult, op1=ALU.mult)
            return c_new

        def recon(j, h0, c8):
            """w[:, t, :] = sum_s C8[t,s] * h0[:, s, :] (ones col rides along)."""
            w = wp.tile([P, T, SLOT], FR, tag=f"w{j}", bufs=1)
            jw = j % 2
            for t in range(T):
                nc.scalar.activation(
                    w[:, t, :], h0[:, 0, :], AF.Copy,
                    scale=c8[:, 4 * jw + t, 0:1])
            for t in range(3):
                for s in range(1, T):
                    nc.vector.scalar_tensor_tensor(
                        out=w[:, t, :], in0=h0[:, s, :],
                        scalar=c8[:, 4 * jw + t, s : s + 1], in1=w[:, t, :],
                        op0=ALU.mult, op1=ALU.add)
            for t in range(3, T):
                pct = wp.tile([P, SLOT], F32, tag="pct", bufs=2)
                for s in range(1, T):
                    nc.gpsimd.tensor_scalar_mul(
                        pct, h0[:, s, :], c8[:, 4 * jw + t, s : s + 1])
                    nc.gpsimd.tensor_add(w[:, t, :], w[:, t, :], pct)
            return w

        def decoder(ws2, g, wave):
            """Decoder over one wave of 2 tiles (N = 256 wide matmuls)."""
            W = 2 * P
            ht1 = wkd.tile([P, T, W], FR, tag=f"ht1w{wave}")
            ht2 = wkd.tile([69, T, W], FR, tag=f"ht2w{wave}")
            for t in range(T):
                t1_ps = pp.tile([P, W], FR, tag="dt1ps", bufs=1)
                t2_ps = pp.tile([69, W], FR, tag="dt2ps", bufs=1)
                for j in range(2):
                    nc.tensor.transpose(
                        t1_ps[:, j * P : (j + 1) * P], ws2[j][:, t, 0:P], ident_r
                    )
                    nc.tensor.transpose(
                        t2_ps[:, j * P : (j + 1) * P], ws2[j][:, t, P : P + 69],
                        ident_r
                    )
                nc.scalar.copy(ht1[:, t, :], t1_ps)
                nc.vector.tensor_copy(ht2[:, t, :], t2_ps)

            # dec1 = relu(Wd1~ @ w.T + bd1~), feature-major, 7 M-chunks
            d1a = wkd.tile([P, 6, W], FR, tag=f"d1aw{wave}")
            d1b = wkd.tile([17, W], FR, tag=f"d1bw{wave}")
            nc.vector.tensor_copy(d1b, ones_c[0:17, 0:W])
            for m in range(7):
                mw = min(P, FEAT - m * P)
                mp = pp.tile([P, W], F32, tag="mp")
                msl = slice(m * P, m * P + mw)
                for t in range(T):
                    nc.tensor.matmul(mp[0:mw, :], d1_w[:, t, msl], ht1[:, t, :],
                                     start=(t == 0), stop=False)
                for t in range(T):
                    nc.tensor.matmul(mp[0:mw, :], d1_w[0:69, 4 + t, msl],
                                     ht2[:, t, :], start=False, stop=(t == 3))
                if m < 6:
                    nc.scalar.activation(d1a[:, m, :], mp, AF.Relu)
                else:
                    nc.scalar.activation(d1b[0:16, :], mp[0:16, :], AF.Relu)

            # dec2 = Wd2 @ relu1 + bd2, feature-major
            d2a = wkd.tile([P, 6, W], FR, tag=f"d2aw{wave}")
            d2b = wkd.tile([17, W], FR, tag=f"d2bw{wave}")
            nc.vector.tensor_copy(d2b, ones_c[0:17, 0:W])
            for m in range(7):
                mw = min(P, FEAT - m * P)
                mp = pp.tile([P, W], F32, tag="mp")
                msl = slice(m * P, m * P + mw)
                for c in range(6):
                    nc.tensor.matmul(mp[0:mw, :], d2_w[:, c, msl], d1a[:, c, :],
                                     start=(c == 0), stop=False)
                nc.tensor.matmul(mp[0:mw, :], d2_w[0:17, 6, msl], d1b,
                                 start=False, stop=True)
                if m < 6:
                    nc.scalar.copy(d2a[:, m, :], mp)
                else:
                    nc.scalar.copy(d2b[0:16, :], mp[0:16, :])

            # logits + softmax per subtile
            for j in range(2):
                jsl = slice(j * P, (j + 1) * P)
                lgt = pp.tile([P, W], F32, tag="mp")
                lg = lgt[:, 0:10]
                for c in range(6):
                    nc.tensor.matmul(lg, d2a[:, c, jsl], ow_w[:, c, :],
                                     start=(c == 0), stop=False)
                nc.tensor.matmul(lg, d2b[:, jsl], ow_w[0:17, 6, :],
                                 start=False, stop=True)
                mx = sm.tile([P, 1], F32, tag="mx")
                nc.vector.reduce_max(mx, lg, axis=mybir.AxisListType.X)
                nmx = sm.tile([P, 1], F32, tag="nmx")
                nc.vector.tensor_scalar_mul(nmx, mx, -1.0)
                e10 = sm.tile([P, 10], F32, tag="e10")
                s10 = sm.tile([P, 1], F32, tag="s10")
                nc.scalar.activation(e10, lg, AF.Exp, bias=nmx, accum_out=s10)
                r10 = sm.tile([P, 1], F32, tag="r10")
                nc.vector.reciprocal(r10, s10)
                o10 = sm.tile([P, 10], F32, tag="o10")
                nc.vector.tensor_scalar_mul(o10, e10, r10)
                nc.sync.dma_start(
                    out=out_d[ds(g * (nsub * P) + (2 * wave + j) * P, P), :],
                    in_=o10
                )

        def body(g, preloaded=None):
            h0s, hbs, cs = [], [], [None, None]
            vts = []
            # k=0 proj+dots interleaved per tile so the first tile's chain
            # races ahead of later tiles' loads
            mtc0 = [mtp.tile([P, 8, T], F32, tag=f"mtk{w}", bufs=3,
                             name=f"mt0w{w}") for w in range(2)]
            for j in range(nsub):
                h0, hb, vt1, vt2 = (
                    load_tile(g, j) if preloaded is None else preloaded[j]
                )
                h0s.append(h0)
                hbs.append(hb)
                vts.append((vt1, vt2))
                pk = proj(j, 0, vt1, vt2)
                dots(j, 0, mtc0[j // 2], pk)
            mtk = [mtc0]  # per-wave Mt tiles per k
            for k in range(1, 8):
                mtcs = [mtp.tile([P, 8, T], F32, tag=f"mtk{w}", bufs=3,
                                 name=f"mt{k}w{w}") for w in range(2)]
                for j in range(nsub):
                    pk = proj(j, k, *vts[j])
                    dots(j, k, mtcs[j // 2], pk)
                mtk.append(mtcs)
                # serial chains run one k behind so the independent dots of
                # the next k fill engine gaps while the chains ping-pong
                for w in range(2):
                    cs[w] = serial_phase(k - 1, w, mtk[k - 1][w], cs[w])
            # wave 0 finishes first and its decoder overlaps wave 1's tail
            cs[0] = serial_phase(7, 0, mtk[7][0], cs[0])
            ws0 = [recon(0, h0s[0], cs[0]), recon(1, h0s[1], cs[0])]
            cs[1] = serial_phase(7, 1, mtk[7][1], cs[1])
            decoder(ws0, g, 0)
            ws1 = [recon(2, h0s[2], cs[1]), recon(3, h0s[3], cs[1])]
            decoder(ws1, g, 1)

        # group 0 loads first so its x DMAs precede the 6MB of decoder
        # weights; weight DMAs ride the Act queue so their triggers land
        # after the x transfers are in flight
        pre0 = [load_tile(0, j) for j in range(nsub)]
        nc.sync.dma_start(out=d1_w, in_=d1_d[:, :, :])
        nc.sync.dma_start(out=d2_w, in_=d2_d[:, :, :])
        nc.sync.dma_start(out=ow_w, in_=ow_d[:, :, :])
        body(0, preloaded=pre0)
        if ngroups > 1:
            with tc.For_i(1, ngroups, 1) as g:
                body(g)
        for _pool in (pp, wkd, wp, sm, mtp, scp, pkp, vp, hp, consts):
            _pool.release()

    nc.compile()
    return nc


def pack_weights(W1, b1, W2, b2, W3, b3, Wd1, bd1, Wd2, bd2, Wo, bo):
    f64 = np.float64
    W1, b1, W2, b2, W3, b3 = (np.asarray(t, f64) for t in (W1, b1, W2, b2, W3, b3))
    G = W1.T @ W2
    a = W2.T @ b1

    A = np.eye(FV)
    m = np.zeros(FV)
    pw = np.zeros((P, 2, PTOT), np.float32)
    for k in range(8):
        Gk = A.T @ G @ A
        ak = A.T @ (G.T @ m + a)
        nco = NCOLS[k]
        Wk = np.zeros((197, nco), f64)
        r = RANKS[k]
        r1 = r + 1
        U, S, Vh = np.linalg.svd(Gk)
        Wk[:FV, :r] = (np.diag(S[:r]) @ Vh[:r]).T
        Wk[:FV, r] = ak
        Wk[:FV, r1 : r1 + r] = U[:, :r]
        Wk[FV, r1 + r] = 1.0
        off = POFF[k]
        pw[:, 0, off : off + nco] = Wk[0:128]
        pw[0:69, 1, off : off + nco] = Wk[128:197]
        A = W3 @ A
        m = W3 @ m + b3
    A8, m8 = A, m

    # fold W3^8 / m8 into the first decoder layer
    BD = np.zeros((FEAT, FEAT), f64)
    mm = np.zeros(FEAT, f64)
    for t in range(T):
        BD[t * FV : (t + 1) * FV, t * FV : (t + 1) * FV] = A8
        mm[t * FV : (t + 1) * FV] = m8
    Wd1f = np.asarray(Wd1, f64) @ BD
    bd1f = np.asarray(bd1, f64) + np.asarray(Wd1, f64) @ mm

    d1 = np.zeros((P, 8, FEAT), np.float32)
    W1T = Wd1f.T  # [784 f_in, 784 j]
    for t in range(T):
        d1[:, t, :] = W1T[t * FV : t * FV + P, :]
        d1[0:68, 4 + t, :] = W1T[t * FV + P : (t + 1) * FV, :]
    d1[68, 4, :] = bd1f

    d2 = np.zeros((P, 7, FEAT), np.float32)
    W2T = np.asarray(Wd2, f64).T
    for cidx in range(6):
        d2[:, cidx, :] = W2T[cidx * P : (cidx + 1) * P, :]
    d2[0:16, 6, :] = W2T[768:784, :]
    d2[16, 6, :] = np.asarray(bd2, f64)

    ow = np.zeros((P, 7, 10), np.float32)
    WoT = np.asarray(Wo, f64).T
    for cidx in range(6):
        ow[:, cidx, :] = WoT[cidx * P : (cidx + 1) * P, :]
    ow[0:16, 6, :] = WoT[768:784, :]
    ow[16, 6, :] = np.asarray(bo, f64)
    return pw.astype(ml_dtypes.bfloat16), d1, d2, ow


_NC_CACHE = {}


def kernel(**inputs):
    x = np.ascontiguousarray(np.asarray(inputs["x"], np.float32))
    zu, d1, d2, ow = pack_weights(
        inputs["W1"], inputs["b1"], inputs["W2"], inputs["b2"], inputs["W3"],
        inputs["b3"], inputs["Wd1"], inputs["bd1"], inputs["Wd2"],
        inputs["bd2"], inputs["Wo"], inputs["bo"],
    )
    if "nc" not in _NC_CACHE:
        _NC_CACHE["nc"] = build(4, 8)
    nc = _NC_CACHE["nc"]
    bpc = B // NCORES
    in_maps = [
        {
            "x": x[c * bpc : (c + 1) * bpc],
            "zu_w": zu,
            "dec1_w": d1,
            "dec2_w": d2,
            "out_w": ow,
        }
        for c in range(NCORES)
    ]
    res = run_bass_kernel_spmd(nc, in_maps, core_ids=list(range(NCORES)))
    return np.concatenate([res.results[c]["out"] for c in range(NCORES)], axis=0)


# revision 63
# speedup vs baseline: 2.7334x; 1.0133x over previous
"""Trainium2 Bass kernel for nn_CapsuleNeuralNetworkV2 (8 cores, data-parallel).

Reference math (per sample, 8 capsule iterations then decoder):
  v = h.reshape(4, 196); q,k,u = affine(v); scores = q k^T;
  P = softmax(scores); h' = P u;  dec = relu(h Wd1^T+bd1) Wd2^T+bd2;
  out = softmax(dec Wo^T + bo).

Restructuring (host-side algebra):
  Since each P has rows summing to 1, the state stays in the span of the 4
  initial slots: v^(k) = W3^k w^(k) + m_k with w^(k) = C^(k) V (C is a
  per-sample 4x4 convex-coefficient matrix, V the initial slots).
  scores^(k)[t,s] = C[t] M_k C[s]^T (mod per-t constants that cancel in
  softmax), where M_k[i,j] = v_i.(G_k v_j) + a_k.v_j depends only on the
  INITIAL slots: G_k = (W3^k)^T G W3^k, G = W1^T W2,
  a_k = (W3^k)^T (G^T m_k + W2^T b1).  G_k is numerically low-rank for k>=1
  (powers of a random matrix), so M_k is computed from rank-r_k SVD
  projections p_i = U_r^T v_i, q_j = (S V_r^T) v_j: M[i,j] ~ p_i.q_j + r_j.
  Per iteration only the tiny 4x4 chain is sequential:
  scores = C M C^T -> softmax -> C' = P C.  All projections/M_k are
  C-independent and pipeline on PE/Act/DVE ahead of the chain.
  Final w^(8) = C^(8) V; W3^8/m_8 are folded into Wd1/bd1 on the host.

Engines: PE transposes V once per tile + small bf16 projection matmuls +
decoder; DVE/Pool share the per-sample dot products and the 4x4 chain; Act
does PSUM evacuation, exp, and decoder activations.
"""

import numpy as np
import ml_dtypes

import concourse.bass as bass
import concourse.tile as tile
from concourse import bacc, mybir
from concourse.bass import ds
from concourse.bass_utils import run_bass_kernel_spmd
from concourse.masks import make_identity

FR = mybir.dt.float32r
BF = mybir.dt.bfloat16
F32 = mybir.dt.float32
AF = mybir.ActivationFunctionType
ALU = mybir.AluOpType

B = 32768
NCORES = 8
P = 128
T = 4
FV = 196
FEAT = 784
SLOT = 198  # h slot: 196 data + ones col (196) + spare (197)

RANKS = [96, 48, 32, 24, 16, 12, 8, 8]
NCOLS = [2 * (r + 1) for r in RANKS]  # proj cols per slot per k
POFF = [0]
for _n in NCOLS:
    POFF.append(POFF[-1] + _n)
PTOT = POFF[-1]
NCMAX = max(NCOLS)


def _ap(t, dims, offset_elems=0):
    """Hand-built AP over a tile's tensor: dims = [[step, count], ...]."""
    a = t[:] if hasattr(t, "tile") or not isinstance(t, bass.AP) else t
    return bass.AP(tensor=a.tensor, offset=a.offset + offset_elems, ap=dims)


def build(nsub=4, ngroups=8):
    """One NeuronCore program processing nsub*ngroups*128 samples."""
    bpc = nsub * ngroups * P
    nc = bacc.Bacc("TRN2", target_bir_lowering=False, debug=False)

    x_d = nc.dram_tensor("x", [bpc, FEAT], FR, kind="ExternalInput")
    pw_d = nc.dram_tensor("zu_w", [P, 2, PTOT], BF, kind="ExternalInput")
    d1_d = nc.dram_tensor("dec1_w", [P, 8, FEAT], FR, kind="ExternalInput")
    d2_d = nc.dram_tensor("dec2_w", [P, 7, FEAT], FR, kind="ExternalInput")
    ow_d = nc.dram_tensor("out_w", [P, 7, 10], FR, kind="ExternalInput")
    out_d = nc.dram_tensor("out", [bpc, 10], F32, kind="ExternalOutput")

    with tile.TileContext(nc) as tc:
        consts = tc.alloc_tile_pool(name="consts", bufs=1)
        hp = tc.alloc_tile_pool(name="h", bufs=2)
        vp = tc.alloc_tile_pool(name="vt", bufs=2)
        pkp = tc.alloc_tile_pool(name="pk", bufs=3)
        scp = tc.alloc_tile_pool(name="scr", bufs=4)
        mtp = tc.alloc_tile_pool(name="mt", bufs=8)
        sm = tc.alloc_tile_pool(name="small", bufs=3)
        wp = tc.alloc_tile_pool(name="w", bufs=2)
        wkd = tc.alloc_tile_pool(name="wkd", bufs=1)
        pp = tc.alloc_tile_pool(name="ps", bufs=2, space="PSUM")

        ident_f = consts.tile([P, P], F32)
        make_identity(nc, ident_f)
        ident_r = consts.tile([P, P], FR)
        nc.vector.tensor_copy(ident_r, ident_f)
        ones_c = consts.tile([P, 512], F32)
        nc.vector.memset(ones_c, 1.0)
        pw = consts.tile([P, 2, PTOT], BF)
        nc.sync.dma_start(out=pw, in_=pw_d[:, :, :])
        # decoder weights DMA'd after group 0's x tiles (emitted in build
        # below) so the first group's compute isn't starved behind 6MB
        d1_w = consts.tile([P, 8, FEAT], FR)
        d2_w = consts.tile([P, 7, FEAT], FR)
        ow_w = consts.tile([P, 7, 10], FR)

        def load_tile(g, j):
            h0 = hp.tile([P, T, SLOT], FR, tag=f"h{j}")
            nc.sync.dma_start(
                out=h0[:, :, 0:FV],
                in_=x_d[ds(g * (nsub * P) + j * P, P), :].rearrange(
                    "p (t f) -> p t f", t=T
                ),
            )
            nc.gpsimd.tensor_copy(h0[:, :, 196:198], ones_c[:, 0 : 2 * T])
            vt1 = vp.tile([P, T, P], BF, tag=f"vt1{j}")
            vt2 = vp.tile([69, T, P], BF, tag=f"vt2{j}")
            t1_ps = pp.tile([P, T, P], FR, tag="t1ps", bufs=1)
            t2_ps = pp.tile([69, T, P], FR, tag="t2ps", bufs=1)
            for t in range(T):
                nc.tensor.transpose(t1_ps[:, t, :], h0[:, t, 0:P], ident_r)
                nc.tensor.transpose(t2_ps[:, t, :], h0[:, t, P : P + 69], ident_r)
            nc.scalar.copy(vt1, t1_ps)
            nc.scalar.copy(vt2, t2_ps)
            return h0, None, vt1, vt2

        def proj(j, k, vt1, vt2):
            """PE projections for iteration k -> pk [128, 4, nc] bf16."""
            nco = NCOLS[k]
            off = POFF[k]
            pk = pkp.tile([P, T, NCMAX], BF, tag=f"pk{j}")
            for half in range(2):
                ps = pp.tile([P, 2, NCMAX], F32, tag="pkps", bufs=2)
                for sl in range(2):
                    s = half * 2 + sl
                    nc.tensor.matmul(
                        ps[:, sl, 0:nco], vt1[:, s, :],
                        pw[:, 0, off : off + nco], start=True, stop=False)
                    nc.tensor.matmul(
                        ps[:, sl, 0:nco], vt2[0:69, s, :],
                        pw[0:69, 1, off : off + nco], start=False, stop=True)
                nc.scalar.copy(
                    pk[:, 2 * half : 2 * half + 2, 0:nco], ps[:, :, 0:nco])
            return pk

        def dots(j, k, mtc, pk):
            """M_k[i,j] for all 16 slot pairs -> wave-mtc rows 4(j%2)+i."""
            r1 = RANKS[k] + 1
            pap = pk[:].ap[0]
            jw = j % 2
            if k == 0:
                # big r: fused stt (mult + f32 accumulate in one 1x pass)
                scr = scp.tile([P, 256], BF, tag="scr197", bufs=6)
                for idx in range(16):
                    i, jj = idx // 4, idx % 4
                    in0 = _ap(pk, [pap, [1, r1]],
                              offset_elems=i * NCMAX + r1)
                    in1 = _ap(pk, [pap, [1, r1]], offset_elems=jj * NCMAX)
                    nc.vector.scalar_tensor_tensor(
                        out=scr[:, 0:r1], in0=in0, scalar=1.0, in1=in1,
                        op0=ALU.mult, op1=ALU.mult,
                        accum_out=mtc[:, 4 * jw + i, jj : jj + 1])
            else:
                # small r: one bf16 2x tensor_tensor + one inner-axis reduce
                scr = scp.tile([P, T, T, 65], BF, tag="scr")
                in0 = _ap(pk, [pap, [NCMAX, 4], [0, 4], [1, r1]],
                          offset_elems=r1)
                in1 = _ap(pk, [pap, [0, 4], [NCMAX, 4], [1, r1]])
                nc.vector.tensor_tensor(
                    out=scr[:, :, :, 0:r1], in0=in0, in1=in1, op=ALU.mult)
                nc.vector.tensor_reduce(
                    out=mtc[:, 4 * jw : 4 * jw + 4, :], in_=scr[:, :, :, 0:r1],
                    axis=mybir.AxisListType.X, op=ALU.add)

        def serial_phase(k, w, mtc, c_prev):
            """Per-k 4x4 chain for one WAVE (2 tiles) in wide DVE ops over a
            [128, (j,t), s] layout (j in the wave): scores = C mt C^T ->
            e = exp -> C'u = e C -> C' = C'u / rowsum. Returns new C tile."""
            JT = 8   # (2 tiles) x (4 slots)
            JR = 32  # replicated size per tile pair
            if k == 0:
                s_t = mtc
            else:
                cap = c_prev[:].ap[0]
                # replicate C 4x -> crep[j, rep, s, jj] so every TT operand
                # stays within the ISA's 3-free-dim AP limit
                crep = sm.tile([P, 4 * JR], F32, tag=f"crep{w}", bufs=2)
                nc.gpsimd.tensor_copy(
                    _ap(crep, [crep[:].ap[0], [64, 2], [16, 4], [1, 16]]),
                    _ap(c_prev, [cap, [16, 2], [0, 4], [1, 16]]))
                tt_eng = nc.vector if k >= 4 else nc.gpsimd
                scrd = scp.tile([P, JT, T, T], F32, tag="scrd", bufs=6)
                tt_eng.tensor_tensor(  # D[j,i,s] = sum_jj mt[j,i,jj] C[j,s,jj]
                    out=scrd,
                    in0=_ap(mtc, [mtc[:].ap[0], [4, JT], [0, 4], [1, 4]]),
                    in1=crep[:],
                    op=ALU.mult)
                dm = sm.tile([P, JT, T], F32, tag=f"dm{w}")
                nc.vector.tensor_reduce(
                    out=dm, in_=scrd, axis=mybir.AxisListType.X, op=ALU.add)
                drep = sm.tile([P, 4 * JR], F32, tag=f"drep{w}", bufs=2)
                nc.gpsimd.tensor_copy(
                    _ap(drep, [drep[:].ap[0], [64, 2], [16, 4], [1, 16]]),
                    _ap(dm, [dm[:].ap[0], [16, 2], [0, 4], [1, 16]]))
                scrd2 = scp.tile([P, JT, T, T], F32, tag="scrd", bufs=6)
                tt_eng.tensor_tensor(  # S[j,t,s] = sum_i C[j,t,i] D[j,i,s]
                    out=scrd2,
                    in0=_ap(c_prev, [cap, [4, JT], [0, 4], [1, 4]]),
                    in1=_ap(drep, [drep[:].ap[0], [16, JT], [1, 4], [4, 4]]),
                    op=ALU.mult)
                s_t = sm.tile([P, JT, T], F32, tag=f"st{w}")
                nc.vector.tensor_reduce(
                    out=s_t, in_=scrd2, axis=mybir.AxisListType.X, op=ALU.add)
            e = sm.tile([P, JT, T], F32, tag=f"e{w}")
            nc.scalar.activation(e, s_t, AF.Exp)
            sums = sm.tile([P, JT], F32, tag=f"su{w}")
            nc.vector.reduce_sum(sums, e, axis=mybir.AxisListType.X)
            rec = sm.tile([P, JT], F32, tag=f"re{w}")
            nc.vector.reciprocal(rec, sums)
            if k == 0:
                cnum = e
            else:
                scrd3 = scp.tile([P, JT, T, T], F32, tag="scrd", bufs=6)
                tt_eng.tensor_tensor(  # C'u[j,t,jj] = sum_s e[j,t,s] C[j,s,jj]
                    out=scrd3,
                    in0=_ap(e, [e[:].ap[0], [4, JT], [0, 4], [1, 4]]),
                    in1=_ap(crep, [crep[:].ap[0], [16, JT], [1, 4], [4, 4]]),
                    op=ALU.mult)
                cnum = sm.tile([P, JT, T], F32, tag=f"cu{w}")
                nc.vector.tensor_reduce(
                    out=cnum, in_=scrd3, axis=mybir.AxisListType.X, op=ALU.add)
            c_new = sm.tile([P, JT, T], F32, tag=f"call{w}", bufs=3)
            nc.vector.scalar_tensor_tensor(
                out=c_new, in0=cnum, scalar=1.0,
                in1=_ap(rec, [rec[:].ap[0], [1, JT], [0, T]]),
                op0=ALU.mult, op1=ALU.mult)
            return c_new

        def recon(j, h0, c8):
            """w[:, t, :] = sum_s C8[t,s] * h0[:, s, :] (ones col rides along)."""
            w = wp.tile([P, T, SLOT], FR, tag=f"w{j}", bufs=1)
            jw = j % 2
            for t in range(T):
                nc.scalar.activation(
                    w[:, t, :], h0[:, 0, :], AF.Copy,
                    scale=c8[:, 4 * jw + t, 0:1])
            for t in range(3):
                for s in range(1, T):
                    nc.vector.scalar_tensor_tensor(
                        out=w[:, t, :], in0=h0[:, s, :],
                        scalar=c8[:, 4 * jw + t, s : s + 1], in1=w[:, t, :],
                        op0=ALU.mult, op1=ALU.add)
            for t in range(3, T):
                pct = wp.tile([P, SLOT], F32, tag="pct", bufs=2)
                for s in range(1, T):
                    nc.gpsimd.tensor_scalar_mul(
                        pct, h0[:, s, :], c8[:, 4 * jw + t, s : s + 1])
                    nc.gpsimd.tensor_add(w[:, t, :], w[:, t, :], pct)
            return w

        def decoder(ws2, g, wave):
            """Decoder over one wave of 2 tiles (N = 256 wide matmuls)."""
            W = 2 * P
            ht1 = wkd.tile([P, T, W], FR, tag=f"ht1w{wave}")
            ht2 = wkd.tile([69, T, W], FR, tag=f"ht2w{wave}")
            for t in range(T):
                t1_ps = pp.tile([P, W], FR, tag="dt1ps", bufs=1)
                t2_ps = pp.tile([69, W], FR, tag="dt2ps", bufs=1)
                for j in range(2):
                    nc.tensor.transpose(
                        t1_ps[:, j * P : (j + 1) * P], ws2[j][:, t, 0:P], ident_r
                    )
                    nc.tensor.transpose(
                        t2_ps[:, j * P : (j + 1) * P], ws2[j][:, t, P : P + 69],
                        ident_r
                    )
                nc.scalar.copy(ht1[:, t, :], t1_ps)
                nc.vector.tensor_copy(ht2[:, t, :], t2_ps)

            # dec1 = relu(Wd1~ @ w.T + bd1~), feature-major, 7 M-chunks
            d1a = wkd.tile([P, 6, W], FR, tag=f"d1aw{wave}")
            d1b = wkd.tile([17, W], FR, tag=f"d1bw{wave}")
            nc.vector.tensor_copy(d1b, ones_c[0:17, 0:W])
            for m in range(7):
                mw = min(P, FEAT - m * P)
                mp = pp.tile([P, W], F32, tag="mp")
                msl = slice(m * P, m * P + mw)
                for t in range(T):
                    nc.tensor.matmul(mp[0:mw, :], d1_w[:, t, msl], ht1[:, t, :],
                                     start=(t == 0), stop=False)
                for t in range(T):
                    nc.tensor.matmul(mp[0:mw, :], d1_w[0:69, 4 + t, msl],
                                     ht2[:, t, :], start=False, stop=(t == 3))
                if m < 6:
                    nc.scalar.activation(d1a[:, m, :], mp, AF.Relu)
                else:
                    nc.scalar.activation(d1b[0:16, :], mp[0:16, :], AF.Relu)

            # dec2 = Wd2 @ relu1 + bd2, feature-major
            d2a = wkd.tile([P, 6, W], FR, tag=f"d2aw{wave}")
            d2b = wkd.tile([17, W], FR, tag=f"d2bw{wave}")
            nc.vector.tensor_copy(d2b, ones_c[0:17, 0:W])
            for m in range(7):
                mw = min(P, FEAT - m * P)
                mp = pp.tile([P, W], F32, tag="mp")
                msl = slice(m * P, m * P + mw)
                for c in range(6):
                    nc.tensor.matmul(mp[0:mw, :], d2_w[:, c, msl], d1a[:, c, :],
                                     start=(c == 0), stop=False)
                nc.tensor.matmul(mp[0:mw, :], d2_w[0:17, 6, msl], d1b,
                                 start=False, stop=True)
                if m < 6:
                    nc.scalar.copy(d2a[:, m, :], mp)
                else:
                    nc.scalar.copy(d2b[0:16, :], mp[0:16, :])

            # logits + softmax per subtile
            for j in range(2):
                jsl = slice(j * P, (j + 1) * P)
                lgt = pp.tile([P, W], F32, tag="mp")
                lg = lgt[:, 0:10]
                for c in range(6):
                    nc.tensor.matmul(lg, d2a[:, c, jsl], ow_w[:, c, :],
                                     start=(c == 0), stop=False)
                nc.tensor.matmul(lg, d2b[:, jsl], ow_w[0:17, 6, :],
                                 start=False, stop=True)
                mx = sm.tile([P, 1], F32, tag="mx")
                nc.vector.reduce_max(mx, lg, axis=mybir.AxisListType.X)
                nmx = sm.tile([P, 1], F32, tag="nmx")
                nc.vector.tensor_scalar_mul(nmx, mx, -1.0)
                e10 = sm.tile([P, 10], F32, tag="e10")
                s10 = sm.tile([P, 1], F32, tag="s10")
                nc.scalar.activation(e10, lg, AF.Exp, bias=nmx, accum_out=s10)
                r10 = sm.tile([P, 1], F32, tag="r10")
                nc.vector.reciprocal(r10, s10)
                o10 = sm.tile([P, 10], F32, tag="o10")
                nc.vector.tensor_scalar_mul(o10, e10, r10)
                nc.sync.dma_start(
                    out=out_d[ds(g * (nsub * P) + (2 * wave + j) * P, P), :],
                    in_=o10
                )

        def body(g, preloaded=None):
            h0s, hbs, cs = [], [], [None, None]
            vts = []
            # k=0 proj+dots interleaved per tile so the first tile's chain
            # races ahead of later tiles' loads
            mtc0 = [mtp.tile([P, 8, T], F32, tag=f"mtk{w}", bufs=3,
                             name=f"mt0w{w}") for w in range(2)]
            for j in range(nsub):
                h0, hb, vt1, vt2 = (
                    load_tile(g, j) if preloaded is None else preloaded[j]
                )
                h0s.append(h0)
                hbs.append(hb)
                vts.append((vt1, vt2))
                pk = proj(j, 0, vt1, vt2)
                dots(j, 0, mtc0[j // 2], pk)
            mtk = [mtc0]  # per-wave Mt tiles per k
            for k in range(1, 8):
                mtcs = [mtp.tile([P, 8, T], F32, tag=f"mtk{w}", bufs=3,
                                 name=f"mt{k}w{w}") for w in range(2)]
                for j in range(nsub):
                    pk = proj(j, k, *vts[j])
                    dots(j, k, mtcs[j // 2], pk)
                mtk.append(mtcs)
                # serial chains run one k behind so the independent dots of
                # the next k fill engine gaps while the chains ping-pong
                for w in range(2):
                    cs[w] = serial_phase(k - 1, w, mtk[k - 1][w], cs[w])
            # wave 0 finishes first and its decoder overlaps wave 1's tail
            cs[0] = serial_phase(7, 0, mtk[7][0], cs[0])
            ws0 = [recon(0, h0s[0], cs[0]), recon(1, h0s[1], cs[0])]
            cs[1] = serial_phase(7, 1, mtk[7][1], cs[1])
            decoder(ws0, g, 0)
            ws1 = [recon(2, h0s[2], cs[1]), recon(3, h0s[3], cs[1])]
            decoder(ws1, g, 1)

        # group 0 loads first so its x DMAs precede the 6MB of decoder
        # weights; weight DMAs ride the Act queue so their triggers land
        # after the x transfers are in flight
        pre0 = [load_tile(0, j) for j in range(nsub)]
        nc.sync.dma_start(out=d1_w, in_=d1_d[:, :, :])
        nc.sync.dma_start(out=d2_w, in_=d2_d[:, :, :])
        nc.sync.dma_start(out=ow_w, in_=ow_d[:, :, :])
        body(0, preloaded=pre0)
        if ngroups > 1:
            with tc.For_i(1, ngroups, 1) as g:
                body(g)
        for _pool in (pp, wkd, wp, sm, mtp, scp, pkp, vp, hp, consts):
            _pool.release()

    nc.compile()
    return nc


def pack_weights(W1, b1, W2, b2, W3, b3, Wd1, bd1, Wd2, bd2, Wo, bo):
    f64 = np.float64
    W1, b1, W2, b2, W3, b3 = (np.asarray(t, f64) for t in (W1, b1, W2, b2, W3, b3))
    G = W1.T @ W2
    a = W2.T @ b1

    A = np.eye(FV)
    m = np.zeros(FV)
    pw = np.zeros((P, 2, PTOT), np.float32)
    for k in range(8):
        Gk = A.T @ G @ A
        ak = A.T @ (G.T @ m + a)
        nco = NCOLS[k]
        Wk = np.zeros((197, nco), f64)
        r = RANKS[k]
        r1 = r + 1
        U, S, Vh = np.linalg.svd(Gk)
        Wk[:FV, :r] = (np.diag(S[:r]) @ Vh[:r]).T
        Wk[:FV, r] = ak
        Wk[:FV, r1 : r1 + r] = U[:, :r]
        Wk[FV, r1 + r] = 1.0
        off = POFF[k]
        pw[:, 0, off : off + nco] = Wk[0:128]
        pw[0:69, 1, off : off + nco] = Wk[128:197]
        A = W3 @ A
        m = W3 @ m + b3
    A8, m8 = A, m

    # fold W3^8 / m8 into the first decoder layer
    BD = np.zeros((FEAT, FEAT), f64)
    mm = np.zeros(FEAT, f64)
    for t in range(T):
        BD[t * FV : (t + 1) * FV, t * FV : (t + 1) * FV] = A8
        mm[t * FV : (t + 1) * FV] = m8
    Wd1f = np.asarray(Wd1, f64) @ BD
    bd1f = np.asarray(bd1, f64) + np.asarray(Wd1, f64) @ mm

    d1 = np.zeros((P, 8, FEAT), np.float32)
    W1T = Wd1f.T  # [784 f_in, 784 j]
    for t in range(T):
        d1[:, t, :] = W1T[t * FV : t * FV + P, :]
        d1[0:68, 4 + t, :] = W1T[t * FV + P : (t + 1) * FV, :]
    d1[68, 4, :] = bd1f

    d2 = np.zeros((P, 7, FEAT), np.float32)
    W2T = np.asarray(Wd2, f64).T
    for cidx in range(6):
        d2[:, cidx, :] = W2T[cidx * P : (cidx + 1) * P, :]
    d2[0:16, 6, :] = W2T[768:784, :]
    d2[16, 6, :] = np.asarray(bd2, f64)

    ow = np.zeros((P, 7, 10), np.float32)
    WoT = np.asarray(Wo, f64).T
    for cidx in range(6):
        ow[:, cidx, :] = WoT[cidx * P : (cidx + 1) * P, :]
    ow[0:16, 6, :] = WoT[768:784, :]
    ow[16, 6, :] = np.asarray(bo, f64)
    return pw.astype(ml_dtypes.bfloat16), d1, d2, ow


_NC_CACHE = {}


def kernel(**inputs):
    x = np.ascontiguousarray(np.asarray(inputs["x"], np.float32))
    zu, d1, d2, ow = pack_weights(
        inputs["W1"], inputs["b1"], inputs["W2"], inputs["b2"], inputs["W3"],
        inputs["b3"], inputs["Wd1"], inputs["bd1"], inputs["Wd2"],
        inputs["bd2"], inputs["Wo"], inputs["bo"],
    )
    if "nc" not in _NC_CACHE:
        _NC_CACHE["nc"] = build(4, 8)
    nc = _NC_CACHE["nc"]
    bpc = B // NCORES
    in_maps = [
        {
            "x": x[c * bpc : (c + 1) * bpc],
            "zu_w": zu,
            "dec1_w": d1,
            "dec2_w": d2,
            "out_w": ow,
        }
        for c in range(NCORES)
    ]
    res = run_bass_kernel_spmd(nc, in_maps, core_ids=list(range(NCORES)))
    return np.concatenate([res.results[c]["out"] for c in range(NCORES)], axis=0)


# revision 66
# speedup vs baseline: 2.7461x; 1.0046x over previous
"""Trainium2 Bass kernel for nn_CapsuleNeuralNetworkV2 (8 cores, data-parallel).

Reference math (per sample, 8 capsule iterations then decoder):
  v = h.reshape(4, 196); q,k,u = affine(v); scores = q k^T;
  P = softmax(scores); h' = P u;  dec = relu(h Wd1^T+bd1) Wd2^T+bd2;
  out = softmax(dec Wo^T + bo).

Restructuring (host-side algebra):
  Since each P has rows summing to 1, the state stays in the span of the 4
  initial slots: v^(k) = W3^k w^(k) + m_k with w^(k) = C^(k) V (C is a
  per-sample 4x4 convex-coefficient matrix, V the initial slots).
  scores^(k)[t,s] = C[t] M_k C[s]^T (mod per-t constants that cancel in
  softmax), where M_k[i,j] = v_i.(G_k v_j) + a_k.v_j depends only on the
  INITIAL slots: G_k = (W3^k)^T G W3^k, G = W1^T W2,
  a_k = (W3^k)^T (G^T m_k + W2^T b1).  G_k is numerically low-rank for k>=1
  (powers of a random matrix), so M_k is computed from rank-r_k SVD
  projections p_i = U_r^T v_i, q_j = (S V_r^T) v_j: M[i,j] ~ p_i.q_j + r_j.
  Per iteration only the tiny 4x4 chain is sequential:
  scores = C M C^T -> softmax -> C' = P C.  All projections/M_k are
  C-independent and pipeline on PE/Act/DVE ahead of the chain.
  Final w^(8) = C^(8) V; W3^8/m_8 are folded into Wd1/bd1 on the host.

Engines: PE transposes V once per tile + small bf16 projection matmuls +
decoder; DVE/Pool share the per-sample dot products and the 4x4 chain; Act
does PSUM evacuation, exp, and decoder activations.
"""

import numpy as np
import ml_dtypes

import concourse.bass as bass
import concourse.tile as tile
from concourse import bacc, mybir
from concourse.bass import ds
from concourse.bass_utils import run_bass_kernel_spmd
from concourse.masks import make_identity

FR = mybir.dt.float32r
BF = mybir.dt.bfloat16
F32 = mybir.dt.float32
AF = mybir.ActivationFunctionType
ALU = mybir.AluOpType

B = 32768
NCORES = 8
P = 128
T = 4
FV = 196
FEAT = 784
SLOT = 198  # h slot: 196 data + ones col (196) + spare (197)

RANKS = [96, 48, 32, 24, 16, 12, 8, 8]
NCOLS = [2 * (r + 1) for r in RANKS]  # proj cols per slot per k
POFF = [0]
for _n in NCOLS:
    POFF.append(POFF[-1] + _n)
PTOT = POFF[-1]
NCMAX = max(NCOLS)


def _ap(t, dims, offset_elems=0):
    """Hand-built AP over a tile's tensor: dims = [[step, count], ...]."""
    a = t[:] if hasattr(t, "tile") or not isinstance(t, bass.AP) else t
    return bass.AP(tensor=a.tensor, offset=a.offset + offset_elems, ap=dims)


def build(nsub=4, ngroups=8):
    """One NeuronCore program processing nsub*ngroups*128 samples."""
    bpc = nsub * ngroups * P
    nc = bacc.Bacc("TRN2", target_bir_lowering=False, debug=False)

    x_d = nc.dram_tensor("x", [bpc, FEAT], FR, kind="ExternalInput")
    pw_d = nc.dram_tensor("zu_w", [P, 2, PTOT], BF, kind="ExternalInput")
    d1_d = nc.dram_tensor("dec1_w", [P, 8, FEAT], FR, kind="ExternalInput")
    d2_d = nc.dram_tensor("dec2_w", [P, 7, FEAT], FR, kind="ExternalInput")
    ow_d = nc.dram_tensor("out_w", [P, 7, 10], FR, kind="ExternalInput")
    out_d = nc.dram_tensor("out", [bpc, 10], F32, kind="ExternalOutput")

    with tile.TileContext(nc) as tc:
        consts = tc.alloc_tile_pool(name="consts", bufs=1)
        hp = tc.alloc_tile_pool(name="h", bufs=2)
        vp = tc.alloc_tile_pool(name="vt", bufs=2)
        pkp = tc.alloc_tile_pool(name="pk", bufs=3)
        scp = tc.alloc_tile_pool(name="scr", bufs=4)
        mtp = tc.alloc_tile_pool(name="mt", bufs=8)
        sm = tc.alloc_tile_pool(name="small", bufs=3)
        wp = tc.alloc_tile_pool(name="w", bufs=2)
        wkd = tc.alloc_tile_pool(name="wkd", bufs=1)
        pp = tc.alloc_tile_pool(name="ps", bufs=2, space="PSUM")

        ident_f = consts.tile([P, P], F32)
        make_identity(nc, ident_f)
        ident_r = consts.tile([P, P], FR)
        nc.vector.tensor_copy(ident_r, ident_f)
        ones_c = consts.tile([P, 512], F32)
        nc.vector.memset(ones_c, 1.0)
        pw = consts.tile([P, 2, PTOT], BF)
        nc.sync.dma_start(out=pw, in_=pw_d[:, :, :])
        # decoder weights DMA'd after group 0's x tiles (emitted in build
        # below) so the first group's compute isn't starved behind 6MB
        d1_w = consts.tile([P, 8, FEAT], FR)
        d2_w = consts.tile([P, 7, FEAT], FR)
        ow_w = consts.tile([P, 7, 10], FR)

        def load_tile(g, j):
            h0 = hp.tile([P, T, SLOT], FR, tag=f"h{j}")
            nc.sync.dma_start(
                out=h0[:, :, 0:FV],
                in_=x_d[ds(g * (nsub * P) + j * P, P), :].rearrange(
                    "p (t f) -> p t f", t=T
                ),
            )
            nc.gpsimd.tensor_copy(h0[:, :, 196:198], ones_c[:, 0 : 2 * T])
            vt1 = vp.tile([P, T, P], BF, tag=f"vt1{j}")
            vt2 = vp.tile([69, T, P], BF, tag=f"vt2{j}")
            t1_ps = pp.tile([P, T, P], FR, tag="t1ps", bufs=1)
            t2_ps = pp.tile([69, T, P], FR, tag="t2ps", bufs=1)
            for t in range(T):
                nc.tensor.transpose(t1_ps[:, t, :], h0[:, t, 0:P], ident_r)
                nc.tensor.transpose(t2_ps[:, t, :], h0[:, t, P : P + 69], ident_r)
            nc.scalar.copy(vt1, t1_ps)
            nc.scalar.copy(vt2, t2_ps)
            return h0, None, vt1, vt2

        def proj(j, k, vt1, vt2):
            """PE projections for iteration k -> pk [128, 4, nc] bf16."""
            nco = NCOLS[k]
            off = POFF[k]
            pk = pkp.tile([P, T, NCMAX], BF, tag=f"pk{j}")
            if k == 0:
                for half in range(2):
                    ps = pp.tile([P, 2, NCMAX], F32, tag="pkps", bufs=2)
                    for sl in range(2):
                        s = half * 2 + sl
                        nc.tensor.matmul(
                            ps[:, sl, 0:nco], vt1[:, s, :],
                            pw[:, 0, off : off + nco], start=True, stop=False)
                        nc.tensor.matmul(
                            ps[:, sl, 0:nco], vt2[0:69, s, :],
                            pw[0:69, 1, off : off + nco], start=False, stop=True)
                    nc.scalar.copy(
                        pk[:, 2 * half : 2 * half + 2, 0:nco], ps[:, :, 0:nco])
            else:
                ps = pp.tile([P, T, 98], F32, tag="pkps1", bufs=2)
                for s in range(T):
                    nc.tensor.matmul(
                        ps[:, s, 0:nco], vt1[:, s, :],
                        pw[:, 0, off : off + nco], start=True, stop=False)
                    nc.tensor.matmul(
                        ps[:, s, 0:nco], vt2[0:69, s, :],
                        pw[0:69, 1, off : off + nco], start=False, stop=True)
                nc.scalar.copy(pk[:, :, 0:nco], ps[:, :, 0:nco])
            return pk

        def dots(j, k, mtc, pk):
            """M_k[i,j] for all 16 slot pairs -> wave-mtc rows 4(j%2)+i."""
            r1 = RANKS[k] + 1
            pap = pk[:].ap[0]
            jw = j % 2
            if k == 0:
                # big r: fused stt (mult + f32 accumulate in one 1x pass)
                scr = scp.tile([P, 256], BF, tag="scr197", bufs=6)
                for idx in range(16):
                    i, jj = idx // 4, idx % 4
                    in0 = _ap(pk, [pap, [1, r1]],
                              offset_elems=i * NCMAX + r1)
                    in1 = _ap(pk, [pap, [1, r1]], offset_elems=jj * NCMAX)
                    nc.vector.scalar_tensor_tensor(
                        out=scr[:, 0:r1], in0=in0, scalar=1.0, in1=in1,
                        op0=ALU.mult, op1=ALU.mult,
                        accum_out=mtc[:, 4 * jw + i, jj : jj + 1])
            else:
                # small r: one bf16 2x tensor_tensor + one inner-axis reduce
                scr = scp.tile([P, T, T, 65], BF, tag="scr")
                in0 = _ap(pk, [pap, [NCMAX, 4], [0, 4], [1, r1]],
                          offset_elems=r1)
                in1 = _ap(pk, [pap, [0, 4], [NCMAX, 4], [1, r1]])
                nc.vector.tensor_tensor(
                    out=scr[:, :, :, 0:r1], in0=in0, in1=in1, op=ALU.mult)
                nc.vector.tensor_reduce(
                    out=mtc[:, 4 * jw : 4 * jw + 4, :], in_=scr[:, :, :, 0:r1],
                    axis=mybir.AxisListType.X, op=ALU.add)

        def serial_phase(k, w, mtc, c_prev):
            """Per-k 4x4 chain for one WAVE (2 tiles) in wide DVE ops over a
            [128, (j,t), s] layout (j in the wave): scores = C mt C^T ->
            e = exp -> C'u = e C -> C' = C'u / rowsum. Returns new C tile."""
            JT = 8   # (2 tiles) x (4 slots)
            JR = 32  # replicated size per tile pair
            if k == 0:
                s_t = mtc
            else:
                cap = c_prev[:].ap[0]
                # replicate C 4x -> crep[j, rep, s, jj] so every TT operand
                # stays within the ISA's 3-free-dim AP limit
                crep = sm.tile([P, 4 * JR], F32, tag=f"crep{w}", bufs=2)
                nc.gpsimd.tensor_copy(
                    _ap(crep, [crep[:].ap[0], [64, 2], [16, 4], [1, 16]]),
                    _ap(c_prev, [cap, [16, 2], [0, 4], [1, 16]]))
                tt_eng = nc.vector if k >= 4 else nc.gpsimd
                scrd = scp.tile([P, JT, T, T], F32, tag="scrd", bufs=6)
                tt_eng.tensor_tensor(  # D[j,i,s] = sum_jj mt[j,i,jj] C[j,s,jj]
                    out=scrd,
                    in0=_ap(mtc, [mtc[:].ap[0], [4, JT], [0, 4], [1, 4]]),
                    in1=crep[:],
                    op=ALU.mult)
                dm = sm.tile([P, JT, T], F32, tag=f"dm{w}")
                nc.vector.tensor_reduce(
                    out=dm, in_=scrd, axis=mybir.AxisListType.X, op=ALU.add)
                drep = sm.tile([P, 4 * JR], F32, tag=f"drep{w}", bufs=2)
                nc.gpsimd.tensor_copy(
                    _ap(drep, [drep[:].ap[0], [64, 2], [16, 4], [1, 16]]),
                    _ap(dm, [dm[:].ap[0], [16, 2], [0, 4], [1, 16]]))
                scrd2 = scp.tile([P, JT, T, T], F32, tag="scrd", bufs=6)
                tt_eng.tensor_tensor(  # S[j,t,s] = sum_i C[j,t,i] D[j,i,s]
                    out=scrd2,
                    in0=_ap(c_prev, [cap, [4, JT], [0, 4], [1, 4]]),
                    in1=_ap(drep, [drep[:].ap[0], [16, JT], [1, 4], [4, 4]]),
                    op=ALU.mult)
                s_t = sm.tile([P, JT, T], F32, tag=f"st{w}")
                nc.vector.tensor_reduce(
                    out=s_t, in_=scrd2, axis=mybir.AxisListType.X, op=ALU.add)
            e = sm.tile([P, JT, T], F32, tag=f"e{w}")
            nc.scalar.activation(e, s_t, AF.Exp)
            sums = sm.tile([P, JT], F32, tag=f"su{w}")
            nc.vector.reduce_sum(sums, e, axis=mybir.AxisListType.X)
            rec = sm.tile([P, JT], F32, tag=f"re{w}")
            nc.vector.reciprocal(rec, sums)
            if k == 0:
                cnum = e
            else:
                scrd3 = scp.tile([P, JT, T, T], F32, tag="scrd", bufs=6)
                tt_eng.tensor_tensor(  # C'u[j,t,jj] = sum_s e[j,t,s] C[j,s,jj]
                    out=scrd3,
                    in0=_ap(e, [e[:].ap[0], [4, JT], [0, 4], [1, 4]]),
                    in1=_ap(crep, [crep[:].ap[0], [16, JT], [1, 4], [4, 4]]),
                    op=ALU.mult)
                cnum = sm.tile([P, JT, T], F32, tag=f"cu{w}")
                nc.vector.tensor_reduce(
                    out=cnum, in_=scrd3, axis=mybir.AxisListType.X, op=ALU.add)
            c_new = sm.tile([P, JT, T], F32, tag=f"call{w}", bufs=3)
            nc.vector.scalar_tensor_tensor(
                out=c_new, in0=cnum, scalar=1.0,
                in1=_ap(rec, [rec[:].ap[0], [1, JT], [0, T]]),
                op0=ALU.mult, op1=ALU.mult)
            return c_new

        def recon(j, h0, c8):
            """w[:, t, :] = sum_s C8[t,s] * h0[:, s, :] (ones col rides along)."""
            w = wp.tile([P, T, SLOT], FR, tag=f"w{j}", bufs=1)
            jw = j % 2
            for t in range(T):
                nc.scalar.activation(
                    w[:, t, :], h0[:, 0, :], AF.Copy,
                    scale=c8[:, 4 * jw + t, 0:1])
            for t in range(3):
                for s in range(1, T):
                    nc.vector.scalar_tensor_tensor(
                        out=w[:, t, :], in0=h0[:, s, :],
                        scalar=c8[:, 4 * jw + t, s : s + 1], in1=w[:, t, :],
                        op0=ALU.mult, op1=ALU.add)
            for t in range(3, T):
                pct = wp.tile([P, SLOT], F32, tag="pct", bufs=2)
                for s in range(1, T):
                    nc.gpsimd.tensor_scalar_mul(
                        pct, h0[:, s, :], c8[:, 4 * jw + t, s : s + 1])
                    nc.gpsimd.tensor_add(w[:, t, :], w[:, t, :], pct)
            return w

        def decoder(ws2, g, wave):
            """Decoder over one wave of 2 tiles (N = 256 wide matmuls)."""
            W = 2 * P
            ht1 = wkd.tile([P, T, W], FR, tag=f"ht1w{wave}")
            ht2 = wkd.tile([69, T, W], FR, tag=f"ht2w{wave}")
            for t in range(T):
                t1_ps = pp.tile([P, T, P], FR, tag="t1ps", bufs=1)
                t2_ps = pp.tile([69, T, P], FR, tag="t2ps", bufs=1)
                for j in range(2):
                    nc.tensor.transpose(
                        t1_ps[:, j, :], ws2[j][:, t, 0:P], ident_r
                    )
                    nc.tensor.transpose(
                        t2_ps[:, j, :], ws2[j][:, t, P : P + 69], ident_r
                    )
                nc.scalar.copy(ht1[:, t, :], t1_ps[:, 0:2, :])
                nc.vector.tensor_copy(ht2[:, t, :], t2_ps[:, 0:2, :])

            # dec1 = relu(Wd1~ @ w.T + bd1~), feature-major, 7 M-chunks
            d1a = wkd.tile([P, 6, W], FR, tag=f"d1aw{wave}")
            d1b = wkd.tile([17, W], FR, tag=f"d1bw{wave}")
            nc.vector.tensor_copy(d1b, ones_c[0:17, 0:W])
            for m in range(7):
                mw = min(P, FEAT - m * P)
                mp = pp.tile([P, W], F32, tag="mp")
                msl = slice(m * P, m * P + mw)
                for t in range(T):
                    nc.tensor.matmul(mp[0:mw, :], d1_w[:, t, msl], ht1[:, t, :],
                                     start=(t == 0), stop=False)
                for t in range(T):
                    nc.tensor.matmul(mp[0:mw, :], d1_w[0:69, 4 + t, msl],
                                     ht2[:, t, :], start=False, stop=(t == 3))
                if m < 6:
                    nc.scalar.activation(d1a[:, m, :], mp, AF.Relu)
                else:
                    nc.scalar.activation(d1b[0:16, :], mp[0:16, :], AF.Relu)

            # dec2 = Wd2 @ relu1 + bd2, feature-major
            d2a = wkd.tile([P, 6, W], FR, tag=f"d2aw{wave}")
            d2b = wkd.tile([17, W], FR, tag=f"d2bw{wave}")
            nc.vector.tensor_copy(d2b, ones_c[0:17, 0:W])
            for m in range(7):
                mw = min(P, FEAT - m * P)
                mp = pp.tile([P, W], F32, tag="mp")
                msl = slice(m * P, m * P + mw)
                for c in range(6):
                    nc.tensor.matmul(mp[0:mw, :], d2_w[:, c, msl], d1a[:, c, :],
                                     start=(c == 0), stop=False)
                nc.tensor.matmul(mp[0:mw, :], d2_w[0:17, 6, msl], d1b,
                                 start=False, stop=True)
                if m < 6:
                    nc.scalar.copy(d2a[:, m, :], mp)
                else:
                    nc.scalar.copy(d2b[0:16, :], mp[0:16, :])

            # logits + softmax per subtile
            for j in range(2):
                jsl = slice(j * P, (j + 1) * P)
                lgt = pp.tile([P, W], F32, tag="mp")
                lg = lgt[:, 0:10]
                for c in range(6):
                    nc.tensor.matmul(lg, d2a[:, c, jsl], ow_w[:, c, :],
                                     start=(c == 0), stop=False)
                nc.tensor.matmul(lg, d2b[:, jsl], ow_w[0:17, 6, :],
                                 start=False, stop=True)
                e10 = sm.tile([P, 10], F32, tag="e10")
                s10 = sm.tile([P, 1], F32, tag="s10")
                nc.scalar.activation(e10, lg, AF.Exp, accum_out=s10)
                r10 = sm.tile([P, 1], F32, tag="r10")
                nc.vector.reciprocal(r10, s10)
                o10 = sm.tile([P, 10], F32, tag="o10")
                nc.vector.tensor_scalar_mul(o10, e10, r10)
                nc.sync.dma_start(
                    out=out_d[ds(g * (nsub * P) + (2 * wave + j) * P, P), :],
                    in_=o10
                )

        def body(g, preloaded=None):
            h0s, hbs, cs = [], [], [None, None]
            vts = []
            # k=0 proj+dots interleaved per tile so the first tile's chain
            # races ahead of later tiles' loads
            mtc0 = [mtp.tile([P, 8, T], F32, tag=f"mtk{w}", bufs=3,
                             name=f"mt0w{w}") for w in range(2)]
            for j in range(nsub):
                h0, hb, vt1, vt2 = (
                    load_tile(g, j) if preloaded is None else preloaded[j]
                )
                h0s.append(h0)
                hbs.append(hb)
                vts.append((vt1, vt2))
                pk = proj(j, 0, vt1, vt2)
                dots(j, 0, mtc0[j // 2], pk)
            mtk = [mtc0]  # per-wave Mt tiles per k
            for k in range(1, 8):
                mtcs = [mtp.tile([P, 8, T], F32, tag=f"mtk{w}", bufs=3,
                                 name=f"mt{k}w{w}") for w in range(2)]
                for j in range(nsub):
                    pk = proj(j, k, *vts[j])
                    dots(j, k, mtcs[j // 2], pk)
                mtk.append(mtcs)
                # serial chains run one k behind so the independent dots of
                # the next k fill engine gaps while the chains ping-pong
                for w in range(2):
                    cs[w] = serial_phase(k - 1, w, mtk[k - 1][w], cs[w])
            # wave 0 finishes first and its decoder overlaps wave 1's tail
            cs[0] = serial_phase(7, 0, mtk[7][0], cs[0])
            ws0 = [recon(0, h0s[0], cs[0]), recon(1, h0s[1], cs[0])]
            cs[1] = serial_phase(7, 1, mtk[7][1], cs[1])
            decoder(ws0, g, 0)
            ws1 = [recon(2, h0s[2], cs[1]), recon(3, h0s[3], cs[1])]
            decoder(ws1, g, 1)

        # group 0 loads first so its x DMAs precede the 6MB of decoder
        # weights; weight DMAs ride the Act queue so their triggers land
        # after the x transfers are in flight
        pre0 = [load_tile(0, j) for j in range(nsub)]
        nc.sync.dma_start(out=d1_w, in_=d1_d[:, :, :])
        nc.sync.dma_start(out=d2_w, in_=d2_d[:, :, :])
        nc.sync.dma_start(out=ow_w, in_=ow_d[:, :, :])
        body(0, preloaded=pre0)
        if ngroups > 1:
            with tc.For_i(1, ngroups, 1) as g:
                body(g)
        for _pool in (pp, wkd, wp, sm, mtp, scp, pkp, vp, hp, consts):
            _pool.release()

    nc.compile()
    return nc


def pack_weights(W1, b1, W2, b2, W3, b3, Wd1, bd1, Wd2, bd2, Wo, bo):
    f64 = np.float64
    W1, b1, W2, b2, W3, b3 = (np.asarray(t, f64) for t in (W1, b1, W2, b2, W3, b3))
    G = W1.T @ W2
    a = W2.T @ b1

    A = np.eye(FV)
    m = np.zeros(FV)
    pw = np.zeros((P, 2, PTOT), np.float32)
    for k in range(8):
        Gk = A.T @ G @ A
        ak = A.T @ (G.T @ m + a)
        nco = NCOLS[k]
        Wk = np.zeros((197, nco), f64)
        r = RANKS[k]
        r1 = r + 1
        U, S, Vh = np.linalg.svd(Gk)
        Wk[:FV, :r] = (np.diag(S[:r]) @ Vh[:r]).T
        Wk[:FV, r] = ak
        Wk[:FV, r1 : r1 + r] = U[:, :r]
        Wk[FV, r1 + r] = 1.0
        off = POFF[k]
        pw[:, 0, off : off + nco] = Wk[0:128]
        pw[0:69, 1, off : off + nco] = Wk[128:197]
        A = W3 @ A
        m = W3 @ m + b3
    A8, m8 = A, m

    # fold W3^8 / m8 into the first decoder layer
    BD = np.zeros((FEAT, FEAT), f64)
    mm = np.zeros(FEAT, f64)
    for t in range(T):
        BD[t * FV : (t + 1) * FV, t * FV : (t + 1) * FV] = A8
        mm[t * FV : (t + 1) * FV] = m8
    Wd1f = np.asarray(Wd1, f64) @ BD
    bd1f = np.asarray(bd1, f64) + np.asarray(Wd1, f64) @ mm

    d1 = np.zeros((P, 8, FEAT), np.float32)
    W1T = Wd1f.T  # [784 f_in, 784 j]
    for t in range(T):
        d1[:, t, :] = W1T[t * FV : t * FV + P, :]
        d1[0:68, 4 + t, :] = W1T[t * FV + P : (t + 1) * FV, :]
    d1[68, 4, :] = bd1f

    d2 = np.zeros((P, 7, FEAT), np.float32)
    W2T = np.asarray(Wd2, f64).T
    for cidx in range(6):
        d2[:, cidx, :] = W2T[cidx * P : (cidx + 1) * P, :]
    d2[0:16, 6, :] = W2T[768:784, :]
    d2[16, 6, :] = np.asarray(bd2, f64)

    ow = np.zeros((P, 7, 10), np.float32)
    WoT = np.asarray(Wo, f64).T
    for cidx in range(6):
        ow[:, cidx, :] = WoT[cidx * P : (cidx + 1) * P, :]
    ow[0:16, 6, :] = WoT[768:784, :]
    ow[16, 6, :] = np.asarray(bo, f64)
    return pw.astype(ml_dtypes.bfloat16), d1, d2, ow


_NC_CACHE = {}


def kernel(**inputs):
    x = np.ascontiguousarray(np.asarray(inputs["x"], np.float32))
    zu, d1, d2, ow = pack_weights(
        inputs["W1"], inputs["b1"], inputs["W2"], inputs["b2"], inputs["W3"],
        inputs["b3"], inputs["Wd1"], inputs["bd1"], inputs["Wd2"],
        inputs["bd2"], inputs["Wo"], inputs["bo"],
    )
    if "nc" not in _NC_CACHE:
        _NC_CACHE["nc"] = build(4, 8)
    nc = _NC_CACHE["nc"]
    bpc = B // NCORES
    in_maps = [
        {
            "x": x[c * bpc : (c + 1) * bpc],
            "zu_w": zu,
            "dec1_w": d1,
            "dec2_w": d2,
            "out_w": ow,
        }
        for c in range(NCORES)
    ]
    res = run_bass_kernel_spmd(nc, in_maps, core_ids=list(range(NCORES)))
    return np.concatenate([res.results[c]["out"] for c in range(NCORES)], axis=0)


# revision 69
# speedup vs baseline: 2.8057x; 1.0217x over previous
"""Trainium2 Bass kernel for nn_CapsuleNeuralNetworkV2 (8 cores, data-parallel).

Reference math (per sample, 8 capsule iterations then decoder):
  v = h.reshape(4, 196); q,k,u = affine(v); scores = q k^T;
  P = softmax(scores); h' = P u;  dec = relu(h Wd1^T+bd1) Wd2^T+bd2;
  out = softmax(dec Wo^T + bo).

Restructuring (host-side algebra):
  Since each P has rows summing to 1, the state stays in the span of the 4
  initial slots: v^(k) = W3^k w^(k) + m_k with w^(k) = C^(k) V (C is a
  per-sample 4x4 convex-coefficient matrix, V the initial slots).
  scores^(k)[t,s] = C[t] M_k C[s]^T (mod per-t constants that cancel in
  softmax), where M_k[i,j] = v_i.(G_k v_j) + a_k.v_j depends only on the
  INITIAL slots: G_k = (W3^k)^T G W3^k, G = W1^T W2,
  a_k = (W3^k)^T (G^T m_k + W2^T b1).  G_k is numerically low-rank for k>=1
  (powers of a random matrix), so M_k is computed from rank-r_k SVD
  projections p_i = U_r^T v_i, q_j = (S V_r^T) v_j: M[i,j] ~ p_i.q_j + r_j.
  Per iteration only the tiny 4x4 chain is sequential:
  scores = C M C^T -> softmax -> C' = P C.  All projections/M_k are
  C-independent and pipeline on PE/Act/DVE ahead of the chain.
  Final w^(8) = C^(8) V; W3^8/m_8 are folded into Wd1/bd1 on the host.

Engines: PE transposes V once per tile + small bf16 projection matmuls +
decoder; DVE/Pool share the per-sample dot products and the 4x4 chain; Act
does PSUM evacuation, exp, and decoder activations.
"""

import numpy as np
import ml_dtypes

import concourse.bass as bass
import concourse.tile as tile
from concourse import bacc, mybir
from concourse.bass import ds
from concourse.bass_utils import run_bass_kernel_spmd
from concourse.masks import make_identity

FR = mybir.dt.float32r
BF = mybir.dt.bfloat16
F32 = mybir.dt.float32
AF = mybir.ActivationFunctionType
ALU = mybir.AluOpType

B = 32768
NCORES = 8
P = 128
T = 4
FV = 196
FEAT = 784
SLOT = 198  # h slot: 196 data + ones col (196) + spare (197)

RANKS = [96, 48, 32, 24, 16, 12, 8, 8]
NCOLS = [2 * (r + 1) for r in RANKS]  # proj cols per slot per k
POFF = [0]
for _n in NCOLS:
    POFF.append(POFF[-1] + _n)
PTOT = POFF[-1]
NCMAX = max(NCOLS)


def _ap(t, dims, offset_elems=0):
    """Hand-built AP over a tile's tensor: dims = [[step, count], ...]."""
    a = t[:] if hasattr(t, "tile") or not isinstance(t, bass.AP) else t
    return bass.AP(tensor=a.tensor, offset=a.offset + offset_elems, ap=dims)


def build(nsub=4, ngroups=8):
    """One NeuronCore program processing nsub*ngroups*128 samples."""
    bpc = nsub * ngroups * P
    nc = bacc.Bacc("TRN2", target_bir_lowering=False, debug=False)

    x_d = nc.dram_tensor("x", [bpc, FEAT], FR, kind="ExternalInput")
    pw_d = nc.dram_tensor("zu_w", [P, 2, PTOT], BF, kind="ExternalInput")
    d1_d = nc.dram_tensor("dec1_w", [P, 8, FEAT], FR, kind="ExternalInput")
    d2_d = nc.dram_tensor("dec2_w", [P, 7, FEAT], FR, kind="ExternalInput")
    ow_d = nc.dram_tensor("out_w", [P, 7, 10], FR, kind="ExternalInput")
    out_d = nc.dram_tensor("out", [bpc, 10], F32, kind="ExternalOutput")

    with tile.TileContext(nc) as tc:
        consts = tc.alloc_tile_pool(name="consts", bufs=1)
        hp = tc.alloc_tile_pool(name="h", bufs=2)
        vp = tc.alloc_tile_pool(name="vt", bufs=2)
        pkp = tc.alloc_tile_pool(name="pk", bufs=3)
        scp = tc.alloc_tile_pool(name="scr", bufs=4)
        mtp = tc.alloc_tile_pool(name="mt", bufs=8)
        sm = tc.alloc_tile_pool(name="small", bufs=3)
        wp = tc.alloc_tile_pool(name="w", bufs=2)
        wkd = tc.alloc_tile_pool(name="wkd", bufs=1)
        pp = tc.alloc_tile_pool(name="ps", bufs=2, space="PSUM")

        ident_f = consts.tile([P, P], F32)
        make_identity(nc, ident_f)
        ident_r = consts.tile([P, P], FR)
        nc.vector.tensor_copy(ident_r, ident_f)
        ones_c = consts.tile([P, 512], F32)
        nc.vector.memset(ones_c, 1.0)
        pw = consts.tile([P, 2, PTOT], BF)
        nc.sync.dma_start(out=pw, in_=pw_d[:, :, :])
        # decoder weights DMA'd after group 0's x tiles (emitted in build
        # below) so the first group's compute isn't starved behind 6MB
        d1_w = consts.tile([P, 8, FEAT], FR)
        d2_w = consts.tile([P, 7, FEAT], FR)
        ow_w = consts.tile([P, 7, 10], FR)

        def load_dma(g, j):
            h0 = hp.tile([P, T, SLOT], FR, tag=f"h{j}")
            nc.sync.dma_start(
                out=h0[:, :, 0:FV],
                in_=x_d[ds(g * (nsub * P) + j * P, P), :].rearrange(
                    "p (t f) -> p t f", t=T
                ),
            )
            nc.gpsimd.tensor_copy(h0[:, :, 196:198], ones_c[:, 0 : 2 * T])
            return h0

        def prep_tile(j, h0):
            vt1 = vp.tile([P, T, P], BF, tag=f"vt1{j}")
            vt2 = vp.tile([69, T, P], BF, tag=f"vt2{j}")
            t1_ps = pp.tile([P, T, P], FR, tag="t1ps", bufs=1)
            t2_ps = pp.tile([69, T, P], FR, tag="t2ps", bufs=1)
            for t in range(T):
                nc.tensor.transpose(t1_ps[:, t, :], h0[:, t, 0:P], ident_r)
                nc.tensor.transpose(t2_ps[:, t, :], h0[:, t, P : P + 69], ident_r)
            nc.scalar.copy(vt1, t1_ps)
            nc.scalar.copy(vt2, t2_ps)
            return vt1, vt2

        def load_tile(g, j):
            h0 = load_dma(g, j)
            return h0, None, None, None

        def proj(j, k, vt1, vt2):
            """PE projections for iteration k -> pk [128, 4, nc] bf16."""
            nco = NCOLS[k]
            off = POFF[k]
            pk = pkp.tile([P, T, NCMAX], BF, tag=f"pk{j}")
            if k == 0:
                for half in range(2):
                    ps = pp.tile([P, 2, NCMAX], F32, tag="pkps", bufs=2)
                    for sl in range(2):
                        s = half * 2 + sl
                        nc.tensor.matmul(
                            ps[:, sl, 0:nco], vt1[:, s, :],
                            pw[:, 0, off : off + nco], start=True, stop=False)
                        nc.tensor.matmul(
                            ps[:, sl, 0:nco], vt2[0:69, s, :],
                            pw[0:69, 1, off : off + nco], start=False, stop=True)
                    nc.scalar.copy(
                        pk[:, 2 * half : 2 * half + 2, 0:nco], ps[:, :, 0:nco])
            else:
                ps = pp.tile([P, T, 98], F32, tag="pkps1", bufs=2)
                for s in range(T):
                    nc.tensor.matmul(
                        ps[:, s, 0:nco], vt1[:, s, :],
                        pw[:, 0, off : off + nco], start=True, stop=False)
                    nc.tensor.matmul(
                        ps[:, s, 0:nco], vt2[0:69, s, :],
                        pw[0:69, 1, off : off + nco], start=False, stop=True)
                nc.scalar.copy(pk[:, :, 0:nco], ps[:, :, 0:nco])
            return pk

        def dots(j, k, mtc, pk):
            """M_k[i,j] for all 16 slot pairs -> wave-mtc rows 4(j%2)+i."""
            r1 = RANKS[k] + 1
            pap = pk[:].ap[0]
            jw = j % 2
            if k == 0:
                # big r: fused stt (mult + f32 accumulate in one 1x pass)
                scr = scp.tile([P, 256], BF, tag="scr197", bufs=6)
                for idx in range(16):
                    i, jj = idx // 4, idx % 4
                    in0 = _ap(pk, [pap, [1, r1]],
                              offset_elems=i * NCMAX + r1)
                    in1 = _ap(pk, [pap, [1, r1]], offset_elems=jj * NCMAX)
                    nc.vector.scalar_tensor_tensor(
                        out=scr[:, 0:r1], in0=in0, scalar=1.0, in1=in1,
                        op0=ALU.mult, op1=ALU.mult,
                        accum_out=mtc[:, 4 * jw + i, jj : jj + 1])
            else:
                # small r: one bf16 2x tensor_tensor + one inner-axis reduce
                scr = scp.tile([P, T, T, 65], BF, tag="scr")
                in0 = _ap(pk, [pap, [NCMAX, 4], [0, 4], [1, r1]],
                          offset_elems=r1)
                in1 = _ap(pk, [pap, [0, 4], [NCMAX, 4], [1, r1]])
                nc.vector.tensor_tensor(
                    out=scr[:, :, :, 0:r1], in0=in0, in1=in1, op=ALU.mult)
                nc.vector.tensor_reduce(
                    out=mtc[:, 4 * jw : 4 * jw + 4, :], in_=scr[:, :, :, 0:r1],
                    axis=mybir.AxisListType.X, op=ALU.add)

        def serial_phase(k, w, mtc, c_prev):
            """Per-k 4x4 chain for one WAVE (2 tiles) in wide DVE ops over a
            [128, (j,t), s] layout (j in the wave): scores = C mt C^T ->
            e = exp -> C'u = e C -> C' = C'u / rowsum. Returns new C tile."""
            JT = 8   # (2 tiles) x (4 slots)
            JR = 32  # replicated size per tile pair
            if k == 0:
                s_t = mtc
            else:
                cap = c_prev[:].ap[0]
                # replicate C 4x -> crep[j, rep, s, jj] so every TT operand
                # stays within the ISA's 3-free-dim AP limit
                crep = sm.tile([P, 4 * JR], F32, tag=f"crep{w}", bufs=2)
                nc.gpsimd.tensor_copy(
                    _ap(crep, [crep[:].ap[0], [64, 2], [16, 4], [1, 16]]),
                    _ap(c_prev, [cap, [16, 2], [0, 4], [1, 16]]))
                tt_eng = nc.vector if k >= 4 else nc.gpsimd
                scrd = scp.tile([P, JT, T, T], F32, tag="scrd", bufs=6)
                tt_eng.tensor_tensor(  # D[j,i,s] = sum_jj mt[j,i,jj] C[j,s,jj]
                    out=scrd,
                    in0=_ap(mtc, [mtc[:].ap[0], [4, JT], [0, 4], [1, 4]]),
                    in1=crep[:],
                    op=ALU.mult)
                dm = sm.tile([P, JT, T], F32, tag=f"dm{w}")
                nc.vector.tensor_reduce(
                    out=dm, in_=scrd, axis=mybir.AxisListType.X, op=ALU.add)
                drep = sm.tile([P, 4 * JR], F32, tag=f"drep{w}", bufs=2)
                nc.gpsimd.tensor_copy(
                    _ap(drep, [drep[:].ap[0], [64, 2], [16, 4], [1, 16]]),
                    _ap(dm, [dm[:].ap[0], [16, 2], [0, 4], [1, 16]]))
                scrd2 = scp.tile([P, JT, T, T], F32, tag="scrd", bufs=6)
                tt_eng.tensor_tensor(  # S[j,t,s] = sum_i C[j,t,i] D[j,i,s]
                    out=scrd2,
                    in0=_ap(c_prev, [cap, [4, JT], [0, 4], [1, 4]]),
                    in1=_ap(drep, [drep[:].ap[0], [16, JT], [1, 4], [4, 4]]),
                    op=ALU.mult)
                s_t = sm.tile([P, JT, T], F32, tag=f"st{w}")
                nc.vector.tensor_reduce(
                    out=s_t, in_=scrd2, axis=mybir.AxisListType.X, op=ALU.add)
            e = sm.tile([P, JT, T], F32, tag=f"e{w}")
            nc.scalar.activation(e, s_t, AF.Exp)
            sums = sm.tile([P, JT], F32, tag=f"su{w}")
            nc.vector.reduce_sum(sums, e, axis=mybir.AxisListType.X)
            rec = sm.tile([P, JT], F32, tag=f"re{w}")
            nc.vector.reciprocal(rec, sums)
            if k == 0:
                cnum = e
            else:
                scrd3 = scp.tile([P, JT, T, T], F32, tag="scrd", bufs=6)
                tt_eng.tensor_tensor(  # C'u[j,t,jj] = sum_s e[j,t,s] C[j,s,jj]
                    out=scrd3,
                    in0=_ap(e, [e[:].ap[0], [4, JT], [0, 4], [1, 4]]),
                    in1=_ap(crep, [crep[:].ap[0], [16, JT], [1, 4], [4, 4]]),
                    op=ALU.mult)
                cnum = sm.tile([P, JT, T], F32, tag=f"cu{w}")
                nc.vector.tensor_reduce(
                    out=cnum, in_=scrd3, axis=mybir.AxisListType.X, op=ALU.add)
            c_new = sm.tile([P, JT, T], F32, tag=f"call{w}", bufs=3)
            nc.vector.scalar_tensor_tensor(
                out=c_new, in0=cnum, scalar=1.0,
                in1=_ap(rec, [rec[:].ap[0], [1, JT], [0, T]]),
                op0=ALU.mult, op1=ALU.mult)
            return c_new

        def recon(j, h0, c8):
            """w[:, t, :] = sum_s C8[t,s] * h0[:, s, :] (ones col rides along)."""
            w = wp.tile([P, T, SLOT], FR, tag=f"w{j}", bufs=1)
            jw = j % 2
            for t in range(T):
                nc.scalar.activation(
                    w[:, t, :], h0[:, 0, :], AF.Copy,
                    scale=c8[:, 4 * jw + t, 0:1])
            for t in range(3):
                for s in range(1, T):
                    nc.vector.scalar_tensor_tensor(
                        out=w[:, t, :], in0=h0[:, s, :],
                        scalar=c8[:, 4 * jw + t, s : s + 1], in1=w[:, t, :],
                        op0=ALU.mult, op1=ALU.add)
            for t in range(3, T):
                pct = wp.tile([P, SLOT], F32, tag="pct", bufs=2)
                for s in range(1, T):
                    nc.gpsimd.tensor_scalar_mul(
                        pct, h0[:, s, :], c8[:, 4 * jw + t, s : s + 1])
                    nc.gpsimd.tensor_add(w[:, t, :], w[:, t, :], pct)
            return w

        def decoder(ws2, g, wave):
            """Decoder over one wave of 2 tiles (N = 256 wide matmuls)."""
            W = 2 * P
            ht1 = wkd.tile([P, T, W], FR, tag=f"ht1w{wave}")
            ht2 = wkd.tile([69, T, W], FR, tag=f"ht2w{wave}")
            for t in range(T):
                t1_ps = pp.tile([P, T, P], FR, tag="t1ps", bufs=1)
                t2_ps = pp.tile([69, T, P], FR, tag="t2ps", bufs=1)
                for j in range(2):
                    nc.tensor.transpose(
                        t1_ps[:, j, :], ws2[j][:, t, 0:P], ident_r
                    )
                    nc.tensor.transpose(
                        t2_ps[:, j, :], ws2[j][:, t, P : P + 69], ident_r
                    )
                nc.scalar.copy(ht1[:, t, :], t1_ps[:, 0:2, :])
                nc.vector.tensor_copy(ht2[:, t, :], t2_ps[:, 0:2, :])

            # dec1 = relu(Wd1~ @ w.T + bd1~), feature-major, 7 M-chunks
            d1a = wkd.tile([P, 6, W], FR, tag=f"d1aw{wave}")
            d1b = wkd.tile([17, W], FR, tag=f"d1bw{wave}")
            nc.vector.tensor_copy(d1b, ones_c[0:17, 0:W])
            for m in range(7):
                mw = min(P, FEAT - m * P)
                mp = pp.tile([P, W], F32, tag="mp")
                msl = slice(m * P, m * P + mw)
                for t in range(T):
                    nc.tensor.matmul(mp[0:mw, :], d1_w[:, t, msl], ht1[:, t, :],
                                     start=(t == 0), stop=False)
                for t in range(T):
                    nc.tensor.matmul(mp[0:mw, :], d1_w[0:69, 4 + t, msl],
                                     ht2[:, t, :], start=False, stop=(t == 3))
                if m < 6:
                    nc.scalar.activation(d1a[:, m, :], mp, AF.Relu)
                else:
                    nc.scalar.activation(d1b[0:16, :], mp[0:16, :], AF.Relu)

            # dec2 = Wd2 @ relu1 + bd2, feature-major
            d2a = wkd.tile([P, 6, W], FR, tag=f"d2aw{wave}")
            d2b = wkd.tile([17, W], FR, tag=f"d2bw{wave}")
            nc.vector.tensor_copy(d2b, ones_c[0:17, 0:W])
            for m in range(7):
                mw = min(P, FEAT - m * P)
                mp = pp.tile([P, W], F32, tag="mp")
                msl = slice(m * P, m * P + mw)
                for c in range(6):
                    nc.tensor.matmul(mp[0:mw, :], d2_w[:, c, msl], d1a[:, c, :],
                                     start=(c == 0), stop=False)
                nc.tensor.matmul(mp[0:mw, :], d2_w[0:17, 6, msl], d1b,
                                 start=False, stop=True)
                if m < 6:
                    nc.scalar.copy(d2a[:, m, :], mp)
                else:
                    nc.scalar.copy(d2b[0:16, :], mp[0:16, :])

            # logits + softmax per subtile
            for j in range(2):
                jsl = slice(j * P, (j + 1) * P)
                lgt = pp.tile([P, W], F32, tag="mp")
                lg = lgt[:, 0:10]
                for c in range(6):
                    nc.tensor.matmul(lg, d2a[:, c, jsl], ow_w[:, c, :],
                                     start=(c == 0), stop=False)
                nc.tensor.matmul(lg, d2b[:, jsl], ow_w[0:17, 6, :],
                                 start=False, stop=True)
                e10 = sm.tile([P, 10], F32, tag="e10")
                s10 = sm.tile([P, 1], F32, tag="s10")
                nc.scalar.activation(e10, lg, AF.Exp, accum_out=s10)
                r10 = sm.tile([P, 1], F32, tag="r10")
                nc.vector.reciprocal(r10, s10)
                o10 = sm.tile([P, 10], F32, tag="o10")
                nc.vector.tensor_scalar_mul(o10, e10, r10)
                nc.sync.dma_start(
                    out=out_d[ds(g * (nsub * P) + (2 * wave + j) * P, P), :],
                    in_=o10
                )

        def body(g, preloaded=None):
            h0s, hbs, cs = [], [], [None, None]
            vts = []
            # k=0 proj+dots interleaved per tile so the first tile's chain
            # races ahead of later tiles' loads
            mtc0 = [mtp.tile([P, 8, T], F32, tag=f"mtk{w}", bufs=3,
                             name=f"mt0w{w}") for w in range(2)]
            for j in range(nsub):
                h0 = load_dma(g, j) if preloaded is None else preloaded[j]
                h0s.append(h0)
                vt1, vt2 = prep_tile(j, h0)
                vts.append((vt1, vt2))
                pk = proj(j, 0, vt1, vt2)
                dots(j, 0, mtc0[j // 2], pk)
            mtk = [mtc0]  # per-wave Mt tiles per k
            for k in range(1, 8):
                mtcs = [mtp.tile([P, 8, T], F32, tag=f"mtk{w}", bufs=3,
                                 name=f"mt{k}w{w}") for w in range(2)]
                for j in range(nsub):
                    pk = proj(j, k, *vts[j])
                    dots(j, k, mtcs[j // 2], pk)
                mtk.append(mtcs)
                # serial chains run one k behind so the independent dots of
                # the next k fill engine gaps while the chains ping-pong
                for w in range(2):
                    cs[w] = serial_phase(k - 1, w, mtk[k - 1][w], cs[w])
            # wave 0 finishes first and its decoder overlaps wave 1's tail
            cs[0] = serial_phase(7, 0, mtk[7][0], cs[0])
            ws0 = [recon(0, h0s[0], cs[0]), recon(1, h0s[1], cs[0])]
            cs[1] = serial_phase(7, 1, mtk[7][1], cs[1])
            decoder(ws0, g, 0)
            ws1 = [recon(2, h0s[2], cs[1]), recon(3, h0s[3], cs[1])]
            decoder(ws1, g, 1)

        # group 0's x DMAs first so they precede the 6MB of decoder weights
        # on the sync queue; weights stream in during attention
        pre0 = [load_dma(0, j) for j in range(nsub)]
        nc.sync.dma_start(out=d1_w, in_=d1_d[:, :, :])
        nc.sync.dma_start(out=d2_w, in_=d2_d[:, :, :])
        nc.sync.dma_start(out=ow_w, in_=ow_d[:, :, :])
        body(0, preloaded=pre0)
        if ngroups > 1:
            with tc.For_i(1, ngroups, 1) as g:
                body(g)
        for _pool in (pp, wkd, wp, sm, mtp, scp, pkp, vp, hp, consts):
            _pool.release()

    nc.compile()
    return nc


def pack_weights(W1, b1, W2, b2, W3, b3, Wd1, bd1, Wd2, bd2, Wo, bo):
    f64 = np.float64
    W1, b1, W2, b2, W3, b3 = (np.asarray(t, f64) for t in (W1, b1, W2, b2, W3, b3))
    G = W1.T @ W2
    a = W2.T @ b1

    A = np.eye(FV)
    m = np.zeros(FV)
    pw = np.zeros((P, 2, PTOT), np.float32)
    for k in range(8):
        Gk = A.T @ G @ A
        ak = A.T @ (G.T @ m + a)
        nco = NCOLS[k]
        Wk = np.zeros((197, nco), f64)
        r = RANKS[k]
        r1 = r + 1
        U, S, Vh = np.linalg.svd(Gk)
        Wk[:FV, :r] = (np.diag(S[:r]) @ Vh[:r]).T
        Wk[:FV, r] = ak
        Wk[:FV, r1 : r1 + r] = U[:, :r]
        Wk[FV, r1 + r] = 1.0
        off = POFF[k]
        pw[:, 0, off : off + nco] = Wk[0:128]
        pw[0:69, 1, off : off + nco] = Wk[128:197]
        A = W3 @ A
        m = W3 @ m + b3
    A8, m8 = A, m

    # fold W3^8 / m8 into the first decoder layer
    BD = np.zeros((FEAT, FEAT), f64)
    mm = np.zeros(FEAT, f64)
    for t in range(T):
        BD[t * FV : (t + 1) * FV, t * FV : (t + 1) * FV] = A8
        mm[t * FV : (t + 1) * FV] = m8
    Wd1f = np.asarray(Wd1, f64) @ BD
    bd1f = np.asarray(bd1, f64) + np.asarray(Wd1, f64) @ mm

    d1 = np.zeros((P, 8, FEAT), np.float32)
    W1T = Wd1f.T  # [784 f_in, 784 j]
    for t in range(T):
        d1[:, t, :] = W1T[t * FV : t * FV + P, :]
        d1[0:68, 4 + t, :] = W1T[t * FV + P : (t + 1) * FV, :]
    d1[68, 4, :] = bd1f

    d2 = np.zeros((P, 7, FEAT), np.float32)
    W2T = np.asarray(Wd2, f64).T
    for cidx in range(6):
        d2[:, cidx, :] = W2T[cidx * P : (cidx + 1) * P, :]
    d2[0:16, 6, :] = W2T[768:784, :]
    d2[16, 6, :] = np.asarray(bd2, f64)

    ow = np.zeros((P, 7, 10), np.float32)
    WoT = np.asarray(Wo, f64).T
    for cidx in range(6):
        ow[:, cidx, :] = WoT[cidx * P : (cidx + 1) * P, :]
    ow[0:16, 6, :] = WoT[768:784, :]
    ow[16, 6, :] = np.asarray(bo, f64)
    return pw.astype(ml_dtypes.bfloat16), d1, d2, ow


_NC_CACHE = {}


def kernel(**inputs):
    x = np.ascontiguousarray(np.asarray(inputs["x"], np.float32))
    zu, d1, d2, ow = pack_weights(
        inputs["W1"], inputs["b1"], inputs["W2"], inputs["b2"], inputs["W3"],
        inputs["b3"], inputs["Wd1"], inputs["bd1"], inputs["Wd2"],
        inputs["bd2"], inputs["Wo"], inputs["bo"],
    )
    if "nc" not in _NC_CACHE:
        _NC_CACHE["nc"] = build(4, 8)
    nc = _NC_CACHE["nc"]
    bpc = B // NCORES
    in_maps = [
        {
            "x": x[c * bpc : (c + 1) * bpc],
            "zu_w": zu,
            "dec1_w": d1,
            "dec2_w": d2,
            "out_w": ow,
        }
        for c in range(NCORES)
    ]
    res = run_bass_kernel_spmd(nc, in_maps, core_ids=list(range(NCORES)))
    return np.concatenate([res.results[c]["out"] for c in range(NCORES)], axis=0)


# revision 72
# speedup vs baseline: 3.1944x; 1.1385x over previous
"""Trainium2 Bass kernel for nn_CapsuleNeuralNetworkV2 (8 cores, data-parallel).

Reference math (per sample, 8 capsule iterations then decoder):
  v = h.reshape(4, 196); q,k,u = affine(v); scores = q k^T;
  P = softmax(scores); h' = P u;  dec = relu(h Wd1^T+bd1) Wd2^T+bd2;
  out = softmax(dec Wo^T + bo).

Restructuring (host-side algebra):
  Since each P has rows summing to 1, the state stays in the span of the 4
  initial slots: v^(k) = W3^k w^(k) + m_k with w^(k) = C^(k) V (C is a
  per-sample 4x4 convex-coefficient matrix, V the initial slots).
  scores^(k)[t,s] = C[t] M_k C[s]^T (mod per-t constants that cancel in
  softmax), where M_k[i,j] = v_i.(G_k v_j) + a_k.v_j depends only on the
  INITIAL slots: G_k = (W3^k)^T G W3^k, G = W1^T W2,
  a_k = (W3^k)^T (G^T m_k + W2^T b1).  G_k is numerically low-rank for k>=1
  (powers of a random matrix), so M_k is computed from rank-r_k SVD
  projections p_i = U_r^T v_i, q_j = (S V_r^T) v_j: M[i,j] ~ p_i.q_j + r_j.
  Per iteration only the tiny 4x4 chain is sequential:
  scores = C M C^T -> softmax -> C' = P C.  All projections/M_k are
  C-independent and pipeline on PE/Act/DVE ahead of the chain.
  Final w^(8) = C^(8) V; W3^8/m_8 are folded into Wd1/bd1 on the host.

Engines: PE transposes V once per tile + small bf16 projection matmuls +
decoder; DVE/Pool share the per-sample dot products and the 4x4 chain; Act
does PSUM evacuation, exp, and decoder activations.
"""

import numpy as np
import ml_dtypes

import concourse.bass as bass
import concourse.tile as tile
from concourse import bacc, mybir
from concourse.bass import ds
from concourse.bass_utils import run_bass_kernel_spmd
from concourse.masks import make_identity

FR = mybir.dt.float32r
BF = mybir.dt.bfloat16
F32 = mybir.dt.float32
AF = mybir.ActivationFunctionType
ALU = mybir.AluOpType

B = 32768
NCORES = 8
NSUB = 8
BPC = B // NCORES
P = 128
T = 4
FV = 196
FEAT = 784
SLOT = 198  # h slot: 196 data + ones col (196) + spare (197)

RANKS = [96, 48, 32, 24, 16, 12, 8, 8]
NCOLS = [2 * (r + 1) for r in RANKS]  # proj cols per slot per k
POFF = [0]
for _n in NCOLS:
    POFF.append(POFF[-1] + _n)
PTOT = POFF[-1]
NCMAX = max(NCOLS)


def _ap(t, dims, offset_elems=0):
    """Hand-built AP over a tile's tensor: dims = [[step, count], ...]."""
    a = t[:] if hasattr(t, "tile") or not isinstance(t, bass.AP) else t
    return bass.AP(tensor=a.tensor, offset=a.offset + offset_elems, ap=dims)


def build(nsub=8, ngroups=4):
    """One NeuronCore program processing nsub*ngroups*128 samples."""
    bpc = nsub * ngroups * P
    nc = bacc.Bacc("TRN2", target_bir_lowering=False, debug=False)

    x_d = nc.dram_tensor("x", [bpc, FEAT], FR, kind="ExternalInput")
    pw_d = nc.dram_tensor("zu_w", [P, 2, PTOT], BF, kind="ExternalInput")
    d1_d = nc.dram_tensor("dec1_w", [P, 8, FEAT], FR, kind="ExternalInput")
    d2_d = nc.dram_tensor("dec2_w", [P, 7, FEAT], FR, kind="ExternalInput")
    ow_d = nc.dram_tensor("out_w", [P, 7, 10], FR, kind="ExternalInput")
    out_d = nc.dram_tensor("out", [bpc, 10], F32, kind="ExternalOutput")

    with tile.TileContext(nc) as tc:
        consts = tc.alloc_tile_pool(name="consts", bufs=1)
        hp = tc.alloc_tile_pool(name="h", bufs=1)
        vp = tc.alloc_tile_pool(name="vt", bufs=1)
        pkp = tc.alloc_tile_pool(name="pk", bufs=2)
        scp = tc.alloc_tile_pool(name="scr", bufs=4)
        mtp = tc.alloc_tile_pool(name="mt", bufs=8)
        sm = tc.alloc_tile_pool(name="small", bufs=3)
        wp = tc.alloc_tile_pool(name="w", bufs=2)
        wkd = tc.alloc_tile_pool(name="wkd", bufs=1)
        pp = tc.alloc_tile_pool(name="ps", bufs=2, space="PSUM")

        ident_f = consts.tile([P, P], F32)
        make_identity(nc, ident_f)
        ident_r = consts.tile([P, P], FR)
        nc.vector.tensor_copy(ident_r, ident_f)
        ones_c = consts.tile([P, 512], F32)
        nc.vector.memset(ones_c, 1.0)
        pw = consts.tile([P, 2, PTOT], BF)
        nc.sync.dma_start(out=pw, in_=pw_d[:, :, :])
        # decoder weights DMA'd after group 0's x tiles (emitted in build
        # below) so the first group's compute isn't starved behind 6MB
        d1_w = consts.tile([P, 8, FEAT], FR)
        d2_w = consts.tile([P, 7, FEAT], FR)
        ow_w = consts.tile([P, 7, 10], FR)

        def load_dma(g, j):
            h0 = hp.tile([P, T, SLOT], FR, tag=f"h{j}")
            nc.sync.dma_start(
                out=h0[:, :, 0:FV],
                in_=x_d[ds(g * (nsub * P) + j * P, P), :].rearrange(
                    "p (t f) -> p t f", t=T
                ),
            )
            nc.gpsimd.tensor_copy(h0[:, :, 196:198], ones_c[:, 0 : 2 * T])
            return h0

        def prep_tile(j, h0):
            vt1 = vp.tile([P, T, P], BF, tag=f"vt1{j}")
            vt2 = vp.tile([69, T, P], BF, tag=f"vt2{j}")
            t1_ps = pp.tile([P, T, P], FR, tag="t1ps", bufs=1)
            t2_ps = pp.tile([69, T, P], FR, tag="t2ps", bufs=1)
            for t in range(T):
                nc.tensor.transpose(t1_ps[:, t, :], h0[:, t, 0:P], ident_r)
                nc.tensor.transpose(t2_ps[:, t, :], h0[:, t, P : P + 69], ident_r)
            nc.scalar.copy(vt1, t1_ps)
            nc.scalar.copy(vt2, t2_ps)
            return vt1, vt2

        def load_tile(g, j):
            h0 = load_dma(g, j)
            return h0, None, None, None

        def proj(j, k, vt1, vt2):
            """PE projections for iteration k -> pk [128, 4, nc] bf16."""
            nco = NCOLS[k]
            off = POFF[k]
            pk = pkp.tile([P, T, NCMAX], BF, tag=f"pk{j}")
            if k == 0:
                for half in range(2):
                    ps = pp.tile([P, 2, NCMAX], F32, tag="pkps", bufs=2)
                    for sl in range(2):
                        s = half * 2 + sl
                        nc.tensor.matmul(
                            ps[:, sl, 0:nco], vt1[:, s, :],
                            pw[:, 0, off : off + nco], start=True, stop=False)
                        nc.tensor.matmul(
                            ps[:, sl, 0:nco], vt2[0:69, s, :],
                            pw[0:69, 1, off : off + nco], start=False, stop=True)
                    nc.scalar.copy(
                        pk[:, 2 * half : 2 * half + 2, 0:nco], ps[:, :, 0:nco])
            else:
                ps = pp.tile([P, T, 98], F32, tag="pkps1", bufs=2)
                for s in range(T):
                    nc.tensor.matmul(
                        ps[:, s, 0:nco], vt1[:, s, :],
                        pw[:, 0, off : off + nco], start=True, stop=False)
                    nc.tensor.matmul(
                        ps[:, s, 0:nco], vt2[0:69, s, :],
                        pw[0:69, 1, off : off + nco], start=False, stop=True)
                nc.scalar.copy(pk[:, :, 0:nco], ps[:, :, 0:nco])
            return pk

        def dots(j, k, mtc, pk):
            """M_k[i,j] for all 16 slot pairs -> wave-mtc rows 4(j%2)+i."""
            r1 = RANKS[k] + 1
            pap = pk[:].ap[0]
            jw = j % 2
            if k == 0:
                # big r: fused stt (mult + f32 accumulate in one 1x pass)
                scr = scp.tile([P, 256], BF, tag="scr197", bufs=6)
                for idx in range(16):
                    i, jj = idx // 4, idx % 4
                    in0 = _ap(pk, [pap, [1, r1]],
                              offset_elems=i * NCMAX + r1)
                    in1 = _ap(pk, [pap, [1, r1]], offset_elems=jj * NCMAX)
                    nc.vector.scalar_tensor_tensor(
                        out=scr[:, 0:r1], in0=in0, scalar=1.0, in1=in1,
                        op0=ALU.mult, op1=ALU.mult,
                        accum_out=mtc[:, 4 * jw + i, jj : jj + 1])
            else:
                # small r: one bf16 2x tensor_tensor + one inner-axis reduce
                scr = scp.tile([P, T, T, 65], BF, tag="scr", bufs=3)
                in0 = _ap(pk, [pap, [NCMAX, 4], [0, 4], [1, r1]],
                          offset_elems=r1)
                in1 = _ap(pk, [pap, [0, 4], [NCMAX, 4], [1, r1]])
                nc.vector.tensor_tensor(
                    out=scr[:, :, :, 0:r1], in0=in0, in1=in1, op=ALU.mult)
                nc.vector.tensor_reduce(
                    out=mtc[:, 4 * jw : 4 * jw + 4, :], in_=scr[:, :, :, 0:r1],
                    axis=mybir.AxisListType.X, op=ALU.add)

        def serial_phase(k, w, mtc, c_prev):
            """Per-k 4x4 chain for one WAVE (2 tiles) in wide DVE ops over a
            [128, (j,t), s] layout (j in the wave): scores = C mt C^T ->
            e = exp -> C'u = e C -> C' = C'u / rowsum. Returns new C tile."""
            JT = 8   # (2 tiles) x (4 slots)
            JR = 32  # replicated size per tile pair
            if k == 0:
                s_t = mtc
            else:
                cap = c_prev[:].ap[0]
                # replicate C 4x -> crep[j, rep, s, jj] so every TT operand
                # stays within the ISA's 3-free-dim AP limit
                crep = sm.tile([P, 4 * JR], F32, tag=f"crep{w}", bufs=2)
                nc.gpsimd.tensor_copy(
                    _ap(crep, [crep[:].ap[0], [64, 2], [16, 4], [1, 16]]),
                    _ap(c_prev, [cap, [16, 2], [0, 4], [1, 16]]))
                tt_eng = nc.vector if k >= 4 else nc.gpsimd
                scrd = scp.tile([P, JT, T, T], F32, tag="scrd", bufs=6)
                tt_eng.tensor_tensor(  # D[j,i,s] = sum_jj mt[j,i,jj] C[j,s,jj]
                    out=scrd,
                    in0=_ap(mtc, [mtc[:].ap[0], [4, JT], [0, 4], [1, 4]]),
                    in1=crep[:],
                    op=ALU.mult)
                dm = sm.tile([P, JT, T], F32, tag=f"dm{w}")
                nc.vector.tensor_reduce(
                    out=dm, in_=scrd, axis=mybir.AxisListType.X, op=ALU.add)
                drep = sm.tile([P, 4 * JR], F32, tag=f"drep{w}", bufs=2)
                nc.gpsimd.tensor_copy(
                    _ap(drep, [drep[:].ap[0], [64, 2], [16, 4], [1, 16]]),
                    _ap(dm, [dm[:].ap[0], [16, 2], [0, 4], [1, 16]]))
                scrd2 = scp.tile([P, JT, T, T], F32, tag="scrd", bufs=6)
                tt_eng.tensor_tensor(  # S[j,t,s] = sum_i C[j,t,i] D[j,i,s]
                    out=scrd2,
                    in0=_ap(c_prev, [cap, [4, JT], [0, 4], [1, 4]]),
                    in1=_ap(drep, [drep[:].ap[0], [16, JT], [1, 4], [4, 4]]),
                    op=ALU.mult)
                s_t = sm.tile([P, JT, T], F32, tag=f"st{w}")
                nc.vector.tensor_reduce(
                    out=s_t, in_=scrd2, axis=mybir.AxisListType.X, op=ALU.add)
            e = sm.tile([P, JT, T], F32, tag=f"e{w}")
            nc.scalar.activation(e, s_t, AF.Exp)
            sums = sm.tile([P, JT], F32, tag=f"su{w}")
            nc.vector.reduce_sum(sums, e, axis=mybir.AxisListType.X)
            rec = sm.tile([P, JT], F32, tag=f"re{w}")
            nc.vector.reciprocal(rec, sums)
            if k == 0:
                cnum = e
            else:
                scrd3 = scp.tile([P, JT, T, T], F32, tag="scrd", bufs=6)
                tt_eng.tensor_tensor(  # C'u[j,t,jj] = sum_s e[j,t,s] C[j,s,jj]
                    out=scrd3,
                    in0=_ap(e, [e[:].ap[0], [4, JT], [0, 4], [1, 4]]),
                    in1=_ap(crep, [crep[:].ap[0], [16, JT], [1, 4], [4, 4]]),
                    op=ALU.mult)
                cnum = sm.tile([P, JT, T], F32, tag=f"cu{w}")
                nc.vector.tensor_reduce(
                    out=cnum, in_=scrd3, axis=mybir.AxisListType.X, op=ALU.add)
            c_new = sm.tile([P, JT, T], F32, tag=f"call{w}", bufs=3)
            nc.vector.scalar_tensor_tensor(
                out=c_new, in0=cnum, scalar=1.0,
                in1=_ap(rec, [rec[:].ap[0], [1, JT], [0, T]]),
                op0=ALU.mult, op1=ALU.mult)
            return c_new

        def recon(j, h0, c8):
            """w[:, t, :] = sum_s C8[t,s] * h0[:, s, :] (ones col rides along)."""
            w = wp.tile([P, T, SLOT], FR, tag=f"w{j % 4}", bufs=1)
            jw = j % 2
            for t in range(T):
                nc.scalar.activation(
                    w[:, t, :], h0[:, 0, :], AF.Copy,
                    scale=c8[:, 4 * jw + t, 0:1])
            for t in range(3):
                for s in range(1, T):
                    nc.vector.scalar_tensor_tensor(
                        out=w[:, t, :], in0=h0[:, s, :],
                        scalar=c8[:, 4 * jw + t, s : s + 1], in1=w[:, t, :],
                        op0=ALU.mult, op1=ALU.add)
            for t in range(3, T):
                pct = wp.tile([P, SLOT], F32, tag="pct", bufs=2)
                for s in range(1, T):
                    nc.gpsimd.tensor_scalar_mul(
                        pct, h0[:, s, :], c8[:, 4 * jw + t, s : s + 1])
                    nc.gpsimd.tensor_add(w[:, t, :], w[:, t, :], pct)
            return w

        def decoder(ws2, g, wave):
            """Decoder over one wave of 2 tiles (N = 256 wide matmuls)."""
            W = 2 * P
            ht1 = wkd.tile([P, T, W], FR, tag=f"ht1w{wave % 2}")
            ht2 = wkd.tile([69, T, W], FR, tag=f"ht2w{wave % 2}")
            for t in range(T):
                t1_ps = pp.tile([P, T, P], FR, tag="t1ps", bufs=1)
                t2_ps = pp.tile([69, T, P], FR, tag="t2ps", bufs=1)
                for j in range(2):
                    nc.tensor.transpose(
                        t1_ps[:, j, :], ws2[j][:, t, 0:P], ident_r
                    )
                    nc.tensor.transpose(
                        t2_ps[:, j, :], ws2[j][:, t, P : P + 69], ident_r
                    )
                nc.scalar.copy(ht1[:, t, :], t1_ps[:, 0:2, :])
                nc.vector.tensor_copy(ht2[:, t, :], t2_ps[:, 0:2, :])

            # dec1 = relu(Wd1~ @ w.T + bd1~), feature-major, 7 M-chunks
            d1a = wkd.tile([P, 6, W], FR, tag=f"d1aw{wave % 2}")
            d1b = wkd.tile([17, W], FR, tag=f"d1bw{wave % 2}")
            nc.vector.tensor_copy(d1b, ones_c[0:17, 0:W])
            for m in range(7):
                mw = min(P, FEAT - m * P)
                mp = pp.tile([P, W], F32, tag="mp")
                msl = slice(m * P, m * P + mw)
                for t in range(T):
                    nc.tensor.matmul(mp[0:mw, :], d1_w[:, t, msl], ht1[:, t, :],
                                     start=(t == 0), stop=False)
                for t in range(T):
                    nc.tensor.matmul(mp[0:mw, :], d1_w[0:69, 4 + t, msl],
                                     ht2[:, t, :], start=False, stop=(t == 3))
                if m < 6:
                    nc.scalar.activation(d1a[:, m, :], mp, AF.Relu)
                else:
                    nc.scalar.activation(d1b[0:16, :], mp[0:16, :], AF.Relu)

            # dec2 = Wd2 @ relu1 + bd2, feature-major
            d2a = wkd.tile([P, 6, W], FR, tag=f"d2aw{wave % 2}")
            d2b = wkd.tile([17, W], FR, tag=f"d2bw{wave % 2}")
            nc.vector.tensor_copy(d2b, ones_c[0:17, 0:W])
            for m in range(7):
                mw = min(P, FEAT - m * P)
                mp = pp.tile([P, W], F32, tag="mp")
                msl = slice(m * P, m * P + mw)
                for c in range(6):
                    nc.tensor.matmul(mp[0:mw, :], d2_w[:, c, msl], d1a[:, c, :],
                                     start=(c == 0), stop=False)
                nc.tensor.matmul(mp[0:mw, :], d2_w[0:17, 6, msl], d1b,
                                 start=False, stop=True)
                if m < 6:
                    nc.scalar.copy(d2a[:, m, :], mp)
                else:
                    nc.scalar.copy(d2b[0:16, :], mp[0:16, :])

            # logits + softmax per subtile
            for j in range(2):
                jsl = slice(j * P, (j + 1) * P)
                lgt = pp.tile([P, W], F32, tag="mp")
                lg = lgt[:, 0:10]
                for c in range(6):
                    nc.tensor.matmul(lg, d2a[:, c, jsl], ow_w[:, c, :],
                                     start=(c == 0), stop=False)
                nc.tensor.matmul(lg, d2b[:, jsl], ow_w[0:17, 6, :],
                                 start=False, stop=True)
                e10 = sm.tile([P, 10], F32, tag="e10")
                s10 = sm.tile([P, 1], F32, tag="s10")
                nc.scalar.activation(e10, lg, AF.Exp, accum_out=s10)
                r10 = sm.tile([P, 1], F32, tag="r10")
                nc.vector.reciprocal(r10, s10)
                o10 = sm.tile([P, 10], F32, tag="o10")
                nc.vector.tensor_scalar_mul(o10, e10, r10)
                nc.sync.dma_start(
                    out=out_d[ds(g * (nsub * P) + (2 * wave + j) * P, P), :],
                    in_=o10
                )

        def body(g, preloaded=None):
            nw = nsub // 2
            h0s, cs = [], [None] * nw
            vts = []
            # k=0 proj+dots interleaved per tile so the first tile's chain
            # races ahead of later tiles' loads
            mtc0 = [mtp.tile([P, 8, T], F32, tag=f"mtk{w}", bufs=3,
                             name=f"mt0w{w}") for w in range(nw)]
            for j in range(nsub):
                h0 = load_dma(g, j) if preloaded is None else preloaded[j]
                h0s.append(h0)
                vt1, vt2 = prep_tile(j, h0)
                vts.append((vt1, vt2))
                pk = proj(j, 0, vt1, vt2)
                dots(j, 0, mtc0[j // 2], pk)
            mtk = [mtc0]  # per-wave Mt tiles per k
            for k in range(1, 8):
                mtcs = [mtp.tile([P, 8, T], F32, tag=f"mtk{w}", bufs=3,
                                 name=f"mt{k}w{w}") for w in range(nw)]
                for j in range(nsub):
                    pk = proj(j, k, *vts[j])
                    dots(j, k, mtcs[j // 2], pk)
                mtk.append(mtcs)
                # serial chains run one k behind so the independent dots of
                # the next k fill engine gaps while the chains ping-pong
                for w in range(nw):
                    cs[w] = serial_phase(k - 1, w, mtk[k - 1][w], cs[w])
            # each wave's decoder overlaps the next wave's serial tail
            for w in range(nw):
                cs[w] = serial_phase(7, w, mtk[7][w], cs[w])
                if w > 0:
                    decoder(wsp, g, w - 1)
                wsp = [recon(2 * w, h0s[2 * w], cs[w]),
                       recon(2 * w + 1, h0s[2 * w + 1], cs[w])]
            decoder(wsp, g, nw - 1)

        # group 0's x DMAs first so they precede the 6MB of decoder weights
        # on the sync queue; weights stream in during attention
        pre0 = [load_dma(0, j) for j in range(nsub)]
        nc.sync.dma_start(out=d1_w, in_=d1_d[:, :, :])
        nc.sync.dma_start(out=d2_w, in_=d2_d[:, :, :])
        nc.sync.dma_start(out=ow_w, in_=ow_d[:, :, :])
        body(0, preloaded=pre0)
        if ngroups > 1:
            with tc.For_i(1, ngroups, 1) as g:
                body(g)
        for _pool in (pp, wkd, wp, sm, mtp, scp, pkp, vp, hp, consts):
            _pool.release()

    nc.compile()
    return nc


def pack_weights(W1, b1, W2, b2, W3, b3, Wd1, bd1, Wd2, bd2, Wo, bo):
    f64 = np.float64
    W1, b1, W2, b2, W3, b3 = (np.asarray(t, f64) for t in (W1, b1, W2, b2, W3, b3))
    G = W1.T @ W2
    a = W2.T @ b1

    A = np.eye(FV)
    m = np.zeros(FV)
    pw = np.zeros((P, 2, PTOT), np.float32)
    for k in range(8):
        Gk = A.T @ G @ A
        ak = A.T @ (G.T @ m + a)
        nco = NCOLS[k]
        Wk = np.zeros((197, nco), f64)
        r = RANKS[k]
        r1 = r + 1
        U, S, Vh = np.linalg.svd(Gk)
        Wk[:FV, :r] = (np.diag(S[:r]) @ Vh[:r]).T
        Wk[:FV, r] = ak
        Wk[:FV, r1 : r1 + r] = U[:, :r]
        Wk[FV, r1 + r] = 1.0
        off = POFF[k]
        pw[:, 0, off : off + nco] = Wk[0:128]
        pw[0:69, 1, off : off + nco] = Wk[128:197]
        A = W3 @ A
        m = W3 @ m + b3
    A8, m8 = A, m

    # fold W3^8 / m8 into the first decoder layer
    BD = np.zeros((FEAT, FEAT), f64)
    mm = np.zeros(FEAT, f64)
    for t in range(T):
        BD[t * FV : (t + 1) * FV, t * FV : (t + 1) * FV] = A8
        mm[t * FV : (t + 1) * FV] = m8
    Wd1f = np.asarray(Wd1, f64) @ BD
    bd1f = np.asarray(bd1, f64) + np.asarray(Wd1, f64) @ mm

    d1 = np.zeros((P, 8, FEAT), np.float32)
    W1T = Wd1f.T  # [784 f_in, 784 j]
    for t in range(T):
        d1[:, t, :] = W1T[t * FV : t * FV + P, :]
        d1[0:68, 4 + t, :] = W1T[t * FV + P : (t + 1) * FV, :]
    d1[68, 4, :] = bd1f

    d2 = np.zeros((P, 7, FEAT), np.float32)
    W2T = np.asarray(Wd2, f64).T
    for cidx in range(6):
        d2[:, cidx, :] = W2T[cidx * P : (cidx + 1) * P, :]
    d2[0:16, 6, :] = W2T[768:784, :]
    d2[16, 6, :] = np.asarray(bd2, f64)

    ow = np.zeros((P, 7, 10), np.float32)
    WoT = np.asarray(Wo, f64).T
    for cidx in range(6):
        ow[:, cidx, :] = WoT[cidx * P : (cidx + 1) * P, :]
    ow[0:16, 6, :] = WoT[768:784, :]
    ow[16, 6, :] = np.asarray(bo, f64)
    return pw.astype(ml_dtypes.bfloat16), d1, d2, ow


_NC_CACHE = {}


def kernel(**inputs):
    x = np.ascontiguousarray(np.asarray(inputs["x"], np.float32))
    zu, d1, d2, ow = pack_weights(
        inputs["W1"], inputs["b1"], inputs["W2"], inputs["b2"], inputs["W3"],
        inputs["b3"], inputs["Wd1"], inputs["bd1"], inputs["Wd2"],
        inputs["bd2"], inputs["Wo"], inputs["bo"],
    )
    if "nc" not in _NC_CACHE:
        _NC_CACHE["nc"] = build(NSUB, BPC // (NSUB * P))
    nc = _NC_CACHE["nc"]
    bpc = B // NCORES
    in_maps = [
        {
            "x": x[c * bpc : (c + 1) * bpc],
            "zu_w": zu,
            "dec1_w": d1,
            "dec2_w": d2,
            "out_w": ow,
        }
        for c in range(NCORES)
    ]
    res = run_bass_kernel_spmd(nc, in_maps, core_ids=list(range(NCORES)))
    return np.concatenate([res.results[c]["out"] for c in range(NCORES)], axis=0)


# revision 73
# speedup vs baseline: 3.5335x; 1.1062x over previous
"""Trainium2 Bass kernel for nn_CapsuleNeuralNetworkV2 (8 cores, data-parallel).

Reference math (per sample, 8 capsule iterations then decoder):
  v = h.reshape(4, 196); q,k,u = affine(v); scores = q k^T;
  P = softmax(scores); h' = P u;  dec = relu(h Wd1^T+bd1) Wd2^T+bd2;
  out = softmax(dec Wo^T + bo).

Restructuring (host-side algebra):
  Since each P has rows summing to 1, the state stays in the span of the 4
  initial slots: v^(k) = W3^k w^(k) + m_k with w^(k) = C^(k) V (C is a
  per-sample 4x4 convex-coefficient matrix, V the initial slots).
  scores^(k)[t,s] = C[t] M_k C[s]^T (mod per-t constants that cancel in
  softmax), where M_k[i,j] = v_i.(G_k v_j) + a_k.v_j depends only on the
  INITIAL slots: G_k = (W3^k)^T G W3^k, G = W1^T W2,
  a_k = (W3^k)^T (G^T m_k + W2^T b1).  G_k is numerically low-rank for k>=1
  (powers of a random matrix), so M_k is computed from rank-r_k SVD
  projections p_i = U_r^T v_i, q_j = (S V_r^T) v_j: M[i,j] ~ p_i.q_j + r_j.
  Per iteration only the tiny 4x4 chain is sequential:
  scores = C M C^T -> softmax -> C' = P C.  All projections/M_k are
  C-independent and pipeline on PE/Act/DVE ahead of the chain.
  Final w^(8) = C^(8) V; W3^8/m_8 are folded into Wd1/bd1 on the host.

Engines: PE transposes V once per tile + small bf16 projection matmuls +
decoder; DVE/Pool share the per-sample dot products and the 4x4 chain; Act
does PSUM evacuation, exp, and decoder activations.
"""

import numpy as np
import ml_dtypes

import concourse.bass as bass
import concourse.tile as tile
from concourse import bacc, mybir
from concourse.bass import ds
from concourse.bass_utils import run_bass_kernel_spmd
from concourse.masks import make_identity

FR = mybir.dt.float32r
BF = mybir.dt.bfloat16
F32 = mybir.dt.float32
AF = mybir.ActivationFunctionType
ALU = mybir.AluOpType

B = 32768
NCORES = 8
NSUB = 8
BPC = B // NCORES
P = 128
T = 4
FV = 196
FEAT = 784
SLOT = 198  # h slot: 196 data + ones col (196) + spare (197)

RANKS = [96, 48, 32, 24, 16, 12, 8, 8]
NCOLS = [2 * (r + 1) for r in RANKS]  # proj cols per slot per k
POFF = [0]
for _n in NCOLS:
    POFF.append(POFF[-1] + _n)
PTOT = POFF[-1]
NCMAX = max(NCOLS)


def _ap(t, dims, offset_elems=0):
    """Hand-built AP over a tile's tensor: dims = [[step, count], ...]."""
    a = t[:] if hasattr(t, "tile") or not isinstance(t, bass.AP) else t
    return bass.AP(tensor=a.tensor, offset=a.offset + offset_elems, ap=dims)


def build(nsub=8, ngroups=4):
    """One NeuronCore program processing nsub*ngroups*128 samples."""
    bpc = nsub * ngroups * P
    nc = bacc.Bacc("TRN2", target_bir_lowering=False, debug=False)

    x_d = nc.dram_tensor("x", [bpc, FEAT], FR, kind="ExternalInput")
    pw_d = nc.dram_tensor("zu_w", [P, 2, PTOT], BF, kind="ExternalInput")
    d1_d = nc.dram_tensor("dec1_w", [P, 8, FEAT], FR, kind="ExternalInput")
    d2_d = nc.dram_tensor("dec2_w", [P, 7, FEAT], FR, kind="ExternalInput")
    ow_d = nc.dram_tensor("out_w", [P, 7, 10], FR, kind="ExternalInput")
    out_d = nc.dram_tensor("out", [bpc, 10], F32, kind="ExternalOutput")

    with tile.TileContext(nc) as tc:
        consts = tc.alloc_tile_pool(name="consts", bufs=1)
        hp = tc.alloc_tile_pool(name="h", bufs=1)
        vp = tc.alloc_tile_pool(name="vt", bufs=1)
        pkp = tc.alloc_tile_pool(name="pk", bufs=2)
        scp = tc.alloc_tile_pool(name="scr", bufs=4)
        mtp = tc.alloc_tile_pool(name="mt", bufs=8)
        sm = tc.alloc_tile_pool(name="small", bufs=3)
        wp = tc.alloc_tile_pool(name="w", bufs=2)
        wkd = tc.alloc_tile_pool(name="wkd", bufs=1)
        pp = tc.alloc_tile_pool(name="ps", bufs=2, space="PSUM")

        ident_f = consts.tile([P, P], F32)
        make_identity(nc, ident_f)
        ident_r = consts.tile([P, P], FR)
        nc.vector.tensor_copy(ident_r, ident_f)
        ones_c = consts.tile([P, 512], F32)
        nc.vector.memset(ones_c, 1.0)
        pw = consts.tile([P, 2, PTOT], BF)
        nc.sync.dma_start(out=pw, in_=pw_d[:, :, :])
        # decoder weights DMA'd after group 0's x tiles (emitted in build
        # below) so the first group's compute isn't starved behind 6MB
        d1_w = consts.tile([P, 8, FEAT], FR)
        d2_w = consts.tile([P, 7, FEAT], FR)
        ow_w = consts.tile([P, 7, 10], FR)

        def load_dma(g, j):
            h0 = hp.tile([P, T, SLOT], FR, tag=f"h{j}")
            nc.sync.dma_start(
                out=h0[:, :, 0:FV],
                in_=x_d[ds(g * (nsub * P) + j * P, P), :].rearrange(
                    "p (t f) -> p t f", t=T
                ),
            )
            nc.gpsimd.tensor_copy(h0[:, :, 196:198], ones_c[:, 0 : 2 * T])
            return h0

        def prep_tile(j, h0):
            vt1 = vp.tile([P, T, P], BF, tag=f"vt1{j}")
            vt2 = vp.tile([69, T, P], BF, tag=f"vt2{j}")
            t1_ps = pp.tile([P, T, P], FR, tag="t1ps", bufs=1)
            t2_ps = pp.tile([69, T, P], FR, tag="t2ps", bufs=1)
            for t in range(T):
                nc.tensor.transpose(t1_ps[:, t, :], h0[:, t, 0:P], ident_r)
                nc.tensor.transpose(t2_ps[:, t, :], h0[:, t, P : P + 69], ident_r)
            nc.scalar.copy(vt1, t1_ps)
            nc.scalar.copy(vt2, t2_ps)
            return vt1, vt2

        def load_tile(g, j):
            h0 = load_dma(g, j)
            return h0, None, None, None

        def proj(j, k, vt1, vt2):
            """PE projections for iteration k -> pk [128, 4, nc] bf16."""
            nco = NCOLS[k]
            off = POFF[k]
            pk = pkp.tile([P, T, NCMAX], BF, tag=f"pk{j}")
            if k == 0:
                for half in range(2):
                    ps = pp.tile([P, 2, NCMAX], F32, tag="pkps", bufs=2)
                    for sl in range(2):
                        s = half * 2 + sl
                        nc.tensor.matmul(
                            ps[:, sl, 0:nco], vt1[:, s, :],
                            pw[:, 0, off : off + nco], start=True, stop=False)
                        nc.tensor.matmul(
                            ps[:, sl, 0:nco], vt2[0:69, s, :],
                            pw[0:69, 1, off : off + nco], start=False, stop=True)
                    nc.scalar.copy(
                        pk[:, 2 * half : 2 * half + 2, 0:nco], ps[:, :, 0:nco])
            else:
                ps = pp.tile([P, T, 98], F32, tag="pkps1", bufs=2)
                for s in range(T):
                    nc.tensor.matmul(
                        ps[:, s, 0:nco], vt1[:, s, :],
                        pw[:, 0, off : off + nco], start=True, stop=False)
                    nc.tensor.matmul(
                        ps[:, s, 0:nco], vt2[0:69, s, :],
                        pw[0:69, 1, off : off + nco], start=False, stop=True)
                nc.scalar.copy(pk[:, :, 0:nco], ps[:, :, 0:nco])
            return pk

        def dots(j, k, mtc, pk):
            """M_k[i,j] for all 16 slot pairs -> wave-mtc rows 4(j%2)+i."""
            r1 = RANKS[k] + 1
            pap = pk[:].ap[0]
            jw = j % 2
            if k == 0:
                # big r: fused stt (mult + f32 accumulate in one 1x pass)
                scr = scp.tile([P, 256], BF, tag="scr197", bufs=6)
                for idx in range(16):
                    i, jj = idx // 4, idx % 4
                    in0 = _ap(pk, [pap, [1, r1]],
                              offset_elems=i * NCMAX + r1)
                    in1 = _ap(pk, [pap, [1, r1]], offset_elems=jj * NCMAX)
                    nc.vector.scalar_tensor_tensor(
                        out=scr[:, 0:r1], in0=in0, scalar=1.0, in1=in1,
                        op0=ALU.mult, op1=ALU.mult,
                        accum_out=mtc[:, 4 * jw + i, jj : jj + 1])
            else:
                # small r: one bf16 2x tensor_tensor + one inner-axis reduce
                scr = scp.tile([P, T, T, 65], BF, tag="scr", bufs=3)
                in0 = _ap(pk, [pap, [NCMAX, 4], [0, 4], [1, r1]],
                          offset_elems=r1)
                in1 = _ap(pk, [pap, [0, 4], [NCMAX, 4], [1, r1]])
                nc.vector.tensor_tensor(
                    out=scr[:, :, :, 0:r1], in0=in0, in1=in1, op=ALU.mult)
                nc.vector.tensor_reduce(
                    out=mtc[:, 4 * jw : 4 * jw + 4, :], in_=scr[:, :, :, 0:r1],
                    axis=mybir.AxisListType.X, op=ALU.add)

        def serial_phase(k, w, mtc, c_prev):
            """Per-k 4x4 chain for one WAVE (2 tiles) in wide DVE ops over a
            [128, (j,t), s] layout (j in the wave): scores = C mt C^T ->
            e = exp -> C'u = e C -> C' = C'u / rowsum. Returns new C tile."""
            JT = 8   # (2 tiles) x (4 slots)
            JR = 32  # replicated size per tile pair
            if k == 0:
                s_t = mtc
            else:
                cap = c_prev[:].ap[0]
                # replicate C 4x -> crep[j, rep, s, jj] so every TT operand
                # stays within the ISA's 3-free-dim AP limit
                crep = sm.tile([P, 4 * JR], F32, tag=f"crep{w}", bufs=2)
                nc.gpsimd.tensor_copy(
                    _ap(crep, [crep[:].ap[0], [64, 2], [16, 4], [1, 16]]),
                    _ap(c_prev, [cap, [16, 2], [0, 4], [1, 16]]))
                tt_eng = nc.vector if k >= 4 else nc.gpsimd
                scrd = scp.tile([P, JT, T, T], F32, tag="scrd", bufs=6)
                tt_eng.tensor_tensor(  # D[j,i,s] = sum_jj mt[j,i,jj] C[j,s,jj]
                    out=scrd,
                    in0=_ap(mtc, [mtc[:].ap[0], [4, JT], [0, 4], [1, 4]]),
                    in1=crep[:],
                    op=ALU.mult)
                dm = sm.tile([P, JT, T], F32, tag=f"dm{w}")
                nc.vector.tensor_reduce(
                    out=dm, in_=scrd, axis=mybir.AxisListType.X, op=ALU.add)
                drep = sm.tile([P, 4 * JR], F32, tag=f"drep{w}", bufs=2)
                nc.gpsimd.tensor_copy(
                    _ap(drep, [drep[:].ap[0], [64, 2], [16, 4], [1, 16]]),
                    _ap(dm, [dm[:].ap[0], [16, 2], [0, 4], [1, 16]]))
                scrd2 = scp.tile([P, JT, T, T], F32, tag="scrd", bufs=6)
                tt_eng.tensor_tensor(  # S[j,t,s] = sum_i C[j,t,i] D[j,i,s]
                    out=scrd2,
                    in0=_ap(c_prev, [cap, [4, JT], [0, 4], [1, 4]]),
                    in1=_ap(drep, [drep[:].ap[0], [16, JT], [1, 4], [4, 4]]),
                    op=ALU.mult)
                s_t = sm.tile([P, JT, T], F32, tag=f"st{w}")
                nc.vector.tensor_reduce(
                    out=s_t, in_=scrd2, axis=mybir.AxisListType.X, op=ALU.add)
            e = sm.tile([P, JT, T], F32, tag=f"e{w}")
            nc.scalar.activation(e, s_t, AF.Exp)
            sums = sm.tile([P, JT], F32, tag=f"su{w}")
            nc.vector.reduce_sum(sums, e, axis=mybir.AxisListType.X)
            rec = sm.tile([P, JT], F32, tag=f"re{w}")
            nc.vector.reciprocal(rec, sums)
            if k == 0:
                cnum = e
            else:
                scrd3 = scp.tile([P, JT, T, T], F32, tag="scrd", bufs=6)
                tt_eng.tensor_tensor(  # C'u[j,t,jj] = sum_s e[j,t,s] C[j,s,jj]
                    out=scrd3,
                    in0=_ap(e, [e[:].ap[0], [4, JT], [0, 4], [1, 4]]),
                    in1=_ap(crep, [crep[:].ap[0], [16, JT], [1, 4], [4, 4]]),
                    op=ALU.mult)
                cnum = sm.tile([P, JT, T], F32, tag=f"cu{w}")
                nc.vector.tensor_reduce(
                    out=cnum, in_=scrd3, axis=mybir.AxisListType.X, op=ALU.add)
            c_new = sm.tile([P, JT, T], F32, tag=f"call{w}", bufs=3)
            nc.vector.scalar_tensor_tensor(
                out=c_new, in0=cnum, scalar=1.0,
                in1=_ap(rec, [rec[:].ap[0], [1, JT], [0, T]]),
                op0=ALU.mult, op1=ALU.mult)
            return c_new

        def recon(j, h0, c8):
            """w[:, t, :] = sum_s C8[t,s] * h0[:, s, :] (ones col rides along)."""
            w = wp.tile([P, T, SLOT], FR, tag=f"w{j % 4}", bufs=1)
            jw = j % 2
            for t in range(T):
                nc.scalar.activation(
                    w[:, t, :], h0[:, 0, :], AF.Copy,
                    scale=c8[:, 4 * jw + t, 0:1])
            for t in range(3):
                for s in range(1, T):
                    nc.vector.scalar_tensor_tensor(
                        out=w[:, t, :], in0=h0[:, s, :],
                        scalar=c8[:, 4 * jw + t, s : s + 1], in1=w[:, t, :],
                        op0=ALU.mult, op1=ALU.add)
            for t in range(3, T):
                pct = wp.tile([P, SLOT], F32, tag="pct", bufs=2)
                for s in range(1, T):
                    nc.gpsimd.tensor_scalar_mul(
                        pct, h0[:, s, :], c8[:, 4 * jw + t, s : s + 1])
                    nc.gpsimd.tensor_add(w[:, t, :], w[:, t, :], pct)
            return w

        def decoder(ws2, g, wave):
            """Decoder over one wave of 2 tiles (N = 256 wide matmuls)."""
            W = 2 * P
            ht1 = wkd.tile([P, T, W], FR, tag=f"ht1w{wave % 2}")
            ht2 = wkd.tile([69, T, W], FR, tag=f"ht2w{wave % 2}")
            for t in range(T):
                t1_ps = pp.tile([P, T, P], FR, tag="t1ps", bufs=1)
                t2_ps = pp.tile([69, T, P], FR, tag="t2ps", bufs=1)
                for j in range(2):
                    nc.tensor.transpose(
                        t1_ps[:, j, :], ws2[j][:, t, 0:P], ident_r
                    )
                    nc.tensor.transpose(
                        t2_ps[:, j, :], ws2[j][:, t, P : P + 69], ident_r
                    )
                nc.scalar.copy(ht1[:, t, :], t1_ps[:, 0:2, :])
                nc.vector.tensor_copy(ht2[:, t, :], t2_ps[:, 0:2, :])

            # dec1 = relu(Wd1~ @ w.T + bd1~), feature-major, 7 M-chunks
            d1a = wkd.tile([P, 6, W], FR, tag=f"d1aw{wave % 2}")
            d1b = wkd.tile([17, W], FR, tag=f"d1bw{wave % 2}")
            nc.vector.tensor_copy(d1b, ones_c[0:17, 0:W])
            for m in range(7):
                mw = min(P, FEAT - m * P)
                mp = pp.tile([P, W], F32, tag="mp")
                msl = slice(m * P, m * P + mw)
                for t in range(T):
                    nc.tensor.matmul(mp[0:mw, :], d1_w[:, t, msl], ht1[:, t, :],
                                     start=(t == 0), stop=False)
                for t in range(T):
                    nc.tensor.matmul(mp[0:mw, :], d1_w[0:69, 4 + t, msl],
                                     ht2[:, t, :], start=False, stop=(t == 3))
                if m < 6:
                    nc.scalar.activation(d1a[:, m, :], mp, AF.Relu)
                else:
                    nc.scalar.activation(d1b[0:16, :], mp[0:16, :], AF.Relu)

            # dec2 = Wd2 @ relu1 + bd2, feature-major
            d2a = wkd.tile([P, 6, W], FR, tag=f"d2aw{wave % 2}")
            d2b = wkd.tile([17, W], FR, tag=f"d2bw{wave % 2}")
            nc.vector.tensor_copy(d2b, ones_c[0:17, 0:W])
            for m in range(7):
                mw = min(P, FEAT - m * P)
                mp = pp.tile([P, W], F32, tag="mp")
                msl = slice(m * P, m * P + mw)
                for c in range(6):
                    nc.tensor.matmul(mp[0:mw, :], d2_w[:, c, msl], d1a[:, c, :],
                                     start=(c == 0), stop=False)
                nc.tensor.matmul(mp[0:mw, :], d2_w[0:17, 6, msl], d1b,
                                 start=False, stop=True)
                if m < 6:
                    nc.scalar.copy(d2a[:, m, :], mp)
                else:
                    nc.scalar.copy(d2b[0:16, :], mp[0:16, :])

            # logits + softmax per subtile
            for j in range(2):
                jsl = slice(j * P, (j + 1) * P)
                lgt = pp.tile([P, W], F32, tag="mp")
                lg = lgt[:, 0:10]
                for c in range(6):
                    nc.tensor.matmul(lg, d2a[:, c, jsl], ow_w[:, c, :],
                                     start=(c == 0), stop=False)
                nc.tensor.matmul(lg, d2b[:, jsl], ow_w[0:17, 6, :],
                                 start=False, stop=True)
                e10 = sm.tile([P, 10], F32, tag="e10")
                s10 = sm.tile([P, 1], F32, tag="s10")
                nc.scalar.activation(e10, lg, AF.Exp, accum_out=s10)
                r10 = sm.tile([P, 1], F32, tag="r10")
                nc.vector.reciprocal(r10, s10)
                o10 = sm.tile([P, 10], F32, tag="o10")
                nc.vector.tensor_scalar_mul(o10, e10, r10)
                nc.sync.dma_start(
                    out=out_d[ds(g * (nsub * P) + (2 * wave + j) * P, P), :],
                    in_=o10
                )

        def body(g, preloaded=None):
            nw = nsub // 2
            h0s, cs = [], [None] * nw
            vts = []
            mtk = {}  # (wave, k) -> Mt tile
            # k=0 proj+dots interleaved per tile so the first tile's chain
            # races ahead of later tiles' loads
            for j in range(nsub):
                w = j // 2
                if j % 2 == 0:
                    mtk[(w, 0)] = mtp.tile([P, 8, T], F32, tag=f"mtk{w}",
                                           bufs=3, name=f"mt0w{w}")
                h0 = load_dma(g, j) if preloaded is None else preloaded[j]
                h0s.append(h0)
                vt1, vt2 = prep_tile(j, h0)
                vts.append((vt1, vt2))
                pk = proj(j, 0, vt1, vt2)
                dots(j, 0, mtk[(w, 0)], pk)
            # waves are staggered one k apart: early waves finish their
            # chains (and start decoding) while late waves still compute
            for step in range(1, 8 + nw):
                for w in range(nw):
                    k = step - w
                    if 1 <= k <= 7:
                        mtk[(w, k)] = mtp.tile([P, 8, T], F32, tag=f"mtk{w}",
                                               bufs=3, name=f"mt{k}w{w}")
                        for j in (2 * w, 2 * w + 1):
                            pk = proj(j, k, *vts[j])
                            dots(j, k, mtk[(w, k)], pk)
                        cs[w] = serial_phase(k - 1, w, mtk[(w, k - 1)], cs[w])
                    elif k == 8:
                        cs[w] = serial_phase(7, w, mtk[(w, 7)], cs[w])
                        wsp = [recon(2 * w, h0s[2 * w], cs[w]),
                               recon(2 * w + 1, h0s[2 * w + 1], cs[w])]
                        decoder(wsp, g, w)

        # group 0's x DMAs first so they precede the 6MB of decoder weights
        # on the sync queue; weights stream in during attention
        pre0 = [load_dma(0, j) for j in range(nsub)]
        nc.sync.dma_start(out=d1_w, in_=d1_d[:, :, :])
        nc.sync.dma_start(out=d2_w, in_=d2_d[:, :, :])
        nc.sync.dma_start(out=ow_w, in_=ow_d[:, :, :])
        body(0, preloaded=pre0)
        if ngroups > 1:
            with tc.For_i(1, ngroups, 1) as g:
                body(g)
        for _pool in (pp, wkd, wp, sm, mtp, scp, pkp, vp, hp, consts):
            _pool.release()

    nc.compile()
    return nc


def pack_weights(W1, b1, W2, b2, W3, b3, Wd1, bd1, Wd2, bd2, Wo, bo):
    f64 = np.float64
    W1, b1, W2, b2, W3, b3 = (np.asarray(t, f64) for t in (W1, b1, W2, b2, W3, b3))
    G = W1.T @ W2
    a = W2.T @ b1

    A = np.eye(FV)
    m = np.zeros(FV)
    pw = np.zeros((P, 2, PTOT), np.float32)
    for k in range(8):
        Gk = A.T @ G @ A
        ak = A.T @ (G.T @ m + a)
        nco = NCOLS[k]
        Wk = np.zeros((197, nco), f64)
        r = RANKS[k]
        r1 = r + 1
        U, S, Vh = np.linalg.svd(Gk)
        Wk[:FV, :r] = (np.diag(S[:r]) @ Vh[:r]).T
        Wk[:FV, r] = ak
        Wk[:FV, r1 : r1 + r] = U[:, :r]
        Wk[FV, r1 + r] = 1.0
        off = POFF[k]
        pw[:, 0, off : off + nco] = Wk[0:128]
        pw[0:69, 1, off : off + nco] = Wk[128:197]
        A = W3 @ A
        m = W3 @ m + b3
    A8, m8 = A, m

    # fold W3^8 / m8 into the first decoder layer
    BD = np.zeros((FEAT, FEAT), f64)
    mm = np.zeros(FEAT, f64)
    for t in range(T):
        BD[t * FV : (t + 1) * FV, t * FV : (t + 1) * FV] = A8
        mm[t * FV : (t + 1) * FV] = m8
    Wd1f = np.asarray(Wd1, f64) @ BD
    bd1f = np.asarray(bd1, f64) + np.asarray(Wd1, f64) @ mm

    d1 = np.zeros((P, 8, FEAT), np.float32)
    W1T = Wd1f.T  # [784 f_in, 784 j]
    for t in range(T):
        d1[:, t, :] = W1T[t * FV : t * FV + P, :]
        d1[0:68, 4 + t, :] = W1T[t * FV + P : (t + 1) * FV, :]
    d1[68, 4, :] = bd1f

    d2 = np.zeros((P, 7, FEAT), np.float32)
    W2T = np.asarray(Wd2, f64).T
    for cidx in range(6):
        d2[:, cidx, :] = W2T[cidx * P : (cidx + 1) * P, :]
    d2[0:16, 6, :] = W2T[768:784, :]
    d2[16, 6, :] = np.asarray(bd2, f64)

    ow = np.zeros((P, 7, 10), np.float32)
    WoT = np.asarray(Wo, f64).T
    for cidx in range(6):
        ow[:, cidx, :] = WoT[cidx * P : (cidx + 1) * P, :]
    ow[0:16, 6, :] = WoT[768:784, :]
    ow[16, 6, :] = np.asarray(bo, f64)
    return pw.astype(ml_dtypes.bfloat16), d1, d2, ow


_NC_CACHE = {}


def kernel(**inputs):
    x = np.ascontiguousarray(np.asarray(inputs["x"], np.float32))
    zu, d1, d2, ow = pack_weights(
        inputs["W1"], inputs["b1"], inputs["W2"], inputs["b2"], inputs["W3"],
        inputs["b3"], inputs["Wd1"], inputs["bd1"], inputs["Wd2"],
        inputs["bd2"], inputs["Wo"], inputs["bo"],
    )
    if "nc" not in _NC_CACHE:
        _NC_CACHE["nc"] = build(NSUB, BPC // (NSUB * P))
    nc = _NC_CACHE["nc"]
    bpc = B // NCORES
    in_maps = [
        {
            "x": x[c * bpc : (c + 1) * bpc],
            "zu_w": zu,
            "dec1_w": d1,
            "dec2_w": d2,
            "out_w": ow,
        }
        for c in range(NCORES)
    ]
    res = run_bass_kernel_spmd(nc, in_maps, core_ids=list(range(NCORES)))
    return np.concatenate([res.results[c]["out"] for c in range(NCORES)], axis=0)


# revision 74
# speedup vs baseline: 3.6670x; 1.0378x over previous
"""Trainium2 Bass kernel for nn_CapsuleNeuralNetworkV2 (8 cores, data-parallel).

Reference math (per sample, 8 capsule iterations then decoder):
  v = h.reshape(4, 196); q,k,u = affine(v); scores = q k^T;
  P = softmax(scores); h' = P u;  dec = relu(h Wd1^T+bd1) Wd2^T+bd2;
  out = softmax(dec Wo^T + bo).

Restructuring (host-side algebra):
  Since each P has rows summing to 1, the state stays in the span of the 4
  initial slots: v^(k) = W3^k w^(k) + m_k with w^(k) = C^(k) V (C is a
  per-sample 4x4 convex-coefficient matrix, V the initial slots).
  scores^(k)[t,s] = C[t] M_k C[s]^T (mod per-t constants that cancel in
  softmax), where M_k[i,j] = v_i.(G_k v_j) + a_k.v_j depends only on the
  INITIAL slots: G_k = (W3^k)^T G W3^k, G = W1^T W2,
  a_k = (W3^k)^T (G^T m_k + W2^T b1).  G_k is numerically low-rank for k>=1
  (powers of a random matrix), so M_k is computed from rank-r_k SVD
  projections p_i = U_r^T v_i, q_j = (S V_r^T) v_j: M[i,j] ~ p_i.q_j + r_j.
  Per iteration only the tiny 4x4 chain is sequential:
  scores = C M C^T -> softmax -> C' = P C.  All projections/M_k are
  C-independent and pipeline on PE/Act/DVE ahead of the chain.
  Final w^(8) = C^(8) V; W3^8/m_8 are folded into Wd1/bd1 on the host.

Engines: PE transposes V once per tile + small bf16 projection matmuls +
decoder; DVE/Pool share the per-sample dot products and the 4x4 chain; Act
does PSUM evacuation, exp, and decoder activations.
"""

import numpy as np
import ml_dtypes

import concourse.bass as bass
import concourse.tile as tile
from concourse import bacc, mybir
from concourse.bass import ds
from concourse.bass_utils import run_bass_kernel_spmd
from concourse.masks import make_identity

FR = mybir.dt.float32r
BF = mybir.dt.bfloat16
F32 = mybir.dt.float32
AF = mybir.ActivationFunctionType
ALU = mybir.AluOpType

B = 32768
NCORES = 8
NSUB = 8
BPC = B // NCORES
P = 128
T = 4
FV = 196
FEAT = 784
SLOT = 198  # h slot: 196 data + ones col (196) + spare (197)

RANKS = [96, 48, 32, 24, 16, 12, 8, 8]
NCOLS = [2 * (r + 1) for r in RANKS]  # proj cols per slot per k
POFF = [0]
for _n in NCOLS:
    POFF.append(POFF[-1] + _n)
PTOT = POFF[-1]
NCMAX = max(NCOLS)


def _ap(t, dims, offset_elems=0):
    """Hand-built AP over a tile's tensor: dims = [[step, count], ...]."""
    a = t[:] if hasattr(t, "tile") or not isinstance(t, bass.AP) else t
    return bass.AP(tensor=a.tensor, offset=a.offset + offset_elems, ap=dims)


def build(nsub=8, ngroups=4):
    """One NeuronCore program processing nsub*ngroups*128 samples."""
    bpc = nsub * ngroups * P
    nc = bacc.Bacc("TRN2", target_bir_lowering=False, debug=False)

    x_d = nc.dram_tensor("x", [bpc, FEAT], FR, kind="ExternalInput")
    pw_d = nc.dram_tensor("zu_w", [P, 2, PTOT], BF, kind="ExternalInput")
    d1_d = nc.dram_tensor("dec1_w", [P, 8, FEAT], FR, kind="ExternalInput")
    d2_d = nc.dram_tensor("dec2_w", [P, 7, FEAT], FR, kind="ExternalInput")
    ow_d = nc.dram_tensor("out_w", [P, 7, 10], FR, kind="ExternalInput")
    out_d = nc.dram_tensor("out", [bpc, 10], F32, kind="ExternalOutput")

    with tile.TileContext(nc) as tc:
        consts = tc.alloc_tile_pool(name="consts", bufs=1)
        hp = tc.alloc_tile_pool(name="h", bufs=1)
        vp = tc.alloc_tile_pool(name="vt", bufs=1)
        pkp = tc.alloc_tile_pool(name="pk", bufs=2)
        scp = tc.alloc_tile_pool(name="scr", bufs=4)
        mtp = tc.alloc_tile_pool(name="mt", bufs=8)
        sm = tc.alloc_tile_pool(name="small", bufs=3)
        wp = tc.alloc_tile_pool(name="w", bufs=2)
        wkd = tc.alloc_tile_pool(name="wkd", bufs=1)
        pp = tc.alloc_tile_pool(name="ps", bufs=2, space="PSUM")

        ident_f = consts.tile([P, P], F32)
        make_identity(nc, ident_f)
        ident_r = consts.tile([P, P], FR)
        nc.vector.tensor_copy(ident_r, ident_f)
        ones_c = consts.tile([P, 512], F32)
        nc.vector.memset(ones_c, 1.0)
        pw = consts.tile([P, 2, PTOT], BF)
        nc.sync.dma_start(out=pw, in_=pw_d[:, :, :])
        # decoder weights DMA'd after group 0's x tiles (emitted in build
        # below) so the first group's compute isn't starved behind 6MB
        d1_w = consts.tile([P, 8, FEAT], FR)
        d2_w = consts.tile([P, 7, FEAT], FR)
        ow_w = consts.tile([P, 7, 10], FR)

        def load_dma(g, j):
            h0 = hp.tile([P, T, SLOT], FR, tag=f"h{j}")
            nc.sync.dma_start(
                out=h0[:, :, 0:FV],
                in_=x_d[ds(g * (nsub * P) + j * P, P), :].rearrange(
                    "p (t f) -> p t f", t=T
                ),
            )
            nc.gpsimd.tensor_copy(h0[:, :, 196:198], ones_c[:, 0 : 2 * T])
            return h0

        def prep_tile(j, h0):
            vt1 = vp.tile([P, T, P], BF, tag=f"vt1{j}")
            vt2 = vp.tile([69, T, P], BF, tag=f"vt2{j}")
            t1_ps = pp.tile([P, T, P], FR, tag="t1ps", bufs=1)
            t2_ps = pp.tile([69, T, P], FR, tag="t2ps", bufs=1)
            for t in range(T):
                nc.tensor.transpose(t1_ps[:, t, :], h0[:, t, 0:P], ident_r)
                nc.tensor.transpose(t2_ps[:, t, :], h0[:, t, P : P + 69], ident_r)
            nc.scalar.copy(vt1, t1_ps)
            nc.scalar.copy(vt2, t2_ps)
            return vt1, vt2

        def load_tile(g, j):
            h0 = load_dma(g, j)
            return h0, None, None, None

        def proj(j, k, vt1, vt2):
            """PE projections for iteration k -> pk [128, 4, nc] bf16."""
            nco = NCOLS[k]
            off = POFF[k]
            pk = pkp.tile([P, T, NCMAX], BF, tag=f"pk{j}")
            if k == 0:
                for half in range(2):
                    ps = pp.tile([P, 2, NCMAX], F32, tag="pkps", bufs=2)
                    for sl in range(2):
                        s = half * 2 + sl
                        nc.tensor.matmul(
                            ps[:, sl, 0:nco], vt1[:, s, :],
                            pw[:, 0, off : off + nco], start=True, stop=False)
                        nc.tensor.matmul(
                            ps[:, sl, 0:nco], vt2[0:69, s, :],
                            pw[0:69, 1, off : off + nco], start=False, stop=True)
                    nc.scalar.copy(
                        pk[:, 2 * half : 2 * half + 2, 0:nco], ps[:, :, 0:nco])
            else:
                ps = pp.tile([P, T, 98], F32, tag="pkps1", bufs=2)
                for s in range(T):
                    nc.tensor.matmul(
                        ps[:, s, 0:nco], vt1[:, s, :],
                        pw[:, 0, off : off + nco], start=True, stop=False)
                    nc.tensor.matmul(
                        ps[:, s, 0:nco], vt2[0:69, s, :],
                        pw[0:69, 1, off : off + nco], start=False, stop=True)
                nc.scalar.copy(pk[:, :, 0:nco], ps[:, :, 0:nco])
            return pk

        def dots(j, k, mtc, pk):
            """M_k[i,j] for all 16 slot pairs -> wave-mtc rows 4(j%2)+i."""
            r1 = RANKS[k] + 1
            pap = pk[:].ap[0]
            jw = j % 2
            if k == 0:
                # big r: fused stt (mult + f32 accumulate in one 1x pass)
                scr = scp.tile([P, 256], BF, tag="scr197", bufs=6)
                for idx in range(16):
                    i, jj = idx // 4, idx % 4
                    in0 = _ap(pk, [pap, [1, r1]],
                              offset_elems=i * NCMAX + r1)
                    in1 = _ap(pk, [pap, [1, r1]], offset_elems=jj * NCMAX)
                    nc.vector.scalar_tensor_tensor(
                        out=scr[:, 0:r1], in0=in0, scalar=1.0, in1=in1,
                        op0=ALU.mult, op1=ALU.mult,
                        accum_out=mtc[:, 4 * jw + i, jj : jj + 1])
            else:
                # small r: one bf16 2x tensor_tensor + one inner-axis reduce
                scr = scp.tile([P, T, T, 65], BF, tag="scr", bufs=3)
                in0 = _ap(pk, [pap, [NCMAX, 4], [0, 4], [1, r1]],
                          offset_elems=r1)
                in1 = _ap(pk, [pap, [0, 4], [NCMAX, 4], [1, r1]])
                nc.vector.tensor_tensor(
                    out=scr[:, :, :, 0:r1], in0=in0, in1=in1, op=ALU.mult)
                nc.vector.tensor_reduce(
                    out=mtc[:, 4 * jw : 4 * jw + 4, :], in_=scr[:, :, :, 0:r1],
                    axis=mybir.AxisListType.X, op=ALU.add)

        def serial_phase(k, w, mtc, c_prev):
            """Per-k 4x4 chain for one WAVE (2 tiles) in wide DVE ops over a
            [128, (j,t), s] layout (j in the wave): scores = C mt C^T ->
            e = exp -> C'u = e C -> C' = C'u / rowsum. Returns new C tile."""
            JT = 8   # (2 tiles) x (4 slots)
            JR = 32  # replicated size per tile pair
            if k == 0:
                s_t = mtc
            else:
                cap = c_prev[:].ap[0]
                # replicate C 4x -> crep[j, rep, s, jj] so every TT operand
                # stays within the ISA's 3-free-dim AP limit
                crep = sm.tile([P, 4 * JR], F32, tag=f"crep{w}", bufs=2)
                nc.gpsimd.tensor_copy(
                    _ap(crep, [crep[:].ap[0], [64, 2], [16, 4], [1, 16]]),
                    _ap(c_prev, [cap, [16, 2], [0, 4], [1, 16]]))
                tt_eng = nc.vector if k >= 4 else nc.gpsimd
                scrd = scp.tile([P, JT, T, T], F32, tag="scrd", bufs=6)
                tt_eng.tensor_tensor(  # D[j,i,s] = sum_jj mt[j,i,jj] C[j,s,jj]
                    out=scrd,
                    in0=_ap(mtc, [mtc[:].ap[0], [4, JT], [0, 4], [1, 4]]),
                    in1=crep[:],
                    op=ALU.mult)
                dm = sm.tile([P, JT, T], F32, tag=f"dm{w}")
                nc.vector.tensor_reduce(
                    out=dm, in_=scrd, axis=mybir.AxisListType.X, op=ALU.add)
                drep = sm.tile([P, 4 * JR], F32, tag=f"drep{w}", bufs=2)
                nc.gpsimd.tensor_copy(
                    _ap(drep, [drep[:].ap[0], [64, 2], [16, 4], [1, 16]]),
                    _ap(dm, [dm[:].ap[0], [16, 2], [0, 4], [1, 16]]))
                scrd2 = scp.tile([P, JT, T, T], F32, tag="scrd", bufs=6)
                tt_eng.tensor_tensor(  # S[j,t,s] = sum_i C[j,t,i] D[j,i,s]
                    out=scrd2,
                    in0=_ap(c_prev, [cap, [4, JT], [0, 4], [1, 4]]),
                    in1=_ap(drep, [drep[:].ap[0], [16, JT], [1, 4], [4, 4]]),
                    op=ALU.mult)
                s_t = sm.tile([P, JT, T], F32, tag=f"st{w}")
                nc.vector.tensor_reduce(
                    out=s_t, in_=scrd2, axis=mybir.AxisListType.X, op=ALU.add)
            e = sm.tile([P, JT, T], F32, tag=f"e{w}")
            nc.scalar.activation(e, s_t, AF.Exp)
            sums = sm.tile([P, JT], F32, tag=f"su{w}")
            nc.vector.reduce_sum(sums, e, axis=mybir.AxisListType.X)
            rec = sm.tile([P, JT], F32, tag=f"re{w}")
            nc.vector.reciprocal(rec, sums)
            if k == 0:
                cnum = e
            else:
                scrd3 = scp.tile([P, JT, T, T], F32, tag="scrd", bufs=6)
                tt_eng.tensor_tensor(  # C'u[j,t,jj] = sum_s e[j,t,s] C[j,s,jj]
                    out=scrd3,
                    in0=_ap(e, [e[:].ap[0], [4, JT], [0, 4], [1, 4]]),
                    in1=_ap(crep, [crep[:].ap[0], [16, JT], [1, 4], [4, 4]]),
                    op=ALU.mult)
                cnum = sm.tile([P, JT, T], F32, tag=f"cu{w}")
                nc.vector.tensor_reduce(
                    out=cnum, in_=scrd3, axis=mybir.AxisListType.X, op=ALU.add)
            c_new = sm.tile([P, JT, T], F32, tag=f"call{w}", bufs=3)
            nc.vector.scalar_tensor_tensor(
                out=c_new, in0=cnum, scalar=1.0,
                in1=_ap(rec, [rec[:].ap[0], [1, JT], [0, T]]),
                op0=ALU.mult, op1=ALU.mult)
            return c_new

        def recon(j, h0, c8):
            """w[:, t, :] = sum_s C8[t,s] * h0[:, s, :] (ones col rides along)."""
            w = wp.tile([P, T, SLOT], FR, tag=f"w{j % 4}", bufs=1)
            jw = j % 2
            for t in range(T):
                nc.scalar.activation(
                    w[:, t, :], h0[:, 0, :], AF.Copy,
                    scale=c8[:, 4 * jw + t, 0:1])
            for t in range(3):
                for s in range(1, T):
                    nc.vector.scalar_tensor_tensor(
                        out=w[:, t, :], in0=h0[:, s, :],
                        scalar=c8[:, 4 * jw + t, s : s + 1], in1=w[:, t, :],
                        op0=ALU.mult, op1=ALU.add)
            for t in range(3, T):
                pct = wp.tile([P, SLOT], F32, tag="pct", bufs=2)
                for s in range(1, T):
                    nc.gpsimd.tensor_scalar_mul(
                        pct, h0[:, s, :], c8[:, 4 * jw + t, s : s + 1])
                    nc.gpsimd.tensor_add(w[:, t, :], w[:, t, :], pct)
            return w

        def decoder(ws2, g, wave):
            """Decoder over one wave of 2 tiles (N = 256 wide matmuls)."""
            W = 2 * P
            ht1 = wkd.tile([P, T, W], FR, tag=f"ht1w{wave % 2}")
            ht2 = wkd.tile([69, T, W], FR, tag=f"ht2w{wave % 2}")
            for t in range(T):
                t1_ps = pp.tile([P, T, P], FR, tag="t1ps", bufs=1)
                t2_ps = pp.tile([69, T, P], FR, tag="t2ps", bufs=1)
                for j in range(2):
                    nc.tensor.transpose(
                        t1_ps[:, j, :], ws2[j][:, t, 0:P], ident_r
                    )
                    nc.tensor.transpose(
                        t2_ps[:, j, :], ws2[j][:, t, P : P + 69], ident_r
                    )
                nc.scalar.copy(ht1[:, t, :], t1_ps[:, 0:2, :])
                nc.vector.tensor_copy(ht2[:, t, :], t2_ps[:, 0:2, :])

            # dec1 = relu(Wd1~ @ w.T + bd1~), feature-major, 7 M-chunks
            d1a = wkd.tile([P, 6, W], FR, tag=f"d1aw{wave % 2}")
            d1b = wkd.tile([17, W], FR, tag=f"d1bw{wave % 2}")
            nc.vector.tensor_copy(d1b, ones_c[0:17, 0:W])
            for m in range(7):
                mw = min(P, FEAT - m * P)
                mp = pp.tile([P, W], F32, tag="mp")
                msl = slice(m * P, m * P + mw)
                for t in range(T):
                    nc.tensor.matmul(mp[0:mw, :], d1_w[:, t, msl], ht1[:, t, :],
                                     start=(t == 0), stop=False)
                for t in range(T):
                    nc.tensor.matmul(mp[0:mw, :], d1_w[0:69, 4 + t, msl],
                                     ht2[:, t, :], start=False, stop=(t == 3))
                if m < 6:
                    nc.scalar.activation(d1a[:, m, :], mp, AF.Relu)
                else:
                    nc.scalar.activation(d1b[0:16, :], mp[0:16, :], AF.Relu)

            # dec2 = Wd2 @ relu1 + bd2, feature-major
            d2a = wkd.tile([P, 6, W], FR, tag=f"d2aw{wave % 2}")
            d2b = wkd.tile([17, W], FR, tag=f"d2bw{wave % 2}")
            nc.vector.tensor_copy(d2b, ones_c[0:17, 0:W])
            for m in range(7):
                mw = min(P, FEAT - m * P)
                mp = pp.tile([P, W], F32, tag="mp")
                msl = slice(m * P, m * P + mw)
                for c in range(6):
                    nc.tensor.matmul(mp[0:mw, :], d2_w[:, c, msl], d1a[:, c, :],
                                     start=(c == 0), stop=False)
                nc.tensor.matmul(mp[0:mw, :], d2_w[0:17, 6, msl], d1b,
                                 start=False, stop=True)
                if m < 6:
                    nc.scalar.copy(d2a[:, m, :], mp)
                else:
                    nc.scalar.copy(d2b[0:16, :], mp[0:16, :])

            # logits + softmax per subtile
            for j in range(2):
                jsl = slice(j * P, (j + 1) * P)
                lgt = pp.tile([P, W], F32, tag="mp")
                lg = lgt[:, 0:10]
                for c in range(6):
                    nc.tensor.matmul(lg, d2a[:, c, jsl], ow_w[:, c, :],
                                     start=(c == 0), stop=False)
                nc.tensor.matmul(lg, d2b[:, jsl], ow_w[0:17, 6, :],
                                 start=False, stop=True)
                e10 = sm.tile([P, 10], F32, tag="e10")
                s10 = sm.tile([P, 1], F32, tag="s10")
                nc.scalar.activation(e10, lg, AF.Exp, accum_out=s10)
                r10 = sm.tile([P, 1], F32, tag="r10")
                nc.vector.reciprocal(r10, s10)
                o10 = sm.tile([P, 10], F32, tag="o10")
                nc.vector.tensor_scalar_mul(o10, e10, r10)
                nc.sync.dma_start(
                    out=out_d[ds(g * (nsub * P) + (2 * wave + j) * P, P), :],
                    in_=o10
                )

        def body(g, preloaded=None):
            nw = nsub // 2
            h0s, cs = [], [None] * nw
            vts = []
            mtk = {}  # (wave, k) -> Mt tile
            # k=0 proj+dots interleaved per tile so the first tile's chain
            # races ahead of later tiles' loads
            for j in range(nsub):
                w = j // 2
                if j % 2 == 0:
                    mtk[(w, 0)] = mtp.tile([P, 8, T], F32, tag=f"mtk{w}",
                                           bufs=3, name=f"mt0w{w}")
                h0 = load_dma(g, j) if preloaded is None else preloaded[j]
                h0s.append(h0)
                vt1, vt2 = prep_tile(j, h0)
                vts.append((vt1, vt2))
                pk = proj(j, 0, vt1, vt2)
                dots(j, 0, mtk[(w, 0)], pk)
            # waves are staggered one k apart: early waves finish their
            # chains (and start decoding) while late waves still compute
            LAG = 2
            for step in range(1, 8 + LAG * (nw - 1) + 1):
                for w in range(nw):
                    k = step - LAG * w
                    if 1 <= k <= 7:
                        mtk[(w, k)] = mtp.tile([P, 8, T], F32, tag=f"mtk{w}",
                                               bufs=3, name=f"mt{k}w{w}")
                        for j in (2 * w, 2 * w + 1):
                            pk = proj(j, k, *vts[j])
                            dots(j, k, mtk[(w, k)], pk)
                        cs[w] = serial_phase(k - 1, w, mtk[(w, k - 1)], cs[w])
                    elif k == 8:
                        cs[w] = serial_phase(7, w, mtk[(w, 7)], cs[w])
                        wsp = [recon(2 * w, h0s[2 * w], cs[w]),
                               recon(2 * w + 1, h0s[2 * w + 1], cs[w])]
                        decoder(wsp, g, w)

        # group 0's x DMAs first so they precede the 6MB of decoder weights
        # on the sync queue; weights stream in during attention
        pre0 = [load_dma(0, j) for j in range(nsub)]
        nc.sync.dma_start(out=d1_w, in_=d1_d[:, :, :])
        nc.sync.dma_start(out=d2_w, in_=d2_d[:, :, :])
        nc.sync.dma_start(out=ow_w, in_=ow_d[:, :, :])
        body(0, preloaded=pre0)
        if ngroups > 1:
            with tc.For_i(1, ngroups, 1) as g:
                body(g)
        for _pool in (pp, wkd, wp, sm, mtp, scp, pkp, vp, hp, consts):
            _pool.release()

    nc.compile()
    return nc


def pack_weights(W1, b1, W2, b2, W3, b3, Wd1, bd1, Wd2, bd2, Wo, bo):
    f64 = np.float64
    W1, b1, W2, b2, W3, b3 = (np.asarray(t, f64) for t in (W1, b1, W2, b2, W3, b3))
    G = W1.T @ W2
    a = W2.T @ b1

    A = np.eye(FV)
    m = np.zeros(FV)
    pw = np.zeros((P, 2, PTOT), np.float32)
    for k in range(8):
        Gk = A.T @ G @ A
        ak = A.T @ (G.T @ m + a)
        nco = NCOLS[k]
        Wk = np.zeros((197, nco), f64)
        r = RANKS[k]
        r1 = r + 1
        U, S, Vh = np.linalg.svd(Gk)
        Wk[:FV, :r] = (np.diag(S[:r]) @ Vh[:r]).T
        Wk[:FV, r] = ak
        Wk[:FV, r1 : r1 + r] = U[:, :r]
        Wk[FV, r1 + r] = 1.0
        off = POFF[k]
        pw[:, 0, off : off + nco] = Wk[0:128]
        pw[0:69, 1, off : off + nco] = Wk[128:197]
        A = W3 @ A
        m = W3 @ m + b3
    A8, m8 = A, m

    # fold W3^8 / m8 into the first decoder layer
    BD = np.zeros((FEAT, FEAT), f64)
    mm = np.zeros(FEAT, f64)
    for t in range(T):
        BD[t * FV : (t + 1) * FV, t * FV : (t + 1) * FV] = A8
        mm[t * FV : (t + 1) * FV] = m8
    Wd1f = np.asarray(Wd1, f64) @ BD
    bd1f = np.asarray(bd1, f64) + np.asarray(Wd1, f64) @ mm

    d1 = np.zeros((P, 8, FEAT), np.float32)
    W1T = Wd1f.T  # [784 f_in, 784 j]
    for t in range(T):
        d1[:, t, :] = W1T[t * FV : t * FV + P, :]
        d1[0:68, 4 + t, :] = W1T[t * FV + P : (t + 1) * FV, :]
    d1[68, 4, :] = bd1f

    d2 = np.zeros((P, 7, FEAT), np.float32)
    W2T = np.asarray(Wd2, f64).T
    for cidx in range(6):
        d2[:, cidx, :] = W2T[cidx * P : (cidx + 1) * P, :]
    d2[0:16, 6, :] = W2T[768:784, :]
    d2[16, 6, :] = np.asarray(bd2, f64)

    ow = np.zeros((P, 7, 10), np.float32)
    WoT = np.asarray(Wo, f64).T
    for cidx in range(6):
        ow[:, cidx, :] = WoT[cidx * P : (cidx + 1) * P, :]
    ow[0:16, 6, :] = WoT[768:784, :]
    ow[16, 6, :] = np.asarray(bo, f64)
    return pw.astype(ml_dtypes.bfloat16), d1, d2, ow


_NC_CACHE = {}


def kernel(**inputs):
    x = np.ascontiguousarray(np.asarray(inputs["x"], np.float32))
    zu, d1, d2, ow = pack_weights(
        inputs["W1"], inputs["b1"], inputs["W2"], inputs["b2"], inputs["W3"],
        inputs["b3"], inputs["Wd1"], inputs["bd1"], inputs["Wd2"],
        inputs["bd2"], inputs["Wo"], inputs["bo"],
    )
    if "nc" not in _NC_CACHE:
        _NC_CACHE["nc"] = build(NSUB, BPC // (NSUB * P))
    nc = _NC_CACHE["nc"]
    bpc = B // NCORES
    in_maps = [
        {
            "x": x[c * bpc : (c + 1) * bpc],
            "zu_w": zu,
            "dec1_w": d1,
            "dec2_w": d2,
            "out_w": ow,
        }
        for c in range(NCORES)
    ]
    res = run_bass_kernel_spmd(nc, in_maps, core_ids=list(range(NCORES)))
    return np.concatenate([res.results[c]["out"] for c in range(NCORES)], axis=0)


# revision 78
# speedup vs baseline: 3.7029x; 1.0098x over previous
"""Trainium2 Bass kernel for nn_CapsuleNeuralNetworkV2 (8 cores, data-parallel).

Reference math (per sample, 8 capsule iterations then decoder):
  v = h.reshape(4, 196); q,k,u = affine(v); scores = q k^T;
  P = softmax(scores); h' = P u;  dec = relu(h Wd1^T+bd1) Wd2^T+bd2;
  out = softmax(dec Wo^T + bo).

Restructuring (host-side algebra):
  Since each P has rows summing to 1, the state stays in the span of the 4
  initial slots: v^(k) = W3^k w^(k) + m_k with w^(k) = C^(k) V (C is a
  per-sample 4x4 convex-coefficient matrix, V the initial slots).
  scores^(k)[t,s] = C[t] M_k C[s]^T (mod per-t constants that cancel in
  softmax), where M_k[i,j] = v_i.(G_k v_j) + a_k.v_j depends only on the
  INITIAL slots: G_k = (W3^k)^T G W3^k, G = W1^T W2,
  a_k = (W3^k)^T (G^T m_k + W2^T b1).  G_k is numerically low-rank for k>=1
  (powers of a random matrix), so M_k is computed from rank-r_k SVD
  projections p_i = U_r^T v_i, q_j = (S V_r^T) v_j: M[i,j] ~ p_i.q_j + r_j.
  Per iteration only the tiny 4x4 chain is sequential:
  scores = C M C^T -> softmax -> C' = P C.  All projections/M_k are
  C-independent and pipeline on PE/Act/DVE ahead of the chain.
  Final w^(8) = C^(8) V; W3^8/m_8 are folded into Wd1/bd1 on the host.

Schedule: 8 tiles of 128 samples per hardware-loop group (4 groups/core),
paired into 4 "waves" whose 4x4 chains are staggered 3 iterations apart so
early waves' recon+decoder (N=256 fp32r matmuls) overlap late waves'
chains.  PE: one set of transposes per tile + small bf16 projection
matmuls + decoder.  DVE: per-sample dot products (stt-accum at k=0, bf16
2x tensor_tensor + inner-axis reduce for k>=1) and the wide per-wave chain
ops (with 4x-replicated C/D copies to stay within 3-free-dim APs).  Pool:
chain tensor_tensors for early k, replicate-copies, recon t=3.  Act: PSUM
evacuation, exp, recon seeds, decoder activations.  Group 0's x DMAs are
emitted before the 6MB of decoder weights on the sync queue so compute
starts immediately; weights stream in during attention.
"""

import numpy as np
import ml_dtypes

import concourse.bass as bass
import concourse.tile as tile
from concourse import bacc, mybir
from concourse.bass import ds
from concourse.bass_utils import run_bass_kernel_spmd
from concourse.masks import make_identity

FR = mybir.dt.float32r
BF = mybir.dt.bfloat16
F32 = mybir.dt.float32
AF = mybir.ActivationFunctionType
ALU = mybir.AluOpType

B = 32768
NCORES = 8
NSUB = 8
BPC = B // NCORES
P = 128
T = 4
FV = 196
FEAT = 784
SLOT = 198  # h slot: 196 data + ones col (196) + spare (197)

RANKS = [96, 48, 32, 24, 16, 12, 8, 8]
NCOLS = [2 * (r + 1) for r in RANKS]  # proj cols per slot per k
POFF = [0]
for _n in NCOLS:
    POFF.append(POFF[-1] + _n)
PTOT = POFF[-1]
NCMAX = max(NCOLS)


def _ap(t, dims, offset_elems=0):
    """Hand-built AP over a tile's tensor: dims = [[step, count], ...]."""
    a = t[:] if hasattr(t, "tile") or not isinstance(t, bass.AP) else t
    return bass.AP(tensor=a.tensor, offset=a.offset + offset_elems, ap=dims)


def build(nsub=8, ngroups=4):
    """One NeuronCore program processing nsub*ngroups*128 samples."""
    bpc = nsub * ngroups * P
    nc = bacc.Bacc("TRN2", target_bir_lowering=False, debug=False)

    x_d = nc.dram_tensor("x", [bpc, FEAT], FR, kind="ExternalInput")
    pw_d = nc.dram_tensor("zu_w", [P, 2, PTOT], BF, kind="ExternalInput")
    d1_d = nc.dram_tensor("dec1_w", [P, 8, FEAT], FR, kind="ExternalInput")
    d2_d = nc.dram_tensor("dec2_w", [P, 7, FEAT], FR, kind="ExternalInput")
    ow_d = nc.dram_tensor("out_w", [P, 7, 10], FR, kind="ExternalInput")
    out_d = nc.dram_tensor("out", [bpc, 10], F32, kind="ExternalOutput")

    with tile.TileContext(nc) as tc:
        consts = tc.alloc_tile_pool(name="consts", bufs=1)
        hp = tc.alloc_tile_pool(name="h", bufs=1)
        vp = tc.alloc_tile_pool(name="vt", bufs=1)
        pkp = tc.alloc_tile_pool(name="pk", bufs=2)
        scp = tc.alloc_tile_pool(name="scr", bufs=4)
        mtp = tc.alloc_tile_pool(name="mt", bufs=8)
        sm = tc.alloc_tile_pool(name="small", bufs=3)
        wp = tc.alloc_tile_pool(name="w", bufs=2)
        wkd = tc.alloc_tile_pool(name="wkd", bufs=1)
        pp = tc.alloc_tile_pool(name="ps", bufs=2, space="PSUM")

        ident_f = consts.tile([P, P], F32)
        make_identity(nc, ident_f)
        ident_r = consts.tile([P, P], FR)
        nc.vector.tensor_copy(ident_r, ident_f)
        ones_c = consts.tile([P, 512], F32)
        nc.vector.memset(ones_c, 1.0)
        pw = consts.tile([P, 2, PTOT], BF)
        nc.sync.dma_start(out=pw, in_=pw_d[:, :, :])
        # decoder weights DMA'd after group 0's x tiles (emitted in build
        # below) so the first group's compute isn't starved behind 6MB
        d1_w = consts.tile([P, 8, FEAT], FR)
        d2_w = consts.tile([P, 7, FEAT], FR)
        ow_w = consts.tile([P, 7, 10], FR)

        def load_dma(g, j):
            h0 = hp.tile([P, T, SLOT], FR, tag=f"h{j}")
            nc.sync.dma_start(
                out=h0[:, :, 0:FV],
                in_=x_d[ds(g * (nsub * P) + j * P, P), :].rearrange(
                    "p (t f) -> p t f", t=T
                ),
            )
            nc.gpsimd.tensor_copy(h0[:, :, 196:198], ones_c[:, 0 : 2 * T])
            return h0

        def prep_tile(j, h0):
            vt1 = vp.tile([P, T, P], BF, tag=f"vt1{j}")
            vt2 = vp.tile([69, T, P], BF, tag=f"vt2{j}")
            t1_ps = pp.tile([P, T, P], FR, tag="t1ps", bufs=1)
            t2_ps = pp.tile([69, T, P], FR, tag="t2ps", bufs=1)
            for t in range(T):
                nc.tensor.transpose(t1_ps[:, t, :], h0[:, t, 0:P], ident_r)
                nc.tensor.transpose(t2_ps[:, t, :], h0[:, t, P : P + 69], ident_r)
            nc.scalar.copy(vt1, t1_ps)
            nc.scalar.copy(vt2, t2_ps)
            return vt1, vt2

        def load_tile(g, j):
            h0 = load_dma(g, j)
            return h0, None, None, None

        def proj(j, k, vt1, vt2):
            """PE projections for iteration k -> pk [128, 4, nc] bf16."""
            nco = NCOLS[k]
            off = POFF[k]
            pk = pkp.tile([P, T, NCMAX], BF, tag=f"pk{j}")
            if k == 0:
                for half in range(2):
                    ps = pp.tile([P, 2, NCMAX], F32, tag="pkps", bufs=2)
                    for sl in range(2):
                        s = half * 2 + sl
                        nc.tensor.matmul(
                            ps[:, sl, 0:nco], vt1[:, s, :],
                            pw[:, 0, off : off + nco], start=True, stop=False)
                        nc.tensor.matmul(
                            ps[:, sl, 0:nco], vt2[0:69, s, :],
                            pw[0:69, 1, off : off + nco], start=False, stop=True)
                    nc.scalar.copy(
                        pk[:, 2 * half : 2 * half + 2, 0:nco], ps[:, :, 0:nco])
            else:
                ps = pp.tile([P, T, 98], F32, tag="pkps1", bufs=2)
                for s in range(T):
                    nc.tensor.matmul(
                        ps[:, s, 0:nco], vt1[:, s, :],
                        pw[:, 0, off : off + nco], start=True, stop=False)
                    nc.tensor.matmul(
                        ps[:, s, 0:nco], vt2[0:69, s, :],
                        pw[0:69, 1, off : off + nco], start=False, stop=True)
                nc.scalar.copy(pk[:, :, 0:nco], ps[:, :, 0:nco])
            return pk

        def dots(j, k, mtc, pk):
            """M_k[i,j] for all 16 slot pairs -> wave-mtc rows 4(j%2)+i."""
            r1 = RANKS[k] + 1
            pap = pk[:].ap[0]
            jw = j % 2
            if k == 0:
                # big r: fused stt (mult + f32 accumulate in one 1x pass)
                scr = scp.tile([P, 256], BF, tag="scr197", bufs=6)
                for idx in range(16):
                    i, jj = idx // 4, idx % 4
                    in0 = _ap(pk, [pap, [1, r1]],
                              offset_elems=i * NCMAX + r1)
                    in1 = _ap(pk, [pap, [1, r1]], offset_elems=jj * NCMAX)
                    nc.vector.scalar_tensor_tensor(
                        out=scr[:, 0:r1], in0=in0, scalar=1.0, in1=in1,
                        op0=ALU.mult, op1=ALU.mult,
                        accum_out=mtc[:, 4 * jw + i, jj : jj + 1])
            else:
                # small r: one bf16 2x tensor_tensor + one inner-axis reduce
                scr = scp.tile([P, T, T, 65], BF, tag="scr", bufs=3)
                in0 = _ap(pk, [pap, [NCMAX, 4], [0, 4], [1, r1]],
                          offset_elems=r1)
                in1 = _ap(pk, [pap, [0, 4], [NCMAX, 4], [1, r1]])
                nc.vector.tensor_tensor(
                    out=scr[:, :, :, 0:r1], in0=in0, in1=in1, op=ALU.mult)
                nc.vector.tensor_reduce(
                    out=mtc[:, 4 * jw : 4 * jw + 4, :], in_=scr[:, :, :, 0:r1],
                    axis=mybir.AxisListType.X, op=ALU.add)

        def serial_phase(k, w, mtc, c_prev):
            """Per-k 4x4 chain for one WAVE (2 tiles) in wide DVE ops over a
            [128, (j,t), s] layout (j in the wave): scores = C mt C^T ->
            e = exp -> C'u = e C -> C' = C'u / rowsum. Returns new C tile."""
            JT = 8   # (2 tiles) x (4 slots)
            JR = 32  # replicated size per tile pair
            if k == 0:
                s_t = mtc
            else:
                cap = c_prev[:].ap[0]
                # replicate C 4x -> crep[j, rep, s, jj] so every TT operand
                # stays within the ISA's 3-free-dim AP limit
                crep = sm.tile([P, 4 * JR], F32, tag=f"crep{w}", bufs=2)
                nc.gpsimd.tensor_copy(
                    _ap(crep, [crep[:].ap[0], [64, 2], [16, 4], [1, 16]]),
                    _ap(c_prev, [cap, [16, 2], [0, 4], [1, 16]]))
                tt_eng = nc.vector if k >= 4 else nc.gpsimd
                scrd = scp.tile([P, JT, T, T], F32, tag="scrd", bufs=6)
                tt_eng.tensor_tensor(  # D[j,i,s] = sum_jj mt[j,i,jj] C[j,s,jj]
                    out=scrd,
                    in0=_ap(mtc, [mtc[:].ap[0], [4, JT], [0, 4], [1, 4]]),
                    in1=crep[:],
                    op=ALU.mult)
                dm = sm.tile([P, JT, T], F32, tag=f"dm{w}")
                nc.vector.tensor_reduce(
                    out=dm, in_=scrd, axis=mybir.AxisListType.X, op=ALU.add)
                drep = sm.tile([P, 4 * JR], F32, tag=f"drep{w}", bufs=2)
                nc.gpsimd.tensor_copy(
                    _ap(drep, [drep[:].ap[0], [64, 2], [16, 4], [1, 16]]),
                    _ap(dm, [dm[:].ap[0], [16, 2], [0, 4], [1, 16]]))
                scrd2 = scp.tile([P, JT, T, T], F32, tag="scrd", bufs=6)
                tt_eng.tensor_tensor(  # S[j,t,s] = sum_i C[j,t,i] D[j,i,s]
                    out=scrd2,
                    in0=_ap(c_prev, [cap, [4, JT], [0, 4], [1, 4]]),
                    in1=_ap(drep, [drep[:].ap[0], [16, JT], [1, 4], [4, 4]]),
                    op=ALU.mult)
                s_t = sm.tile([P, JT, T], F32, tag=f"st{w}")
                nc.vector.tensor_reduce(
                    out=s_t, in_=scrd2, axis=mybir.AxisListType.X, op=ALU.add)
            e = sm.tile([P, JT, T], F32, tag=f"e{w}")
            nc.scalar.activation(e, s_t, AF.Exp)
            sums = sm.tile([P, JT], F32, tag=f"su{w}")
            nc.vector.reduce_sum(sums, e, axis=mybir.AxisListType.X)
            rec = sm.tile([P, JT], F32, tag=f"re{w}")
            nc.vector.reciprocal(rec, sums)
            if k == 0:
                cnum = e
            else:
                scrd3 = scp.tile([P, JT, T, T], F32, tag="scrd", bufs=6)
                tt_eng.tensor_tensor(  # C'u[j,t,jj] = sum_s e[j,t,s] C[j,s,jj]
                    out=scrd3,
                    in0=_ap(e, [e[:].ap[0], [4, JT], [0, 4], [1, 4]]),
                    in1=_ap(crep, [crep[:].ap[0], [16, JT], [1, 4], [4, 4]]),
                    op=ALU.mult)
                cnum = sm.tile([P, JT, T], F32, tag=f"cu{w}")
                nc.vector.tensor_reduce(
                    out=cnum, in_=scrd3, axis=mybir.AxisListType.X, op=ALU.add)
            c_new = sm.tile([P, JT, T], F32, tag=f"call{w}", bufs=3)
            nc.vector.scalar_tensor_tensor(
                out=c_new, in0=cnum, scalar=1.0,
                in1=_ap(rec, [rec[:].ap[0], [1, JT], [0, T]]),
                op0=ALU.mult, op1=ALU.mult)
            return c_new

        def recon(j, h0, c8):
            """w[:, t, :] = sum_s C8[t,s] * h0[:, s, :] (ones col rides along)."""
            w = wp.tile([P, T, SLOT], FR, tag=f"w{j % 4}", bufs=1)
            jw = j % 2
            for t in range(T):
                nc.scalar.activation(
                    w[:, t, :], h0[:, 0, :], AF.Copy,
                    scale=c8[:, 4 * jw + t, 0:1])
            for t in range(3):
                for s in range(1, T):
                    nc.vector.scalar_tensor_tensor(
                        out=w[:, t, :], in0=h0[:, s, :],
                        scalar=c8[:, 4 * jw + t, s : s + 1], in1=w[:, t, :],
                        op0=ALU.mult, op1=ALU.add)
            for t in range(3, T):
                pct = wp.tile([P, SLOT], F32, tag="pct", bufs=2)
                for s in range(1, T):
                    nc.gpsimd.tensor_scalar_mul(
                        pct, h0[:, s, :], c8[:, 4 * jw + t, s : s + 1])
                    nc.gpsimd.tensor_add(w[:, t, :], w[:, t, :], pct)
            return w

        def decoder(ws2, g, wave):
            """Decoder over one wave of 2 tiles (N = 256 wide matmuls)."""
            W = 2 * P
            ht1 = wkd.tile([P, T, W], FR, tag=f"ht1w{wave % 2}")
            ht2 = wkd.tile([69, T, W], FR, tag=f"ht2w{wave % 2}")
            for t in range(T):
                t1_ps = pp.tile([P, T, P], FR, tag="t1ps", bufs=1)
                t2_ps = pp.tile([69, T, P], FR, tag="t2ps", bufs=1)
                for j in range(2):
                    nc.tensor.transpose(
                        t1_ps[:, j, :], ws2[j][:, t, 0:P], ident_r
                    )
                    nc.tensor.transpose(
                        t2_ps[:, j, :], ws2[j][:, t, P : P + 69], ident_r
                    )
                nc.scalar.copy(ht1[:, t, :], t1_ps[:, 0:2, :])
                nc.vector.tensor_copy(ht2[:, t, :], t2_ps[:, 0:2, :])

            # dec1 = relu(Wd1~ @ w.T + bd1~), feature-major, 7 M-chunks
            d1a = wkd.tile([P, 6, W], FR, tag=f"d1aw{wave % 2}")
            d1b = wkd.tile([17, W], FR, tag=f"d1bw{wave % 2}")
            nc.vector.tensor_copy(d1b, ones_c[0:17, 0:W])
            for m in range(7):
                mw = min(P, FEAT - m * P)
                mp = pp.tile([P, W], F32, tag="mp")
                msl = slice(m * P, m * P + mw)
                for t in range(T):
                    nc.tensor.matmul(mp[0:mw, :], d1_w[:, t, msl], ht1[:, t, :],
                                     start=(t == 0), stop=False)
                for t in range(T):
                    nc.tensor.matmul(mp[0:mw, :], d1_w[0:69, 4 + t, msl],
                                     ht2[:, t, :], start=False, stop=(t == 3))
                if m < 6:
                    nc.scalar.activation(d1a[:, m, :], mp, AF.Relu)
                else:
                    nc.scalar.activation(d1b[0:16, :], mp[0:16, :], AF.Relu)

            # dec2 = Wd2 @ relu1 + bd2, feature-major
            d2a = wkd.tile([P, 6, W], FR, tag=f"d2aw{wave % 2}")
            d2b = wkd.tile([17, W], FR, tag=f"d2bw{wave % 2}")
            nc.vector.tensor_copy(d2b, ones_c[0:17, 0:W])
            for m in range(7):
                mw = min(P, FEAT - m * P)
                mp = pp.tile([P, W], F32, tag="mp")
                msl = slice(m * P, m * P + mw)
                for c in range(6):
                    nc.tensor.matmul(mp[0:mw, :], d2_w[:, c, msl], d1a[:, c, :],
                                     start=(c == 0), stop=False)
                nc.tensor.matmul(mp[0:mw, :], d2_w[0:17, 6, msl], d1b,
                                 start=False, stop=True)
                if m < 6:
                    nc.scalar.copy(d2a[:, m, :], mp)
                else:
                    nc.scalar.copy(d2b[0:16, :], mp[0:16, :])

            # logits + softmax per subtile
            for j in range(2):
                jsl = slice(j * P, (j + 1) * P)
                lgt = pp.tile([P, W], F32, tag="mp")
                lg = lgt[:, 0:10]
                for c in range(6):
                    nc.tensor.matmul(lg, d2a[:, c, jsl], ow_w[:, c, :],
                                     start=(c == 0), stop=False)
                nc.tensor.matmul(lg, d2b[:, jsl], ow_w[0:17, 6, :],
                                 start=False, stop=True)
                e10 = sm.tile([P, 10], F32, tag="e10")
                s10 = sm.tile([P, 1], F32, tag="s10")
                nc.scalar.activation(e10, lg, AF.Exp, accum_out=s10)
                r10 = sm.tile([P, 1], F32, tag="r10")
                nc.vector.reciprocal(r10, s10)
                o10 = sm.tile([P, 10], F32, tag="o10")
                nc.vector.tensor_scalar_mul(o10, e10, r10)
                nc.sync.dma_start(
                    out=out_d[ds(g * (nsub * P) + (2 * wave + j) * P, P), :],
                    in_=o10
                )

        def body(g, preloaded=None):
            nw = nsub // 2
            h0s, cs = [], [None] * nw
            vts = []
            mtk = {}  # (wave, k) -> Mt tile
            # k=0 proj+dots interleaved per tile so the first tile's chain
            # races ahead of later tiles' loads
            for j in range(nsub):
                w = j // 2
                if j % 2 == 0:
                    mtk[(w, 0)] = mtp.tile([P, 8, T], F32, tag=f"mtk{w}",
                                           bufs=3, name=f"mt0w{w}")
                h0 = load_dma(g, j) if preloaded is None else preloaded[j]
                h0s.append(h0)
                vt1, vt2 = prep_tile(j, h0)
                vts.append((vt1, vt2))
                pk = proj(j, 0, vt1, vt2)
                dots(j, 0, mtk[(w, 0)], pk)
            # waves are staggered one k apart: early waves finish their
            # chains (and start decoding) while late waves still compute
            LAG = 3
            for step in range(1, 8 + LAG * (nw - 1) + 1):
                for w in range(nw):
                    k = step - LAG * w
                    if 1 <= k <= 7:
                        mtk[(w, k)] = mtp.tile([P, 8, T], F32, tag=f"mtk{w}",
                                               bufs=3, name=f"mt{k}w{w}")
                        for j in (2 * w, 2 * w + 1):
                            pk = proj(j, k, *vts[j])
                            dots(j, k, mtk[(w, k)], pk)
                        cs[w] = serial_phase(k - 1, w, mtk[(w, k - 1)], cs[w])
                    elif k == 8:
                        cs[w] = serial_phase(7, w, mtk[(w, 7)], cs[w])
                        wsp = [recon(2 * w, h0s[2 * w], cs[w]),
                               recon(2 * w + 1, h0s[2 * w + 1], cs[w])]
                        decoder(wsp, g, w)

        # group 0's x DMAs first so they precede the 6MB of decoder weights
        # on the sync queue; weights stream in during attention
        pre0 = [load_dma(0, j) for j in range(nsub)]
        nc.sync.dma_start(out=d1_w, in_=d1_d[:, :, :])
        nc.sync.dma_start(out=d2_w, in_=d2_d[:, :, :])
        nc.sync.dma_start(out=ow_w, in_=ow_d[:, :, :])
        body(0, preloaded=pre0)
        if ngroups > 1:
            with tc.For_i(1, ngroups, 1) as g:
                body(g)
        for _pool in (pp, wkd, wp, sm, mtp, scp, pkp, vp, hp, consts):
            _pool.release()

    nc.compile()
    return nc


def pack_weights(W1, b1, W2, b2, W3, b3, Wd1, bd1, Wd2, bd2, Wo, bo):
    f64 = np.float64
    W1, b1, W2, b2, W3, b3 = (np.asarray(t, f64) for t in (W1, b1, W2, b2, W3, b3))
    G = W1.T @ W2
    a = W2.T @ b1

    A = np.eye(FV)
    m = np.zeros(FV)
    pw = np.zeros((P, 2, PTOT), np.float32)
    for k in range(8):
        Gk = A.T @ G @ A
        ak = A.T @ (G.T @ m + a)
        nco = NCOLS[k]
        Wk = np.zeros((197, nco), f64)
        r = RANKS[k]
        r1 = r + 1
        U, S, Vh = np.linalg.svd(Gk)
        Wk[:FV, :r] = (np.diag(S[:r]) @ Vh[:r]).T
        Wk[:FV, r] = ak
        Wk[:FV, r1 : r1 + r] = U[:, :r]
        Wk[FV, r1 + r] = 1.0
        off = POFF[k]
        pw[:, 0, off : off + nco] = Wk[0:128]
        pw[0:69, 1, off : off + nco] = Wk[128:197]
        A = W3 @ A
        m = W3 @ m + b3
    A8, m8 = A, m

    # fold W3^8 / m8 into the first decoder layer
    BD = np.zeros((FEAT, FEAT), f64)
    mm = np.zeros(FEAT, f64)
    for t in range(T):
        BD[t * FV : (t + 1) * FV, t * FV : (t + 1) * FV] = A8
        mm[t * FV : (t + 1) * FV] = m8
    Wd1f = np.asarray(Wd1, f64) @ BD
    bd1f = np.asarray(bd1, f64) + np.asarray(Wd1, f64) @ mm

    d1 = np.zeros((P, 8, FEAT), np.float32)
    W1T = Wd1f.T  # [784 f_in, 784 j]
    for t in range(T):
        d1[:, t, :] = W1T[t * FV : t * FV + P, :]
        d1[0:68, 4 + t, :] = W1T[t * FV + P : (t + 1) * FV, :]
    d1[68, 4, :] = bd1f

    d2 = np.zeros((P, 7, FEAT), np.float32)
    W2T = np.asarray(Wd2, f64).T
    for cidx in range(6):
        d2[:, cidx, :] = W2T[cidx * P : (cidx + 1) * P, :]
    d2[0:16, 6, :] = W2T[768:784, :]
    d2[16, 6, :] = np.asarray(bd2, f64)

    ow = np.zeros((P, 7, 10), np.float32)
    WoT = np.asarray(Wo, f64).T
    for cidx in range(6):
        ow[:, cidx, :] = WoT[cidx * P : (cidx + 1) * P, :]
    ow[0:16, 6, :] = WoT[768:784, :]
    ow[16, 6, :] = np.asarray(bo, f64)
    return pw.astype(ml_dtypes.bfloat16), d1, d2, ow


_NC_CACHE = {}


def kernel(**inputs):
    x = np.ascontiguousarray(np.asarray(inputs["x"], np.float32))
    zu, d1, d2, ow = pack_weights(
        inputs["W1"], inputs["b1"], inputs["W2"], inputs["b2"], inputs["W3"],
        inputs["b3"], inputs["Wd1"], inputs["bd1"], inputs["Wd2"],
        inputs["bd2"], inputs["Wo"], inputs["bo"],
    )
    if "nc" not in _NC_CACHE:
        _NC_CACHE["nc"] = build(NSUB, BPC // (NSUB * P))
    nc = _NC_CACHE["nc"]
    bpc = B // NCORES
    in_maps = [
        {
            "x": x[c * bpc : (c + 1) * bpc],
            "zu_w": zu,
            "dec1_w": d1,
            "dec2_w": d2,
            "out_w": ow,
        }
        for c in range(NCORES)
    ]
    res = run_bass_kernel_spmd(nc, in_maps, core_ids=list(range(NCORES)))
    return np.concatenate([res.results[c]["out"] for c in range(NCORES)], axis=0)


# revision 84
# speedup vs baseline: 3.7163x; 1.0036x over previous
"""Trainium2 Bass kernel for nn_CapsuleNeuralNetworkV2 (8 cores, data-parallel).

Reference math (per sample, 8 capsule iterations then decoder):
  v = h.reshape(4, 196); q,k,u = affine(v); scores = q k^T;
  P = softmax(scores); h' = P u;  dec = relu(h Wd1^T+bd1) Wd2^T+bd2;
  out = softmax(dec Wo^T + bo).

Restructuring (host-side algebra):
  Since each P has rows summing to 1, the state stays in the span of the 4
  initial slots: v^(k) = W3^k w^(k) + m_k with w^(k) = C^(k) V (C is a
  per-sample 4x4 convex-coefficient matrix, V the initial slots).
  scores^(k)[t,s] = C[t] M_k C[s]^T (mod per-t constants that cancel in
  softmax), where M_k[i,j] = v_i.(G_k v_j) + a_k.v_j depends only on the
  INITIAL slots: G_k = (W3^k)^T G W3^k, G = W1^T W2,
  a_k = (W3^k)^T (G^T m_k + W2^T b1).  G_k is numerically low-rank for k>=1
  (powers of a random matrix), so M_k is computed from rank-r_k SVD
  projections p_i = U_r^T v_i, q_j = (S V_r^T) v_j: M[i,j] ~ p_i.q_j + r_j.
  Per iteration only the tiny 4x4 chain is sequential:
  scores = C M C^T -> softmax -> C' = P C.  All projections/M_k are
  C-independent and pipeline on PE/Act/DVE ahead of the chain.
  Final w^(8) = C^(8) V; W3^8/m_8 are folded into Wd1/bd1 on the host.

Schedule: 8 tiles of 128 samples per hardware-loop group (4 groups/core),
paired into 4 "waves" whose 4x4 chains are staggered 3 iterations apart so
early waves' recon+decoder (N=256 fp32r matmuls) overlap late waves'
chains.  PE: one set of transposes per tile + small bf16 projection
matmuls + decoder.  DVE: per-sample dot products (stt-accum at k=0, bf16
2x tensor_tensor + inner-axis reduce for k>=1) and the wide per-wave chain
ops (with 4x-replicated C/D copies to stay within 3-free-dim APs).  Pool:
chain tensor_tensors for early k, replicate-copies, recon t=3.  Act: PSUM
evacuation, exp, recon seeds, decoder activations.  Group 0's x DMAs are
emitted before the 6MB of decoder weights on the sync queue so compute
starts immediately; weights stream in during attention.
"""

import numpy as np
import ml_dtypes

import concourse.bass as bass
import concourse.tile as tile
from concourse import bacc, mybir
from concourse.bass import ds
from concourse.bass_utils import run_bass_kernel_spmd
from concourse.masks import make_identity

FR = mybir.dt.float32r
BF = mybir.dt.bfloat16
F32 = mybir.dt.float32
AF = mybir.ActivationFunctionType
ALU = mybir.AluOpType

B = 32768
NCORES = 8
NSUB = 8
BPC = B // NCORES
P = 128
T = 4
FV = 196
FEAT = 784
SLOT = 198  # h slot: 196 data + ones col (196) + spare (197)

RANKS = [80, 48, 32, 24, 16, 12, 8, 8]
NCOLS = [2 * (r + 1) for r in RANKS]  # proj cols per slot per k
POFF = [0]
for _n in NCOLS:
    POFF.append(POFF[-1] + _n)
PTOT = POFF[-1]
NCMAX = max(NCOLS)


def _ap(t, dims, offset_elems=0):
    """Hand-built AP over a tile's tensor: dims = [[step, count], ...]."""
    a = t[:] if hasattr(t, "tile") or not isinstance(t, bass.AP) else t
    return bass.AP(tensor=a.tensor, offset=a.offset + offset_elems, ap=dims)


def build(nsub=8, ngroups=4):
    """One NeuronCore program processing nsub*ngroups*128 samples."""
    bpc = nsub * ngroups * P
    nc = bacc.Bacc("TRN2", target_bir_lowering=False, debug=False)

    x_d = nc.dram_tensor("x", [bpc, FEAT], FR, kind="ExternalInput")
    pw_d = nc.dram_tensor("zu_w", [P, 2, PTOT], BF, kind="ExternalInput")
    d1_d = nc.dram_tensor("dec1_w", [P, 8, FEAT], FR, kind="ExternalInput")
    d2_d = nc.dram_tensor("dec2_w", [P, 7, FEAT], FR, kind="ExternalInput")
    ow_d = nc.dram_tensor("out_w", [P, 7, 10], FR, kind="ExternalInput")
    out_d = nc.dram_tensor("out", [bpc, 10], F32, kind="ExternalOutput")

    with tile.TileContext(nc) as tc:
        consts = tc.alloc_tile_pool(name="consts", bufs=1)
        hp = tc.alloc_tile_pool(name="h", bufs=1)
        vp = tc.alloc_tile_pool(name="vt", bufs=1)
        pkp = tc.alloc_tile_pool(name="pk", bufs=2)
        scp = tc.alloc_tile_pool(name="scr", bufs=4)
        mtp = tc.alloc_tile_pool(name="mt", bufs=8)
        sm = tc.alloc_tile_pool(name="small", bufs=3)
        wp = tc.alloc_tile_pool(name="w", bufs=2)
        wkd = tc.alloc_tile_pool(name="wkd", bufs=1)
        pp = tc.alloc_tile_pool(name="ps", bufs=2, space="PSUM")

        ident_f = consts.tile([P, P], F32)
        make_identity(nc, ident_f)
        ident_r = consts.tile([P, P], FR)
        nc.vector.tensor_copy(ident_r, ident_f)
        ones_c = consts.tile([P, 512], F32)
        nc.vector.memset(ones_c, 1.0)
        pw = consts.tile([P, 2, PTOT], BF)
        nc.sync.dma_start(out=pw, in_=pw_d[:, :, :])
        # decoder weights DMA'd after group 0's x tiles (emitted in build
        # below) so the first group's compute isn't starved behind 6MB
        d1_w = consts.tile([P, 8, FEAT], FR)
        d2_w = consts.tile([P, 7, FEAT], FR)
        ow_w = consts.tile([P, 7, 10], FR)

        def load_dma(g, j):
            h0 = hp.tile([P, T, SLOT], FR, tag=f"h{j}")
            nc.sync.dma_start(
                out=h0[:, :, 0:FV],
                in_=x_d[ds(g * (nsub * P) + j * P, P), :].rearrange(
                    "p (t f) -> p t f", t=T
                ),
            )
            nc.gpsimd.tensor_copy(h0[:, :, 196:198], ones_c[:, 0 : 2 * T])
            return h0

        def prep_tile(j, h0):
            vt1 = vp.tile([P, T, P], BF, tag=f"vt1{j}")
            vt2 = vp.tile([69, T, P], BF, tag=f"vt2{j}")
            t1_ps = pp.tile([P, T, P], FR, tag="t1ps", bufs=1)
            t2_ps = pp.tile([69, T, P], FR, tag="t2ps", bufs=1)
            for t in range(T):
                nc.tensor.transpose(t1_ps[:, t, :], h0[:, t, 0:P], ident_r)
                nc.tensor.transpose(t2_ps[:, t, :], h0[:, t, P : P + 69], ident_r)
            nc.scalar.copy(vt1, t1_ps)
            nc.scalar.copy(vt2, t2_ps)
            return vt1, vt2

        def load_tile(g, j):
            h0 = load_dma(g, j)
            return h0, None, None, None

        def proj(j, k, vt1, vt2):
            """PE projections for iteration k -> pk [128, 4, nc] bf16."""
            nco = NCOLS[k]
            off = POFF[k]
            pk = pkp.tile([P, T, NCMAX], BF, tag=f"pk{j}")
            if k == 0:
                for half in range(2):
                    ps = pp.tile([P, 2, NCMAX], F32, tag="pkps", bufs=2)
                    for sl in range(2):
                        s = half * 2 + sl
                        nc.tensor.matmul(
                            ps[:, sl, 0:nco], vt1[:, s, :],
                            pw[:, 0, off : off + nco], start=True, stop=False)
                        nc.tensor.matmul(
                            ps[:, sl, 0:nco], vt2[0:69, s, :],
                            pw[0:69, 1, off : off + nco], start=False, stop=True)
                    nc.scalar.copy(
                        pk[:, 2 * half : 2 * half + 2, 0:nco], ps[:, :, 0:nco])
            else:
                ps = pp.tile([P, T, 98], F32, tag="pkps1", bufs=2)
                for s in range(T):
                    nc.tensor.matmul(
                        ps[:, s, 0:nco], vt1[:, s, :],
                        pw[:, 0, off : off + nco], start=True, stop=False)
                    nc.tensor.matmul(
                        ps[:, s, 0:nco], vt2[0:69, s, :],
                        pw[0:69, 1, off : off + nco], start=False, stop=True)
                nc.scalar.copy(pk[:, :, 0:nco], ps[:, :, 0:nco])
            return pk

        def dots(j, k, mtc, pk):
            """M_k[i,j] for all 16 slot pairs -> wave-mtc rows 4(j%2)+i."""
            r1 = RANKS[k] + 1
            pap = pk[:].ap[0]
            jw = j % 2
            if k == 0:
                # big r: fused stt (mult + f32 accumulate in one 1x pass)
                scr = scp.tile([P, 256], BF, tag="scr197", bufs=4)
                for idx in range(16):
                    i, jj = idx // 4, idx % 4
                    in0 = _ap(pk, [pap, [1, r1]],
                              offset_elems=i * NCMAX + r1)
                    in1 = _ap(pk, [pap, [1, r1]], offset_elems=jj * NCMAX)
                    nc.vector.scalar_tensor_tensor(
                        out=scr[:, 0:r1], in0=in0, scalar=1.0, in1=in1,
                        op0=ALU.mult, op1=ALU.mult,
                        accum_out=mtc[:, 4 * jw + i, jj : jj + 1])
            else:
                # small r: one bf16 2x tensor_tensor + one inner-axis reduce
                scr = scp.tile([P, T, T, 65], BF, tag="scr", bufs=3)
                in0 = _ap(pk, [pap, [NCMAX, 4], [0, 4], [1, r1]],
                          offset_elems=r1)
                in1 = _ap(pk, [pap, [0, 4], [NCMAX, 4], [1, r1]])
                nc.vector.tensor_tensor(
                    out=scr[:, :, :, 0:r1], in0=in0, in1=in1, op=ALU.mult)
                nc.vector.tensor_reduce(
                    out=mtc[:, 4 * jw : 4 * jw + 4, :], in_=scr[:, :, :, 0:r1],
                    axis=mybir.AxisListType.X, op=ALU.add)

        def serial_phase(k, w, mtc, c_prev):
            """Per-k 4x4 chain for one WAVE (2 tiles) in wide DVE ops over a
            [128, (j,t), s] layout (j in the wave): scores = C mt C^T ->
            e = exp -> C'u = e C -> C' = C'u / rowsum. Returns new C tile."""
            JT = 8   # (2 tiles) x (4 slots)
            JR = 32  # replicated size per tile pair
            if k == 0:
                s_t = mtc
            else:
                cap = c_prev[:].ap[0]
                # replicate C 4x -> crep[j, rep, s, jj] so every TT operand
                # stays within the ISA's 3-free-dim AP limit
                crep = sm.tile([P, 4 * JR], F32, tag=f"crep{w}", bufs=2)
                nc.gpsimd.tensor_copy(
                    _ap(crep, [crep[:].ap[0], [64, 2], [16, 4], [1, 16]]),
                    _ap(c_prev, [cap, [16, 2], [0, 4], [1, 16]]))
                tt_eng = nc.gpsimd
                scrd = scp.tile([P, JT, T, T], F32, tag="scrd", bufs=6)
                tt_eng.tensor_tensor(  # D[j,i,s] = sum_jj mt[j,i,jj] C[j,s,jj]
                    out=scrd,
                    in0=_ap(mtc, [mtc[:].ap[0], [4, JT], [0, 4], [1, 4]]),
                    in1=crep[:],
                    op=ALU.mult)
                dm = sm.tile([P, JT, T], F32, tag=f"dm{w}")
                nc.vector.tensor_reduce(
                    out=dm, in_=scrd, axis=mybir.AxisListType.X, op=ALU.add)
                drep = sm.tile([P, 4 * JR], F32, tag=f"drep{w}", bufs=2)
                nc.gpsimd.tensor_copy(
                    _ap(drep, [drep[:].ap[0], [64, 2], [16, 4], [1, 16]]),
                    _ap(dm, [dm[:].ap[0], [16, 2], [0, 4], [1, 16]]))
                scrd2 = scp.tile([P, JT, T, T], F32, tag="scrd", bufs=6)
                tt_eng.tensor_tensor(  # S[j,t,s] = sum_i C[j,t,i] D[j,i,s]
                    out=scrd2,
                    in0=_ap(c_prev, [cap, [4, JT], [0, 4], [1, 4]]),
                    in1=_ap(drep, [drep[:].ap[0], [16, JT], [1, 4], [4, 4]]),
                    op=ALU.mult)
                s_t = sm.tile([P, JT, T], F32, tag=f"st{w}")
                nc.vector.tensor_reduce(
                    out=s_t, in_=scrd2, axis=mybir.AxisListType.X, op=ALU.add)
            e = sm.tile([P, JT, T], F32, tag=f"e{w}")
            nc.scalar.activation(e, s_t, AF.Exp)
            sums = sm.tile([P, JT], F32, tag=f"su{w}")
            nc.vector.reduce_sum(sums, e, axis=mybir.AxisListType.X)
            rec = sm.tile([P, JT], F32, tag=f"re{w}")
            nc.vector.reciprocal(rec, sums)
            if k == 0:
                cnum = e
            else:
                scrd3 = scp.tile([P, JT, T, T], F32, tag="scrd", bufs=6)
                tt_eng.tensor_tensor(  # C'u[j,t,jj] = sum_s e[j,t,s] C[j,s,jj]
                    out=scrd3,
                    in0=_ap(e, [e[:].ap[0], [4, JT], [0, 4], [1, 4]]),
                    in1=_ap(crep, [crep[:].ap[0], [16, JT], [1, 4], [4, 4]]),
                    op=ALU.mult)
                cnum = sm.tile([P, JT, T], F32, tag=f"cu{w}")
                nc.vector.tensor_reduce(
                    out=cnum, in_=scrd3, axis=mybir.AxisListType.X, op=ALU.add)
            c_new = sm.tile([P, JT, T], F32, tag=f"call{w}", bufs=3)
            nc.vector.scalar_tensor_tensor(
                out=c_new, in0=cnum, scalar=1.0,
                in1=_ap(rec, [rec[:].ap[0], [1, JT], [0, T]]),
                op0=ALU.mult, op1=ALU.mult)
            return c_new

        def recon(j, h0, c8):
            """w[:, t, :] = sum_s C8[t,s] * h0[:, s, :] (ones col rides along)."""
            w = wp.tile([P, T, SLOT], FR, tag=f"w{j % 4}", bufs=1)
            jw = j % 2
            for t in range(T):
                nc.scalar.activation(
                    w[:, t, :], h0[:, 0, :], AF.Copy,
                    scale=c8[:, 4 * jw + t, 0:1])
            for t in range(3):
                for s in range(1, T):
                    nc.vector.scalar_tensor_tensor(
                        out=w[:, t, :], in0=h0[:, s, :],
                        scalar=c8[:, 4 * jw + t, s : s + 1], in1=w[:, t, :],
                        op0=ALU.mult, op1=ALU.add)
            for t in range(3, T):
                pct = wp.tile([P, SLOT], F32, tag="pct", bufs=2)
                for s in range(1, T):
                    nc.gpsimd.tensor_scalar_mul(
                        pct, h0[:, s, :], c8[:, 4 * jw + t, s : s + 1])
                    nc.gpsimd.tensor_add(w[:, t, :], w[:, t, :], pct)
            return w

        def decoder(ws2, g, wave):
            """Decoder over one wave of 2 tiles (N = 256 wide matmuls)."""
            W = 2 * P
            ht1 = wkd.tile([P, T, W], FR, tag=f"ht1w{wave % 2}")
            ht2 = wkd.tile([69, T, W], FR, tag=f"ht2w{wave % 2}")
            for t in range(T):
                t1_ps = pp.tile([P, T, P], FR, tag="t1ps", bufs=1)
                t2_ps = pp.tile([69, T, P], FR, tag="t2ps", bufs=1)
                for j in range(2):
                    nc.tensor.transpose(
                        t1_ps[:, j, :], ws2[j][:, t, 0:P], ident_r
                    )
                    nc.tensor.transpose(
                        t2_ps[:, j, :], ws2[j][:, t, P : P + 69], ident_r
                    )
                nc.scalar.copy(ht1[:, t, :], t1_ps[:, 0:2, :])
                nc.vector.tensor_copy(ht2[:, t, :], t2_ps[:, 0:2, :])

            # dec1 = relu(Wd1~ @ w.T + bd1~), feature-major, 7 M-chunks
            d1a = wkd.tile([P, 6, W], FR, tag=f"d1aw{wave % 2}")
            d1b = wkd.tile([17, W], FR, tag=f"d1bw{wave % 2}")
            nc.vector.tensor_copy(d1b, ones_c[0:17, 0:W])
            for m in range(7):
                mw = min(P, FEAT - m * P)
                mp = pp.tile([P, W], F32, tag="mp")
                msl = slice(m * P, m * P + mw)
                for t in range(T):
                    nc.tensor.matmul(mp[0:mw, :], d1_w[:, t, msl], ht1[:, t, :],
                                     start=(t == 0), stop=False)
                for t in range(T):
                    nc.tensor.matmul(mp[0:mw, :], d1_w[0:69, 4 + t, msl],
                                     ht2[:, t, :], start=False, stop=(t == 3))
                if m < 6:
                    nc.scalar.activation(d1a[:, m, :], mp, AF.Relu)
                else:
                    nc.scalar.activation(d1b[0:16, :], mp[0:16, :], AF.Relu)

            # dec2 = Wd2 @ relu1 + bd2, feature-major
            d2a = wkd.tile([P, 6, W], FR, tag=f"d2aw{wave % 2}")
            d2b = wkd.tile([17, W], FR, tag=f"d2bw{wave % 2}")
            nc.vector.tensor_copy(d2b, ones_c[0:17, 0:W])
            for m in range(7):
                mw = min(P, FEAT - m * P)
                mp = pp.tile([P, W], F32, tag="mp")
                msl = slice(m * P, m * P + mw)
                for c in range(6):
                    nc.tensor.matmul(mp[0:mw, :], d2_w[:, c, msl], d1a[:, c, :],
                                     start=(c == 0), stop=False)
                nc.tensor.matmul(mp[0:mw, :], d2_w[0:17, 6, msl], d1b,
                                 start=False, stop=True)
                if m < 6:
                    nc.scalar.copy(d2a[:, m, :], mp)
                else:
                    nc.scalar.copy(d2b[0:16, :], mp[0:16, :])

            # logits + softmax per subtile
            for j in range(2):
                jsl = slice(j * P, (j + 1) * P)
                lgt = pp.tile([P, W], F32, tag="mp")
                lg = lgt[:, 0:10]
                for c in range(6):
                    nc.tensor.matmul(lg, d2a[:, c, jsl], ow_w[:, c, :],
                                     start=(c == 0), stop=False)
                nc.tensor.matmul(lg, d2b[:, jsl], ow_w[0:17, 6, :],
                                 start=False, stop=True)
                e10 = sm.tile([P, 10], F32, tag="e10")
                s10 = sm.tile([P, 1], F32, tag="s10")
                nc.scalar.activation(e10, lg, AF.Exp, accum_out=s10)
                r10 = sm.tile([P, 1], F32, tag="r10")
                nc.vector.reciprocal(r10, s10)
                o10 = sm.tile([P, 10], F32, tag="o10")
                nc.vector.tensor_scalar_mul(o10, e10, r10)
                nc.sync.dma_start(
                    out=out_d[ds(g * (nsub * P) + (2 * wave + j) * P, P), :],
                    in_=o10
                )

        def body(g, preloaded=None):
            nw = nsub // 2
            h0s, cs = [], [None] * nw
            vts = []
            mtk = {}  # (wave, k) -> Mt tile
            # k=0 proj+dots interleaved per tile so the first tile's chain
            # races ahead of later tiles' loads
            for j in range(nsub):
                w = j // 2
                if j % 2 == 0:
                    mtk[(w, 0)] = mtp.tile([P, 8, T], F32, tag=f"mtk{w}",
                                           bufs=3, name=f"mt0w{w}")
                h0 = load_dma(g, j) if preloaded is None else preloaded[j]
                h0s.append(h0)
                vt1, vt2 = prep_tile(j, h0)
                vts.append((vt1, vt2))
                pk = proj(j, 0, vt1, vt2)
                dots(j, 0, mtk[(w, 0)], pk)
            # waves are staggered one k apart: early waves finish their
            # chains (and start decoding) while late waves still compute
            LAG = 3
            for step in range(1, 8 + LAG * (nw - 1) + 1):
                for w in range(nw):
                    k = step - LAG * w
                    if 1 <= k <= 7:
                        mtk[(w, k)] = mtp.tile([P, 8, T], F32, tag=f"mtk{w}",
                                               bufs=3, name=f"mt{k}w{w}")
                        for j in (2 * w, 2 * w + 1):
                            pk = proj(j, k, *vts[j])
                            dots(j, k, mtk[(w, k)], pk)
                        cs[w] = serial_phase(k - 1, w, mtk[(w, k - 1)], cs[w])
                    elif k == 8:
                        cs[w] = serial_phase(7, w, mtk[(w, 7)], cs[w])
                        wsp = [recon(2 * w, h0s[2 * w], cs[w]),
                               recon(2 * w + 1, h0s[2 * w + 1], cs[w])]
                        decoder(wsp, g, w)

        # group 0's x DMAs first so they precede the 6MB of decoder weights
        # on the sync queue; weights stream in during attention
        pre0 = [load_dma(0, j) for j in range(nsub)]
        nc.sync.dma_start(out=d1_w, in_=d1_d[:, :, :])
        nc.sync.dma_start(out=d2_w, in_=d2_d[:, :, :])
        nc.sync.dma_start(out=ow_w, in_=ow_d[:, :, :])
        body(0, preloaded=pre0)
        if ngroups > 1:
            with tc.For_i(1, ngroups, 1) as g:
                body(g)
        for _pool in (pp, wkd, wp, sm, mtp, scp, pkp, vp, hp, consts):
            _pool.release()

    nc.compile()
    return nc


def pack_weights(W1, b1, W2, b2, W3, b3, Wd1, bd1, Wd2, bd2, Wo, bo):
    f64 = np.float64
    W1, b1, W2, b2, W3, b3 = (np.asarray(t, f64) for t in (W1, b1, W2, b2, W3, b3))
    G = W1.T @ W2
    a = W2.T @ b1

    A = np.eye(FV)
    m = np.zeros(FV)
    pw = np.zeros((P, 2, PTOT), np.float32)
    for k in range(8):
        Gk = A.T @ G @ A
        ak = A.T @ (G.T @ m + a)
        nco = NCOLS[k]
        Wk = np.zeros((197, nco), f64)
        r = RANKS[k]
        r1 = r + 1
        U, S, Vh = np.linalg.svd(Gk)
        Wk[:FV, :r] = (np.diag(S[:r]) @ Vh[:r]).T
        Wk[:FV, r] = ak
        Wk[:FV, r1 : r1 + r] = U[:, :r]
        Wk[FV, r1 + r] = 1.0
        off = POFF[k]
        pw[:, 0, off : off + nco] = Wk[0:128]
        pw[0:69, 1, off : off + nco] = Wk[128:197]
        A = W3 @ A
        m = W3 @ m + b3
    A8, m8 = A, m

    # fold W3^8 / m8 into the first decoder layer
    BD = np.zeros((FEAT, FEAT), f64)
    mm = np.zeros(FEAT, f64)
    for t in range(T):
        BD[t * FV : (t + 1) * FV, t * FV : (t + 1) * FV] = A8
        mm[t * FV : (t + 1) * FV] = m8
    Wd1f = np.asarray(Wd1, f64) @ BD
    bd1f = np.asarray(bd1, f64) + np.asarray(Wd1, f64) @ mm

    d1 = np.zeros((P, 8, FEAT), np.float32)
    W1T = Wd1f.T  # [784 f_in, 784 j]
    for t in range(T):
        d1[:, t, :] = W1T[t * FV : t * FV + P, :]
        d1[0:68, 4 + t, :] = W1T[t * FV + P : (t + 1) * FV, :]
    d1[68, 4, :] = bd1f

    d2 = np.zeros((P, 7, FEAT), np.float32)
    W2T = np.asarray(Wd2, f64).T
    for cidx in range(6):
        d2[:, cidx, :] = W2T[cidx * P : (cidx + 1) * P, :]
    d2[0:16, 6, :] = W2T[768:784, :]
    d2[16, 6, :] = np.asarray(bd2, f64)

    ow = np.zeros((P, 7, 10), np.float32)
    WoT = np.asarray(Wo, f64).T
    for cidx in range(6):
        ow[:, cidx, :] = WoT[cidx * P : (cidx + 1) * P, :]
    ow[0:16, 6, :] = WoT[768:784, :]
    ow[16, 6, :] = np.asarray(bo, f64)
    return pw.astype(ml_dtypes.bfloat16), d1, d2, ow


_NC_CACHE = {}


def kernel(**inputs):
    x = np.ascontiguousarray(np.asarray(inputs["x"], np.float32))
    zu, d1, d2, ow = pack_weights(
        inputs["W1"], inputs["b1"], inputs["W2"], inputs["b2"], inputs["W3"],
        inputs["b3"], inputs["Wd1"], inputs["bd1"], inputs["Wd2"],
        inputs["bd2"], inputs["Wo"], inputs["bo"],
    )
    if "nc" not in _NC_CACHE:
        _NC_CACHE["nc"] = build(NSUB, BPC // (NSUB * P))
    nc = _NC_CACHE["nc"]
    bpc = B // NCORES
    in_maps = [
        {
            "x": x[c * bpc : (c + 1) * bpc],
            "zu_w": zu,
            "dec1_w": d1,
            "dec2_w": d2,
            "out_w": ow,
        }
        for c in range(NCORES)
    ]
    res = run_bass_kernel_spmd(nc, in_maps, core_ids=list(range(NCORES)))
    return np.concatenate([res.results[c]["out"] for c in range(NCORES)], axis=0)


# revision 85
# speedup vs baseline: 3.7972x; 1.0218x over previous
"""Trainium2 Bass kernel for nn_CapsuleNeuralNetworkV2 (8 cores, data-parallel).

Reference math (per sample, 8 capsule iterations then decoder):
  v = h.reshape(4, 196); q,k,u = affine(v); scores = q k^T;
  P = softmax(scores); h' = P u;  dec = relu(h Wd1^T+bd1) Wd2^T+bd2;
  out = softmax(dec Wo^T + bo).

Restructuring (host-side algebra):
  Since each P has rows summing to 1, the state stays in the span of the 4
  initial slots: v^(k) = W3^k w^(k) + m_k with w^(k) = C^(k) V (C is a
  per-sample 4x4 convex-coefficient matrix, V the initial slots).
  scores^(k)[t,s] = C[t] M_k C[s]^T (mod per-t constants that cancel in
  softmax), where M_k[i,j] = v_i.(G_k v_j) + a_k.v_j depends only on the
  INITIAL slots: G_k = (W3^k)^T G W3^k, G = W1^T W2,
  a_k = (W3^k)^T (G^T m_k + W2^T b1).  G_k is numerically low-rank for k>=1
  (powers of a random matrix), so M_k is computed from rank-r_k SVD
  projections p_i = U_r^T v_i, q_j = (S V_r^T) v_j: M[i,j] ~ p_i.q_j + r_j.
  Per iteration only the tiny 4x4 chain is sequential:
  scores = C M C^T -> softmax -> C' = P C.  All projections/M_k are
  C-independent and pipeline on PE/Act/DVE ahead of the chain.
  Final w^(8) = C^(8) V; W3^8/m_8 are folded into Wd1/bd1 on the host.

Schedule: 8 tiles of 128 samples per hardware-loop group (4 groups/core),
paired into 4 "waves" whose 4x4 chains are staggered 3 iterations apart so
early waves' recon+decoder (N=256 fp32r matmuls) overlap late waves'
chains.  PE: one set of transposes per tile + small bf16 projection
matmuls + decoder.  DVE: per-sample dot products (stt-accum at k=0, bf16
2x tensor_tensor + inner-axis reduce for k>=1) and the wide per-wave chain
ops (with 4x-replicated C/D copies to stay within 3-free-dim APs).  Pool:
chain tensor_tensors for early k, replicate-copies, recon t=3.  Act: PSUM
evacuation, exp, recon seeds, decoder activations.  Group 0's x DMAs are
emitted before the 6MB of decoder weights on the sync queue so compute
starts immediately; weights stream in during attention.
"""

import numpy as np
import ml_dtypes

import concourse.bass as bass
import concourse.tile as tile
from concourse import bacc, mybir
from concourse.bass import ds
from concourse.bass_utils import run_bass_kernel_spmd
from concourse.masks import make_identity

FR = mybir.dt.float32r
BF = mybir.dt.bfloat16
F32 = mybir.dt.float32
AF = mybir.ActivationFunctionType
ALU = mybir.AluOpType

B = 32768
NCORES = 8
NSUB = 8
BPC = B // NCORES
P = 128
T = 4
FV = 196
FEAT = 784
SLOT = 198  # h slot: 196 data + ones col (196) + spare (197)

RANKS = [64, 40, 28, 20, 14, 10, 8, 8]
NCOLS = [2 * (r + 1) for r in RANKS]  # proj cols per slot per k
POFF = [0]
for _n in NCOLS:
    POFF.append(POFF[-1] + _n)
PTOT = POFF[-1]
NCMAX = max(NCOLS)


def _ap(t, dims, offset_elems=0):
    """Hand-built AP over a tile's tensor: dims = [[step, count], ...]."""
    a = t[:] if hasattr(t, "tile") or not isinstance(t, bass.AP) else t
    return bass.AP(tensor=a.tensor, offset=a.offset + offset_elems, ap=dims)


def build(nsub=8, ngroups=4):
    """One NeuronCore program processing nsub*ngroups*128 samples."""
    bpc = nsub * ngroups * P
    nc = bacc.Bacc("TRN2", target_bir_lowering=False, debug=False)

    x_d = nc.dram_tensor("x", [bpc, FEAT], FR, kind="ExternalInput")
    pw_d = nc.dram_tensor("zu_w", [P, 2, PTOT], BF, kind="ExternalInput")
    d1_d = nc.dram_tensor("dec1_w", [P, 8, FEAT], FR, kind="ExternalInput")
    d2_d = nc.dram_tensor("dec2_w", [P, 7, FEAT], FR, kind="ExternalInput")
    ow_d = nc.dram_tensor("out_w", [P, 7, 10], FR, kind="ExternalInput")
    out_d = nc.dram_tensor("out", [bpc, 10], F32, kind="ExternalOutput")

    with tile.TileContext(nc) as tc:
        consts = tc.alloc_tile_pool(name="consts", bufs=1)
        hp = tc.alloc_tile_pool(name="h", bufs=1)
        vp = tc.alloc_tile_pool(name="vt", bufs=1)
        pkp = tc.alloc_tile_pool(name="pk", bufs=2)
        scp = tc.alloc_tile_pool(name="scr", bufs=4)
        mtp = tc.alloc_tile_pool(name="mt", bufs=8)
        sm = tc.alloc_tile_pool(name="small", bufs=3)
        wp = tc.alloc_tile_pool(name="w", bufs=2)
        wkd = tc.alloc_tile_pool(name="wkd", bufs=1)
        pp = tc.alloc_tile_pool(name="ps", bufs=2, space="PSUM")

        ident_f = consts.tile([P, P], F32)
        make_identity(nc, ident_f)
        ident_r = consts.tile([P, P], FR)
        nc.vector.tensor_copy(ident_r, ident_f)
        ones_c = consts.tile([P, 512], F32)
        nc.vector.memset(ones_c, 1.0)
        pw = consts.tile([P, 2, PTOT], BF)
        nc.sync.dma_start(out=pw, in_=pw_d[:, :, :])
        # decoder weights DMA'd after group 0's x tiles (emitted in build
        # below) so the first group's compute isn't starved behind 6MB
        d1_w = consts.tile([P, 8, FEAT], FR)
        d2_w = consts.tile([P, 7, FEAT], FR)
        ow_w = consts.tile([P, 7, 10], FR)

        def load_dma(g, j):
            h0 = hp.tile([P, T, SLOT], FR, tag=f"h{j}")
            nc.sync.dma_start(
                out=h0[:, :, 0:FV],
                in_=x_d[ds(g * (nsub * P) + j * P, P), :].rearrange(
                    "p (t f) -> p t f", t=T
                ),
            )
            nc.gpsimd.tensor_copy(h0[:, :, 196:198], ones_c[:, 0 : 2 * T])
            return h0

        def prep_tile(j, h0):
            vt1 = vp.tile([P, T, P], BF, tag=f"vt1{j}")
            vt2 = vp.tile([69, T, P], BF, tag=f"vt2{j}")
            t1_ps = pp.tile([P, T, P], FR, tag="t1ps", bufs=1)
            t2_ps = pp.tile([69, T, P], FR, tag="t2ps", bufs=1)
            for t in range(T):
                nc.tensor.transpose(t1_ps[:, t, :], h0[:, t, 0:P], ident_r)
                nc.tensor.transpose(t2_ps[:, t, :], h0[:, t, P : P + 69], ident_r)
            nc.scalar.copy(vt1, t1_ps)
            nc.scalar.copy(vt2, t2_ps)
            return vt1, vt2

        def load_tile(g, j):
            h0 = load_dma(g, j)
            return h0, None, None, None

        def proj(j, k, vt1, vt2):
            """PE projections for iteration k -> pk [128, 4, nc] bf16."""
            nco = NCOLS[k]
            off = POFF[k]
            pk = pkp.tile([P, T, NCMAX], BF, tag=f"pk{j}")
            if k == 0:
                for half in range(2):
                    ps = pp.tile([P, 2, NCMAX], F32, tag="pkps", bufs=2)
                    for sl in range(2):
                        s = half * 2 + sl
                        nc.tensor.matmul(
                            ps[:, sl, 0:nco], vt1[:, s, :],
                            pw[:, 0, off : off + nco], start=True, stop=False)
                        nc.tensor.matmul(
                            ps[:, sl, 0:nco], vt2[0:69, s, :],
                            pw[0:69, 1, off : off + nco], start=False, stop=True)
                    nc.scalar.copy(
                        pk[:, 2 * half : 2 * half + 2, 0:nco], ps[:, :, 0:nco])
            else:
                ps = pp.tile([P, T, 98], F32, tag="pkps1", bufs=2)
                for s in range(T):
                    nc.tensor.matmul(
                        ps[:, s, 0:nco], vt1[:, s, :],
                        pw[:, 0, off : off + nco], start=True, stop=False)
                    nc.tensor.matmul(
                        ps[:, s, 0:nco], vt2[0:69, s, :],
                        pw[0:69, 1, off : off + nco], start=False, stop=True)
                nc.scalar.copy(pk[:, :, 0:nco], ps[:, :, 0:nco])
            return pk

        def dots(j, k, mtc, pk):
            """M_k[i,j] for all 16 slot pairs -> wave-mtc rows 4(j%2)+i."""
            r1 = RANKS[k] + 1
            pap = pk[:].ap[0]
            jw = j % 2
            if True:
                # one bf16 2x tensor_tensor + one inner-axis reduce
                scr = scp.tile([P, T, T, 65], BF, tag="scr", bufs=3)
                in0 = _ap(pk, [pap, [NCMAX, 4], [0, 4], [1, r1]],
                          offset_elems=r1)
                in1 = _ap(pk, [pap, [0, 4], [NCMAX, 4], [1, r1]])
                nc.vector.tensor_tensor(
                    out=scr[:, :, :, 0:r1], in0=in0, in1=in1, op=ALU.mult)
                nc.vector.tensor_reduce(
                    out=mtc[:, 4 * jw : 4 * jw + 4, :], in_=scr[:, :, :, 0:r1],
                    axis=mybir.AxisListType.X, op=ALU.add)

        def serial_phase(k, w, mtc, c_prev):
            """Per-k 4x4 chain for one WAVE (2 tiles) in wide DVE ops over a
            [128, (j,t), s] layout (j in the wave): scores = C mt C^T ->
            e = exp -> C'u = e C -> C' = C'u / rowsum. Returns new C tile."""
            JT = 8   # (2 tiles) x (4 slots)
            JR = 32  # replicated size per tile pair
            if k == 0:
                s_t = mtc
            else:
                cap = c_prev[:].ap[0]
                # replicate C 4x -> crep[j, rep, s, jj] so every TT operand
                # stays within the ISA's 3-free-dim AP limit
                crep = sm.tile([P, 4 * JR], F32, tag=f"crep{w}", bufs=2)
                nc.gpsimd.tensor_copy(
                    _ap(crep, [crep[:].ap[0], [64, 2], [16, 4], [1, 16]]),
                    _ap(c_prev, [cap, [16, 2], [0, 4], [1, 16]]))
                tt_eng = nc.gpsimd
                scrd = scp.tile([P, JT, T, T], F32, tag="scrd", bufs=6)
                tt_eng.tensor_tensor(  # D[j,i,s] = sum_jj mt[j,i,jj] C[j,s,jj]
                    out=scrd,
                    in0=_ap(mtc, [mtc[:].ap[0], [4, JT], [0, 4], [1, 4]]),
                    in1=crep[:],
                    op=ALU.mult)
                dm = sm.tile([P, JT, T], F32, tag=f"dm{w}")
                nc.vector.tensor_reduce(
                    out=dm, in_=scrd, axis=mybir.AxisListType.X, op=ALU.add)
                drep = sm.tile([P, 4 * JR], F32, tag=f"drep{w}", bufs=2)
                nc.gpsimd.tensor_copy(
                    _ap(drep, [drep[:].ap[0], [64, 2], [16, 4], [1, 16]]),
                    _ap(dm, [dm[:].ap[0], [16, 2], [0, 4], [1, 16]]))
                scrd2 = scp.tile([P, JT, T, T], F32, tag="scrd", bufs=6)
                tt_eng.tensor_tensor(  # S[j,t,s] = sum_i C[j,t,i] D[j,i,s]
                    out=scrd2,
                    in0=_ap(c_prev, [cap, [4, JT], [0, 4], [1, 4]]),
                    in1=_ap(drep, [drep[:].ap[0], [16, JT], [1, 4], [4, 4]]),
                    op=ALU.mult)
                s_t = sm.tile([P, JT, T], F32, tag=f"st{w}")
                nc.vector.tensor_reduce(
                    out=s_t, in_=scrd2, axis=mybir.AxisListType.X, op=ALU.add)
            e = sm.tile([P, JT, T], F32, tag=f"e{w}")
            nc.scalar.activation(e, s_t, AF.Exp)
            sums = sm.tile([P, JT], F32, tag=f"su{w}")
            nc.vector.reduce_sum(sums, e, axis=mybir.AxisListType.X)
            rec = sm.tile([P, JT], F32, tag=f"re{w}")
            nc.vector.reciprocal(rec, sums)
            if k == 0:
                cnum = e
            else:
                scrd3 = scp.tile([P, JT, T, T], F32, tag="scrd", bufs=6)
                tt_eng.tensor_tensor(  # C'u[j,t,jj] = sum_s e[j,t,s] C[j,s,jj]
                    out=scrd3,
                    in0=_ap(e, [e[:].ap[0], [4, JT], [0, 4], [1, 4]]),
                    in1=_ap(crep, [crep[:].ap[0], [16, JT], [1, 4], [4, 4]]),
                    op=ALU.mult)
                cnum = sm.tile([P, JT, T], F32, tag=f"cu{w}")
                nc.vector.tensor_reduce(
                    out=cnum, in_=scrd3, axis=mybir.AxisListType.X, op=ALU.add)
            c_new = sm.tile([P, JT, T], F32, tag=f"call{w}", bufs=3)
            nc.vector.scalar_tensor_tensor(
                out=c_new, in0=cnum, scalar=1.0,
                in1=_ap(rec, [rec[:].ap[0], [1, JT], [0, T]]),
                op0=ALU.mult, op1=ALU.mult)
            return c_new

        def recon(j, h0, c8):
            """w[:, t, :] = sum_s C8[t,s] * h0[:, s, :] (ones col rides along)."""
            w = wp.tile([P, T, SLOT], FR, tag=f"w{j % 4}", bufs=1)
            jw = j % 2
            for t in range(T):
                nc.scalar.activation(
                    w[:, t, :], h0[:, 0, :], AF.Copy,
                    scale=c8[:, 4 * jw + t, 0:1])
            for t in range(3):
                for s in range(1, T):
                    nc.vector.scalar_tensor_tensor(
                        out=w[:, t, :], in0=h0[:, s, :],
                        scalar=c8[:, 4 * jw + t, s : s + 1], in1=w[:, t, :],
                        op0=ALU.mult, op1=ALU.add)
            for t in range(3, T):
                pct = wp.tile([P, SLOT], F32, tag="pct", bufs=2)
                for s in range(1, T):
                    nc.gpsimd.tensor_scalar_mul(
                        pct, h0[:, s, :], c8[:, 4 * jw + t, s : s + 1])
                    nc.gpsimd.tensor_add(w[:, t, :], w[:, t, :], pct)
            return w

        def decoder(ws2, g, wave):
            """Decoder over one wave of 2 tiles (N = 256 wide matmuls)."""
            W = 2 * P
            ht1 = wkd.tile([P, T, W], FR, tag=f"ht1w{wave % 2}")
            ht2 = wkd.tile([69, T, W], FR, tag=f"ht2w{wave % 2}")
            for t in range(T):
                t1_ps = pp.tile([P, T, P], FR, tag="t1ps", bufs=1)
                t2_ps = pp.tile([69, T, P], FR, tag="t2ps", bufs=1)
                for j in range(2):
                    nc.tensor.transpose(
                        t1_ps[:, j, :], ws2[j][:, t, 0:P], ident_r
                    )
                    nc.tensor.transpose(
                        t2_ps[:, j, :], ws2[j][:, t, P : P + 69], ident_r
                    )
                nc.scalar.copy(ht1[:, t, :], t1_ps[:, 0:2, :])
                nc.vector.tensor_copy(ht2[:, t, :], t2_ps[:, 0:2, :])

            # dec1 = relu(Wd1~ @ w.T + bd1~), feature-major, 7 M-chunks
            d1a = wkd.tile([P, 6, W], FR, tag=f"d1aw{wave % 2}")
            d1b = wkd.tile([17, W], FR, tag=f"d1bw{wave % 2}")
            nc.vector.tensor_copy(d1b, ones_c[0:17, 0:W])
            for m in range(7):
                mw = min(P, FEAT - m * P)
                mp = pp.tile([P, W], F32, tag="mp")
                msl = slice(m * P, m * P + mw)
                for t in range(T):
                    nc.tensor.matmul(mp[0:mw, :], d1_w[:, t, msl], ht1[:, t, :],
                                     start=(t == 0), stop=False)
                for t in range(T):
                    nc.tensor.matmul(mp[0:mw, :], d1_w[0:69, 4 + t, msl],
                                     ht2[:, t, :], start=False, stop=(t == 3))
                if m < 6:
                    nc.scalar.activation(d1a[:, m, :], mp, AF.Relu)
                else:
                    nc.scalar.activation(d1b[0:16, :], mp[0:16, :], AF.Relu)

            # dec2 = Wd2 @ relu1 + bd2, feature-major
            d2a = wkd.tile([P, 6, W], FR, tag=f"d2aw{wave % 2}")
            d2b = wkd.tile([17, W], FR, tag=f"d2bw{wave % 2}")
            nc.vector.tensor_copy(d2b, ones_c[0:17, 0:W])
            for m in range(7):
                mw = min(P, FEAT - m * P)
                mp = pp.tile([P, W], F32, tag="mp")
                msl = slice(m * P, m * P + mw)
                for c in range(6):
                    nc.tensor.matmul(mp[0:mw, :], d2_w[:, c, msl], d1a[:, c, :],
                                     start=(c == 0), stop=False)
                nc.tensor.matmul(mp[0:mw, :], d2_w[0:17, 6, msl], d1b,
                                 start=False, stop=True)
                if m < 6:
                    nc.scalar.copy(d2a[:, m, :], mp)
                else:
                    nc.scalar.copy(d2b[0:16, :], mp[0:16, :])

            # logits + softmax per subtile
            for j in range(2):
                jsl = slice(j * P, (j + 1) * P)
                lgt = pp.tile([P, W], F32, tag="mp")
                lg = lgt[:, 0:10]
                for c in range(6):
                    nc.tensor.matmul(lg, d2a[:, c, jsl], ow_w[:, c, :],
                                     start=(c == 0), stop=False)
                nc.tensor.matmul(lg, d2b[:, jsl], ow_w[0:17, 6, :],
                                 start=False, stop=True)
                e10 = sm.tile([P, 10], F32, tag="e10")
                s10 = sm.tile([P, 1], F32, tag="s10")
                nc.scalar.activation(e10, lg, AF.Exp, accum_out=s10)
                r10 = sm.tile([P, 1], F32, tag="r10")
                nc.vector.reciprocal(r10, s10)
                o10 = sm.tile([P, 10], F32, tag="o10")
                nc.vector.tensor_scalar_mul(o10, e10, r10)
                nc.sync.dma_start(
                    out=out_d[ds(g * (nsub * P) + (2 * wave + j) * P, P), :],
                    in_=o10
                )

        def body(g, preloaded=None):
            nw = nsub // 2
            h0s, cs = [], [None] * nw
            vts = []
            mtk = {}  # (wave, k) -> Mt tile
            # k=0 proj+dots interleaved per tile so the first tile's chain
            # races ahead of later tiles' loads
            for j in range(nsub):
                w = j // 2
                if j % 2 == 0:
                    mtk[(w, 0)] = mtp.tile([P, 8, T], F32, tag=f"mtk{w}",
                                           bufs=3, name=f"mt0w{w}")
                h0 = load_dma(g, j) if preloaded is None else preloaded[j]
                h0s.append(h0)
                vt1, vt2 = prep_tile(j, h0)
                vts.append((vt1, vt2))
                pk = proj(j, 0, vt1, vt2)
                dots(j, 0, mtk[(w, 0)], pk)
            # waves are staggered one k apart: early waves finish their
            # chains (and start decoding) while late waves still compute
            LAG = 3
            for step in range(1, 8 + LAG * (nw - 1) + 1):
                for w in range(nw):
                    k = step - LAG * w
                    if 1 <= k <= 7:
                        mtk[(w, k)] = mtp.tile([P, 8, T], F32, tag=f"mtk{w}",
                                               bufs=3, name=f"mt{k}w{w}")
                        for j in (2 * w, 2 * w + 1):
                            pk = proj(j, k, *vts[j])
                            dots(j, k, mtk[(w, k)], pk)
                        cs[w] = serial_phase(k - 1, w, mtk[(w, k - 1)], cs[w])
                    elif k == 8:
                        cs[w] = serial_phase(7, w, mtk[(w, 7)], cs[w])
                        wsp = [recon(2 * w, h0s[2 * w], cs[w]),
                               recon(2 * w + 1, h0s[2 * w + 1], cs[w])]
                        decoder(wsp, g, w)

        # group 0's x DMAs first so they precede the 6MB of decoder weights
        # on the sync queue; weights stream in during attention
        pre0 = [load_dma(0, j) for j in range(nsub)]
        nc.sync.dma_start(out=d1_w, in_=d1_d[:, :, :])
        nc.sync.dma_start(out=d2_w, in_=d2_d[:, :, :])
        nc.sync.dma_start(out=ow_w, in_=ow_d[:, :, :])
        body(0, preloaded=pre0)
        if ngroups > 1:
            with tc.For_i(1, ngroups, 1) as g:
                body(g)
        for _pool in (pp, wkd, wp, sm, mtp, scp, pkp, vp, hp, consts):
            _pool.release()

    nc.compile()
    return nc


def pack_weights(W1, b1, W2, b2, W3, b3, Wd1, bd1, Wd2, bd2, Wo, bo):
    f64 = np.float64
    W1, b1, W2, b2, W3, b3 = (np.asarray(t, f64) for t in (W1, b1, W2, b2, W3, b3))
    G = W1.T @ W2
    a = W2.T @ b1

    A = np.eye(FV)
    m = np.zeros(FV)
    pw = np.zeros((P, 2, PTOT), np.float32)
    for k in range(8):
        Gk = A.T @ G @ A
        ak = A.T @ (G.T @ m + a)
        nco = NCOLS[k]
        Wk = np.zeros((197, nco), f64)
        r = RANKS[k]
        r1 = r + 1
        U, S, Vh = np.linalg.svd(Gk)
        Wk[:FV, :r] = (np.diag(S[:r]) @ Vh[:r]).T
        Wk[:FV, r] = ak
        Wk[:FV, r1 : r1 + r] = U[:, :r]
        Wk[FV, r1 + r] = 1.0
        off = POFF[k]
        pw[:, 0, off : off + nco] = Wk[0:128]
        pw[0:69, 1, off : off + nco] = Wk[128:197]
        A = W3 @ A
        m = W3 @ m + b3
    A8, m8 = A, m

    # fold W3^8 / m8 into the first decoder layer
    BD = np.zeros((FEAT, FEAT), f64)
    mm = np.zeros(FEAT, f64)
    for t in range(T):
        BD[t * FV : (t + 1) * FV, t * FV : (t + 1) * FV] = A8
        mm[t * FV : (t + 1) * FV] = m8
    Wd1f = np.asarray(Wd1, f64) @ BD
    bd1f = np.asarray(bd1, f64) + np.asarray(Wd1, f64) @ mm

    d1 = np.zeros((P, 8, FEAT), np.float32)
    W1T = Wd1f.T  # [784 f_in, 784 j]
    for t in range(T):
        d1[:, t, :] = W1T[t * FV : t * FV + P, :]
        d1[0:68, 4 + t, :] = W1T[t * FV + P : (t + 1) * FV, :]
    d1[68, 4, :] = bd1f

    d2 = np.zeros((P, 7, FEAT), np.float32)
    W2T = np.asarray(Wd2, f64).T
    for cidx in range(6):
        d2[:, cidx, :] = W2T[cidx * P : (cidx + 1) * P, :]
    d2[0:16, 6, :] = W2T[768:784, :]
    d2[16, 6, :] = np.asarray(bd2, f64)

    ow = np.zeros((P, 7, 10), np.float32)
    WoT = np.asarray(Wo, f64).T
    for cidx in range(6):
        ow[:, cidx, :] = WoT[cidx * P : (cidx + 1) * P, :]
    ow[0:16, 6, :] = WoT[768:784, :]
    ow[16, 6, :] = np.asarray(bo, f64)
    return pw.astype(ml_dtypes.bfloat16), d1, d2, ow


_NC_CACHE = {}


def kernel(**inputs):
    x = np.ascontiguousarray(np.asarray(inputs["x"], np.float32))
    zu, d1, d2, ow = pack_weights(
        inputs["W1"], inputs["b1"], inputs["W2"], inputs["b2"], inputs["W3"],
        inputs["b3"], inputs["Wd1"], inputs["bd1"], inputs["Wd2"],
        inputs["bd2"], inputs["Wo"], inputs["bo"],
    )
    if "nc" not in _NC_CACHE:
        _NC_CACHE["nc"] = build(NSUB, BPC // (NSUB * P))
    nc = _NC_CACHE["nc"]
    bpc = B // NCORES
    in_maps = [
        {
            "x": x[c * bpc : (c + 1) * bpc],
            "zu_w": zu,
            "dec1_w": d1,
            "dec2_w": d2,
            "out_w": ow,
        }
        for c in range(NCORES)
    ]
    res = run_bass_kernel_spmd(nc, in_maps, core_ids=list(range(NCORES)))
    return np.concatenate([res.results[c]["out"] for c in range(NCORES)], axis=0)
